# revision 1
# baseline (speedup 1.0000x reference)
"""Trainium2 Bass kernel for nn_DriftScene_88270167868070.

Contract: kernel(**inputs) takes FULL unsharded inputs (as produced by
setup_inputs()) and returns the FULL output (a scalar np.float32).

Strategy (8 NeuronCores, one SPMD launch):
  - Data-parallel transformer generator over the batch (64 scenes/core),
    fp32 matmuls (precision required: the final loss is dominated by fp32
    rounding of xf + V, so xf must be fp32-accurate; bf16/tf32 generators
    fail by 5-7e-2 relative).
  - Activations resident in transposed layout X_T [d_model on partitions,
    tokens on free]; weights pre-transposed on host.
  - Matching stage (cdist + double softmax + V) row-sharded, bf16 matmuls;
    one packed AllGather shares xf (both layouts) + ||xf||^2, one AllReduce
    shares column-softmax sums.
  - loss = mean((xf - fl32(xf + V))^2) with explicit fp32 rounding.
"""

import numpy as np
from contextlib import ExitStack

import concourse.bass as bass
import concourse.tile as tile
from concourse import bacc, mybir
from concourse.bass_utils import run_bass_kernel_spmd
from concourse.masks import make_identity
import ml_dtypes

F32 = mybir.dt.float32
BF16 = mybir.dt.bfloat16
AF = mybir.ActivationFunctionType
ALU = mybir.AluOpType
AX = mybir.AxisListType

# Problem dims (hardcoded per contract)
B, L, CH = 512, 32, 128
D, HEADS, DEPTH, FF = 512, 8, 4, 2048
DH = D // HEADS
LN_EPS = 1e-5
NC_ = 8                 # cores
SC = B // NC_           # 64 scenes per core
T = SC * L              # 2048 tokens per core
TB = 256                # tokens per t-block
NB = T // TB            # 8 t-blocks
NS = TB // 128          # 2 subtiles per block
KD = D // 128           # 4 d-tiles
KF = FF // 128          # 16 ff-tiles
FDIM = L * CH           # 4096 flattened feature dim
KFl = FDIM // 128       # 32 f-tiles
M_SHIFT = -20.0         # global shift for column softmax stabilization

# packed AllGather layout (bf16 element offsets)
AG_XFT = 0                      # xf_T   [4096, 64]
AG_XFN = FDIM * SC              # xf_nat [64, 4096]
AG_XN = 2 * FDIM * SC           # xn bits: f32 [64,1] viewed as bf16 [64,2]
AG_SZ = 2 * FDIM * SC + 2 * SC  # 524416


def _build_nc():
    nc = bacc.Bacc("TRN2", target_bir_lowering=False, debug=False, num_devices=NC_)

    # ---------------- I/O ----------------
    def inp(name, shape, dt=F32):
        return nc.dram_tensor(name, shape, dt, kind="ExternalInput").ap()

    epsT = inp("epsT", [128, T])              # eps shard, [ch, tok]
    inwT = inp("inwT", [128, D])              # in_w.T
    inb = inp("inb", [D])
    wqkvT = inp("wqkvT", [DEPTH, D, 3 * D])   # Wqkv[i].T
    bqkv = inp("bqkv", [DEPTH, 3 * D])
    woT = inp("woT", [DEPTH, D, D])
    bo = inp("bo", [DEPTH, D])
    ln1g = inp("ln1g", [DEPTH, D])
    ln1b = inp("ln1b", [DEPTH, D])
    w1T = inp("w1T", [DEPTH, D, FF])
    b1 = inp("b1", [DEPTH, FF])
    w2T = inp("w2T", [DEPTH, FF, D])
    b2 = inp("b2", [DEPTH, D])
    ln2g = inp("ln2g", [DEPTH, D])
    ln2b = inp("ln2b", [DEPTH, D])
    outwT = inp("outwT", [D, CH])             # out_w.T
    outb = inp("outb", [CH])
    pT = inp("pT", [FDIM, B], BF16)           # sample_p transposed [f, scene]
    pnat = inp("pnat", [B, FDIM], BF16)       # sample_p natural
    pn_bc = inp("pn_bc", [SC, B])             # ||p_j||^2 broadcast rows
    attn_mask = inp("attn_mask", [128, 128])  # 4-scene block-diag 0/1
    negdiag = inp("negdiag", [SC, B])         # 1e6 at (i, SC*core + i)

    loss_part = nc.dram_tensor("loss_part", [1, 1], F32, kind="ExternalOutput").ap()

    # ---------------- DRAM scratch ----------------
    ag_in = nc.dram_tensor("ag_in", [AG_SZ], BF16).ap()
    ag_out = nc.dram_tensor("ag_out", [NC_ * AG_SZ], BF16, addr_space="Shared").ap()
    xf32_d = nc.dram_tensor("xf32_d", [SC, FDIM], F32).ap()
    ar_in = nc.dram_tensor("ar_in", [1, 2 * B], F32).ap()
    ar_out = nc.dram_tensor("ar_out", [1, 2 * B], F32, addr_space="Shared").ap()

    with tile.TileContext(nc) as tc, ExitStack() as ctx:
        # ---------------- pools (bufs is PER TAG) ----------------
        const = ctx.enter_context(tc.tile_pool(name="const", bufs=1))
        xTp = ctx.enter_context(tc.tile_pool(name="xT", bufs=1))
        hp = ctx.enter_context(tc.tile_pool(name="h", bufs=3))
        sqp = ctx.enter_context(tc.tile_pool(name="sq", bufs=4))
        rowp = ctx.enter_context(tc.tile_pool(name="rows", bufs=3))
        mrow = ctx.enter_context(tc.tile_pool(name="mrow", bufs=1))
        bcp = ctx.enter_context(tc.tile_pool(name="bc", bufs=3))
        mbcp = ctx.enter_context(tc.tile_pool(name="mbc", bufs=1))
        bw_p = ctx.enter_context(tc.tile_pool(name="bigw", bufs=KD))
        wo_p = ctx.enter_context(tc.tile_pool(name="wo", bufs=KD))
        w2_p = ctx.enter_context(tc.tile_pool(name="w2", bufs=16))
        colp = ctx.enter_context(tc.tile_pool(name="colp", bufs=1))
        col2p = ctx.enter_context(tc.tile_pool(name="col2p", bufs=2))
        qk_p = ctx.enter_context(tc.tile_pool(name="qk", bufs=12))
        v65_p = ctx.enter_context(tc.tile_pool(name="v65", bufs=3))
        e_p = ctx.enter_context(tc.tile_pool(name="et", bufs=2))
        onat_p = ctx.enter_context(tc.tile_pool(name="onat", bufs=3))
        oT_p = ctx.enter_context(tc.tile_pool(name="oT", bufs=4))
        relu_p = ctx.enter_context(tc.tile_pool(name="relu", bufs=4))
        outp = ctx.enter_context(tc.tile_pool(name="outp", bufs=1))
        mtch = ctx.enter_context(tc.tile_pool(name="mtch", bufs=1))
        m2p = ctx.enter_context(tc.tile_pool(name="m2p", bufs=2))
        mov_p = ctx.enter_context(tc.tile_pool(name="mov", bufs=4))
        wT_p = ctx.enter_context(tc.tile_pool(name="wTp", bufs=8))

        ps_mm = ctx.enter_context(tc.tile_pool(name="ps_mm", bufs=2, space="PSUM"))
        ps_acc = ctx.enter_context(tc.tile_pool(name="ps_acc", bufs=4, space="PSUM"))
        ps_at = ctx.enter_context(tc.tile_pool(name="ps_at", bufs=2, space="PSUM"))

        # ---------------- constants ----------------
        ident = const.tile([128, 128], F32)
        make_identity(nc, ident[:])
        ones_col = const.tile([128, 1], F32)
        nc.vector.memset(ones_col[:], 1.0)
        mask_t = const.tile([128, 128], F32)
        nc.sync.dma_start(mask_t[:], attn_mask)
        pn_t = const.tile([SC, B], F32)
        nc.sync.dma_start(pn_t[:], pn_bc)
        nd_t = const.tile([SC, B], F32)
        nc.sync.dma_start(nd_t[:], negdiag)
        eps_col = const.tile([1, 1], F32)
        nc.vector.memset(eps_col[:], LN_EPS)
        m20_col = const.tile([SC, 1], F32)
        nc.vector.memset(m20_col[:], -M_SHIFT)

        # residual stream X_T: KD tiles [128, T] fp32, persistent
        xT = [xTp.tile([128, T], F32, tag=f"xT{k}", name=f"xT{k}") for k in range(KD)]

        def col(ap_1d, base, tag, pool=colp, n=128):
            t = pool.tile([n, 1], F32, tag=tag, name=tag)
            nc.sync.dma_start(t[:], ap_1d[base:base + n])
            return t

        # ========= input projection: X_T = (eps @ in_w.T).T =========
        inw_sb = bw_p.tile([128, FF], F32, tag="bigw", name="inw")
        nc.sync.dma_start(inw_sb[:, 0:D], inwT)
        ib_cols = [col(inb, k * 128, f"bo{k}") for k in range(KD)]
        for b_ in range(NB):
            bsl = slice(b_ * TB, (b_ + 1) * TB)
            eps_blk = sqp.tile([128, TB], F32, tag="sq", name="epsblk")
            nc.sync.dma_start(eps_blk[:], epsT[:, bsl])
            for dt_ in range(KD):
                ps = ps_mm.tile([128, TB], F32, tag="mm", name="ps")
                nc.tensor.matmul(ps[:], inw_sb[:, dt_ * 128:(dt_ + 1) * 128],
                                 eps_blk[:], start=True, stop=True)
                nc.scalar.activation(xT[dt_][:, bsl], ps[:],
                                     AF.Identity, bias=ib_cols[dt_][:], scale=1.0)

        # ========= per-block layernorm =========
        def ln_block(b_, g_cols, b_cols, htag):
            """LN over partition dim for tokens of block b_ -> h tiles."""
            bsl = slice(b_ * TB, (b_ + 1) * TB)
            s_row = rowp.tile([1, TB], F32, tag="srow", name="srow")
            ps_s = ps_mm.tile([1, TB], F32, tag="mm", name="ps")
            for k in range(KD):
                nc.tensor.matmul(ps_s[:], ones_col[:], xT[k][:, bsl],
                                 start=(k == 0), stop=(k == KD - 1))
            nc.vector.tensor_scalar_mul(s_row[:], ps_s[:], -1.0 / D)   # -mean
            q_row = rowp.tile([1, TB], F32, tag="qrow", name="qrow")
            ps_q = ps_mm.tile([1, TB], F32, tag="mm", name="ps")
            for k in range(KD):
                sq = sqp.tile([128, TB], F32, tag="sq", name="sq")
                nc.vector.tensor_mul(sq[:], xT[k][:, bsl], xT[k][:, bsl])
                nc.tensor.matmul(ps_q[:], ones_col[:], sq[:],
                                 start=(k == 0), stop=(k == KD - 1))
            msq = rowp.tile([1, TB], F32, tag="msq", name="msq")
            nc.vector.tensor_mul(msq[:], s_row[:], s_row[:])
            # var = q/D - m^2  (into q_row)
            nc.vector.scalar_tensor_tensor(q_row[:], ps_q[:], 1.0 / D, msq[:],
                                           op0=ALU.mult, op1=ALU.subtract)
            # rstd = 1/sqrt(var + eps): sqrt into msq, recip into q_row
            nc.scalar.activation(msq[:], q_row[:], AF.Sqrt, bias=eps_col[:], scale=1.0)
            nc.vector.reciprocal(q_row[:], msq[:])
            # shift = -m * rstd (into s_row)
            nc.vector.tensor_mul(s_row[:], s_row[:], q_row[:])
            rstd_bc = bcp.tile([128, TB], F32, tag="rstd_bc", name="rstdbc")
            nc.gpsimd.partition_broadcast(rstd_bc[:], q_row[:])
            shift_bc = bcp.tile([128, TB], F32, tag="shift_bc", name="shiftbc")
            nc.gpsimd.partition_broadcast(shift_bc[:], s_row[:])
            hs = []
            for k in range(KD):
                hh = hp.tile([128, TB], F32, tag=f"{htag}{k}", name=f"{htag}{k}")
                nc.vector.tensor_mul(hh[:], xT[k][:, bsl], rstd_bc[:])
                nc.vector.tensor_add(hh[:], hh[:], shift_bc[:])
                nc.scalar.activation(hh[:], hh[:], AF.Identity,
                                     bias=b_cols[k][:], scale=g_cols[k][:])
                hs.append(hh)
            return hs

        # ========= transformer layers =========
        for li in range(DEPTH):
            # ---- attention phase ----
            g1c = [col(ln1g[li], k * 128, f"lng{k}") for k in range(KD)]
            lb1c = [col(ln1b[li], k * 128, f"lnb{k}") for k in range(KD)]
            wq_sl = []
            for k in range(KD):
                w = bw_p.tile([128, FF], F32, tag="bigw", name="wqs")
                nc.sync.dma_start(w[:, 0:3 * D], wqkvT[li, k * 128:(k + 1) * 128, :])
                wq_sl.append(w)
            wo_sl = []
            for k in range(KD):
                w = wo_p.tile([128, D], F32, tag="wo", name="wos")
                nc.sync.dma_start(w[:], woT[li, k * 128:(k + 1) * 128, :])
                wo_sl.append(w)
            bq_cols = [col(bqkv[li], ot * 128, f"bq{ot}") for ot in range(8)]
            bv_bc = bcp.tile([128, D], F32, tag="bvbc", name="bvbc", bufs=1)
            nc.gpsimd.dma_start(bv_bc[:], bass.AP(
                tensor=bqkv.tensor, offset=bqkv.offset + li * 3 * D + 2 * D,
                ap=[[0, 128], [1, D]]))
            bo_cols = [col(bo[li], ot * 128, f"bo{ot}") for ot in range(KD)]

            for b_ in range(NB):
                tsl = slice(b_ * TB, (b_ + 1) * TB)
                h = ln_block(b_, g1c, lb1c, "h")
                # Q,K projections (transposed out)
                qk = []
                for ot in range(8):
                    ps = ps_mm.tile([128, TB], F32, tag="mm", name="ps")
                    for k in range(KD):
                        nc.tensor.matmul(ps[:], wq_sl[k][:, ot * 128:(ot + 1) * 128],
                                         h[k][:], start=(k == 0), stop=(k == KD - 1))
                    t = qk_p.tile([128, TB], F32, tag="qk", name="qk")
                    nc.scalar.activation(t[:], ps[:], AF.Identity,
                                         bias=bq_cols[ot][:], scale=1.0)
                    qk.append(t)
                onats = []
                for tt in range(NS):
                    ssl = slice(tt * 128, (tt + 1) * 128)
                    # V natural for this subtile, 65-strided with ones column
                    ps = ps_mm.tile([128, D], F32, tag="mm", name="ps")
                    for k in range(KD):
                        nc.tensor.matmul(ps[:], h[k][:, ssl], wq_sl[k][:, 2 * D:3 * D],
                                         start=(k == 0), stop=(k == KD - 1))
                    v = v65_p.tile([128, 8 * 65], F32, tag="v65", name="v65")
                    nc.vector.memset(
                        v[:].rearrange("p (hh c) -> p hh c", hh=8)[:, :, 64:65], 1.0)
                    for hh in range(8):
                        nc.vector.tensor_add(v[:, hh * 65:hh * 65 + 64],
                                             ps[:, hh * 64:(hh + 1) * 64],
                                             bv_bc[:, hh * 64:(hh + 1) * 64])
                    # attention
                    onat = onat_p.tile([128, D], F32, tag="onat", name="onat")
                    for hh in range(8):
                        bp = (hh % 2) * 64
                        kt = qk[4 + hh // 2]
                        qt = qk[hh // 2]
                        s_ps = ps_at.tile([128, 128], F32, tag="at", name="sps")
                        nc.tensor.matmul(s_ps[:], kt[bp:bp + 64, ssl], qt[bp:bp + 64, ssl],
                                         start=True, stop=True)
                        et = e_p.tile([128, 128], F32, tag="et", name="et")
                        nc.scalar.activation(et[:], s_ps[:], AF.Exp, bias=0.0, scale=0.125)
                        nc.vector.tensor_mul(et[:], et[:], mask_t[:])
                        o_ps = ps_at.tile([128, 65], F32, tag="at", name="ops")
                        nc.tensor.matmul(o_ps[:], et[:], v[:, hh * 65:(hh + 1) * 65],
                                         start=True, stop=True)
                        rcol = col2p.tile([128, 1], F32, tag="rcol", name="rcol")
                        nc.vector.reciprocal(rcol[:], o_ps[:, 64:65])
                        nc.vector.tensor_scalar_mul(onat[:, hh * 64:(hh + 1) * 64],
                                                    o_ps[:, 0:64], rcol[:])
                    onats.append(onat)
                # transpose O -> O_T
                oT = [oT_p.tile([128, TB], F32, tag="oT", name="oT") for _ in range(KD)]
                for tt in range(NS):
                    for k in range(KD):
                        tp = ps_at.tile([128, 128], F32, tag="at", name="tp")
                        nc.tensor.transpose(tp[:], onats[tt][:, k * 128:(k + 1) * 128],
                                            ident[:])
                        nc.vector.tensor_copy(oT[k][:, tt * 128:(tt + 1) * 128], tp[:])
                # Wo + residual
                for ot in range(KD):
                    ps = ps_mm.tile([128, TB], F32, tag="mm", name="ps")
                    for k in range(KD):
                        nc.tensor.matmul(ps[:], wo_sl[k][:, ot * 128:(ot + 1) * 128],
                                         oT[k][:], start=(k == 0), stop=(k == KD - 1))
                    nc.vector.scalar_tensor_tensor(xT[ot][:, tsl], ps[:], bo_cols[ot][:],
                                                   xT[ot][:, tsl], op0=ALU.add, op1=ALU.add)

            # ---- FF phase ----
            g2c = [col(ln2g[li], k * 128, f"lng{k}") for k in range(KD)]
            lb2c = [col(ln2b[li], k * 128, f"lnb{k}") for k in range(KD)]
            w1_sl = []
            for k in range(KD):
                w = bw_p.tile([128, FF], F32, tag="bigw", name="w1s")
                nc.sync.dma_start(w[:], w1T[li, k * 128:(k + 1) * 128, :])
                w1_sl.append(w)
            w2_sl = []
            for kf in range(KF):
                w = w2_p.tile([128, D], F32, tag="w2", name="w2s")
                nc.sync.dma_start(w[:], w2T[li, kf * 128:(kf + 1) * 128, :])
                w2_sl.append(w)
            bff_cols = [col(b2[li], ot * 128, f"bo{ot}") for ot in range(KD)]
            for b_ in range(NB):
                tsl = slice(b_ * TB, (b_ + 1) * TB)
                h2 = ln_block(b_, g2c, lb2c, "h")
                acc = [ps_acc.tile([128, TB], F32, tag="acc", name="facc")[:]
                       for _ in range(KD)]
                for kf in range(KF):
                    ps = ps_mm.tile([128, TB], F32, tag="mm", name="ps")
                    for k in range(KD):
                        nc.tensor.matmul(ps[:], w1_sl[k][:, kf * 128:(kf + 1) * 128],
                                         h2[k][:], start=(k == 0), stop=(k == KD - 1))
                    b1col = col(b1[li], kf * 128, "b1c", pool=col2p)
                    rl = relu_p.tile([128, TB], F32, tag="relu", name="rl")
                    nc.scalar.activation(rl[:], ps[:], AF.Relu, bias=b1col[:], scale=1.0)
                    for ot in range(KD):
                        nc.tensor.matmul(acc[ot], w2_sl[kf][:, ot * 128:(ot + 1) * 128],
                                         rl[:], start=(kf == 0), stop=(kf == KF - 1))
                for ot in range(KD):
                    nc.vector.scalar_tensor_tensor(xT[ot][:, tsl], acc[ot],
                                                   bff_cols[ot][:], xT[ot][:, tsl],
                                                   op0=ALU.add, op1=ALU.add)

        # ========= output projection =========
        outw_sb = bw_p.tile([128, FF], F32, tag="bigw", name="outw")
        for k in range(KD):
            nc.sync.dma_start(outw_sb[:, k * 128:k * 128 + CH],
                              outwT[k * 128:(k + 1) * 128, :])
        outb_col = col(outb, 0, "outbcol")
        outb_bc = mbcp.tile([128, CH], F32, tag="outbbc", name="outbbc")
        nc.gpsimd.dma_start(outb_bc[:], bass.AP(
            tensor=outb.tensor, offset=outb.offset, ap=[[0, 128], [1, CH]]))

        # y_T [ch, tok] in bf16 (stationary source for matching matmuls)
        yT_bf = outp.tile([128, T], BF16, tag="yTbf")
        for b_ in range(NB):
            ps = ps_mm.tile([128, TB], F32, tag="mm", name="ps")
            for k in range(KD):
                nc.tensor.matmul(ps[:], outw_sb[:, k * 128:k * 128 + CH],
                                 xT[k][:, b_ * TB:(b_ + 1) * TB],
                                 start=(k == 0), stop=(k == KD - 1))
            nc.scalar.activation(yT_bf[:, b_ * TB:(b_ + 1) * TB], ps[:], AF.Identity,
                                 bias=outb_col[:], scale=1.0)

        # y natural -> xf32_d DRAM fp32; bf16 shards straight into ag_in
        for tt in range(T // 128):
            ps = ps_at.tile([128, CH], F32, tag="at", name="yn_ps")
            for k in range(KD):
                nc.tensor.matmul(ps[:], xT[k][:, tt * 128:(tt + 1) * 128],
                                 outw_sb[:, k * 128:k * 128 + CH],
                                 start=(k == 0), stop=(k == KD - 1))
            yn = m2p.tile([128, CH], F32, tag="yn", name="yn")
            nc.vector.tensor_add(yn[:], ps[:], outb_bc[:])
            nc.sync.dma_start(
                xf32_d[tt * 4:(tt + 1) * 4, :].rearrange("p (l c) -> p l c", l=L),
                yn[:])
            ynbf = m2p.tile([128, CH], BF16, tag="ynbf", name="ynbf")
            nc.vector.tensor_copy(ynbf[:], yn[:])
            nc.sync.dma_start(
                ag_in[AG_XFN + tt * 4 * FDIM: AG_XFN + (tt + 1) * 4 * FDIM]
                .rearrange("(i l c) -> i l c", l=L, c=CH),
                ynbf[:])

        # xn = ||xf_i||^2 via gram diag (bf16 inputs, fp32 accum)
        xfT_st = yT_bf[:].rearrange("c (i l) -> c l i", l=L)   # [128, 32, 64]
        gram = ps_at.tile([SC, SC], F32, tag="at", name="gram")
        for l in range(KFl):
            nc.tensor.matmul(gram[:], xfT_st[:, l, :], xfT_st[:, l, :],
                             start=(l == 0), stop=(l == KFl - 1))
        gd = m2p.tile([SC, SC], F32, tag="gd", name="gd")
        nc.vector.tensor_mul(gd[:], gram[:], ident[0:SC, 0:SC])
        xn_col = colp.tile([SC, 1], F32, tag="xncol", name="xncol")
        nc.vector.reduce_sum(xn_col[:], gd[:], axis=AX.X)

        # write AG input: xf_T + xn bits (xf_nat already streamed above)
        for l in range(L):
            nc.sync.dma_start(
                ag_in[AG_XFT + l * 128 * SC: AG_XFT + (l + 1) * 128 * SC]
                .rearrange("(c i) -> c i", c=128),
                xfT_st[:, l, :])
        nc.sync.dma_start(
            ag_in[AG_XN:AG_XN + 2 * SC].rearrange("(i bb) -> i bb", bb=2),
            xn_col[:].bitcast(BF16))
        nc.gpsimd.collective_compute(
            "AllGather", ALU.bypass, replica_groups=[list(range(NC_))],
            ins=[ag_in[:]], outs=[ag_out[:]])

        # xn_full row [1, 512] f32 + broadcast
        ago_f32 = ag_out.bitcast(F32)
        xn_row = mrow.tile([1, B], F32, tag="mr", name="xnrow")
        nc.sync.dma_start(
            xn_row[:],
            bass.AP(tensor=ago_f32.tensor, offset=ago_f32.offset + AG_XN // 2,
                    ap=[[1, 1], [AG_SZ // 2, NC_], [1, SC]]))
        xn_bc = mbcp.tile([SC, B], F32, tag="mbc", name="xnbc")
        nc.gpsimd.partition_broadcast(xn_bc[:], xn_row[:])

        # S_pos / S_neg (bf16 matmuls, fp32 accum)
        spos = ps_acc.tile([SC, B], F32, tag="acc", name="spos")
        for l in range(KFl):
            mv = mov_p.tile([128, B], BF16, tag="mv", name="mv")
            nc.sync.dma_start(mv[:], pT[l * 128:(l + 1) * 128, :])
            nc.tensor.matmul(spos[:], xfT_st[:, l, :], mv[:],
                             start=(l == 0), stop=(l == KFl - 1))
        sneg = ps_acc.tile([SC, B], F32, tag="acc", name="sneg")
        for l in range(KFl):
            mv = mov_p.tile([128, B], BF16, tag="mv", name="mv")
            nc.sync.dma_start(
                mv[:],
                bass.AP(tensor=ag_out.tensor, offset=ag_out.offset + AG_XFT + l * 128 * SC,
                        ap=[[SC, 128], [AG_SZ, NC_], [1, SC]]))
            nc.tensor.matmul(sneg[:], xfT_st[:, l, :], mv[:],
                             start=(l == 0), stop=(l == KFl - 1))

        # distances -> logits -> E (in place)
        dist = mtch.tile([SC, 2 * B], F32, tag="dist")
        nc.vector.scalar_tensor_tensor(dist[:, 0:B], spos[:], -2.0, pn_t[:],
                                       op0=ALU.mult, op1=ALU.add)
        nc.vector.scalar_tensor_tensor(dist[:, B:2 * B], sneg[:], -2.0, xn_bc[:],
                                       op0=ALU.mult, op1=ALU.add)
        nc.vector.tensor_scalar_add(dist[:], dist[:], xn_col[:])
        nc.vector.tensor_scalar_max(dist[:], dist[:], 0.0)
        nc.scalar.activation(dist[:], dist[:], AF.Sqrt, bias=0.0, scale=1.0)
        nc.vector.tensor_add(dist[:, B:2 * B], dist[:, B:2 * B], nd_t[:])
        dmin = colp.tile([SC, 1], F32, tag="dmin", name="dmin")
        nc.vector.tensor_reduce(out=dmin[:], in_=dist[:], axis=AX.X, op=ALU.min)
        E = dist  # in place: E = exp(-d + dmin)
        nc.scalar.activation(E[:], dist[:], AF.Exp, bias=dmin[:], scale=-1.0)
        g_col = colp.tile([SC, 1], F32, tag="gcol", name="gcol")
        nc.scalar.activation(g_col[:], dmin[:], AF.Exp, bias=m20_col[:], scale=-1.0)
        sr_col = colp.tile([SC, 1], F32, tag="srcol", name="srcol")
        nc.vector.reduce_sum(sr_col[:], E[:], axis=AX.X)
        # partial colsums of G = E * g_i via g-weighted stationary
        cs_row = mrow.tile([1, 2 * B], F32, tag="mr", name="csrow")
        for b_ in range(2):
            ps = ps_mm.tile([1, B], F32, tag="mm", name="ps")
            nc.tensor.matmul(ps[:], g_col[:], E[:, b_ * B:(b_ + 1) * B],
                             start=True, stop=True)
            nc.vector.tensor_copy(cs_row[:, b_ * B:(b_ + 1) * B], ps[:])
        nc.sync.dma_start(ar_in, cs_row[:])
        nc.gpsimd.collective_compute(
            "AllReduce", ALU.add, replica_groups=[list(range(NC_))],
            ins=[ar_in[:]], outs=[ar_out[:]])
        cs_g = mrow.tile([1, 2 * B], F32, tag="mr", name="csg")
        nc.sync.dma_start(cs_g[:], ar_out)
        cs_bc = mbcp.tile([SC, 2 * B], F32, tag="csbc", name="csbc")
        nc.gpsimd.partition_broadcast(cs_bc[:], cs_g[:])
        nc.scalar.activation(cs_bc[:], cs_bc[:], AF.Sqrt, bias=0.0, scale=1.0)
        nc.vector.reciprocal(cs_bc[:], cs_bc[:])
        # E' = E * invsqrt(Sc); row scalars BEFORE overwriting E with W
        nc.vector.tensor_mul(E[:], E[:], cs_bc[:])
        snp = colp.tile([SC, 1], F32, tag="snp", name="snp")
        nc.vector.reduce_sum(snp[:], E[:, B:2 * B], axis=AX.X)
        spp = colp.tile([SC, 1], F32, tag="spp", name="spp")
        nc.vector.reduce_sum(spp[:], E[:, 0:B], axis=AX.X)
        tcol = colp.tile([SC, 1], F32, tag="tcol", name="tcol")
        nc.vector.reciprocal(tcol[:], sr_col[:])
        nc.vector.tensor_mul(tcol[:], tcol[:], g_col[:])
        ccol = colp.tile([SC, 1], F32, tag="ccol", name="ccol")
        nc.scalar.activation(ccol[:], tcol[:], AF.Sqrt, bias=0.0, scale=1.0)
        alpha = colp.tile([SC, 1], F32, tag="alpha", name="alpha")
        nc.vector.tensor_mul(alpha[:], tcol[:], snp[:])
        beta = colp.tile([SC, 1], F32, tag="beta", name="beta")
        nc.vector.tensor_mul(beta[:], alpha[:], spp[:])
        nc.vector.tensor_mul(beta[:], beta[:], ccol[:])
        nc.vector.tensor_scalar_mul(beta[:], beta[:], -1.0)
        # W = E' * alpha / -beta (in place), transpose, cast bf16
        nc.vector.tensor_scalar_mul(E[:, 0:B], E[:, 0:B], alpha[:])
        nc.vector.tensor_scalar_mul(E[:, B:2 * B], E[:, B:2 * B], beta[:])
        wT = []
        for half in range(2):
            for jt in range(4):
                tp = ps_at.tile([128, SC], F32, tag="at", name="wtp")
                nc.tensor.transpose(
                    tp[:], E[:, half * B + jt * 128: half * B + (jt + 1) * 128],
                    ident[0:SC, 0:SC])
                t = wT_p.tile([128, SC], BF16, tag="wT", name="wT")
                nc.vector.tensor_copy(t[:], tp[:])
                wT.append(t)
        # V and loss: V = Wpos @ p - Wneg @ xf_full, r = xf - fl(xf + V)
        lacc = m2p.tile([SC, 16], F32, tag="lacc", name="lacc", bufs=1)
        FBW = 256
        for fb in range(FDIM // FBW):
            vps = ps_acc.tile([SC, FBW], F32, tag="acc", name="vps")
            for jt in range(4):
                mv = mov_p.tile([128, FBW], BF16, tag="mv", name="mv")
                nc.sync.dma_start(mv[:], pnat[jt * 128:(jt + 1) * 128,
                                              fb * FBW:(fb + 1) * FBW])
                nc.tensor.matmul(vps[:], wT[jt][:], mv[:], start=(jt == 0), stop=False)
            for jt in range(4):
                mv = mov_p.tile([128, FBW], BF16, tag="mv", name="mv")
                nc.sync.dma_start(
                    mv[:],
                    bass.AP(tensor=ag_out.tensor,
                            offset=ag_out.offset + AG_XFN + 2 * jt * AG_SZ + fb * FBW,
                            ap=[[AG_SZ, 2], [FDIM, SC], [1, FBW]]))
                nc.tensor.matmul(vps[:], wT[4 + jt][:], mv[:], start=False, stop=(jt == 3))
            xfb = m2p.tile([SC, FBW], F32, tag="xfb", name="xfb")
            nc.sync.dma_start(xfb[:], xf32_d[:, fb * FBW:(fb + 1) * FBW])
            t1 = m2p.tile([SC, FBW], F32, tag="t1", name="t1")
            nc.vector.tensor_add(t1[:], xfb[:], vps[:])
            nc.vector.tensor_sub(t1[:], xfb[:], t1[:])
            nc.vector.tensor_mul(t1[:], t1[:], t1[:])
            nc.vector.reduce_sum(lacc[:, fb:fb + 1], t1[:], axis=AX.X)
        lsum = colp.tile([SC, 1], F32, tag="lsum", name="lsum")
        nc.vector.reduce_sum(lsum[:], lacc[:], axis=AX.X)
        tot = ps_mm.tile([1, 1], F32, tag="mm", name="tot")
        nc.tensor.matmul(tot[:], ones_col[0:SC, :], lsum[:], start=True, stop=True)
        tot_sb = colp.tile([1, 1], F32, tag="tot", name="totsb")
        nc.vector.tensor_copy(tot_sb[:], tot[:])
        nc.sync.dma_start(loss_part, tot_sb[:])

    nc.compile()
    return nc


_NC_CACHE = None


def _get_nc():
    global _NC_CACHE
    if _NC_CACHE is None:
        _NC_CACHE = _build_nc()
    return _NC_CACHE


def _prep_inputs(inputs):
    f32 = lambda x: np.ascontiguousarray(np.asarray(x), dtype=np.float32)
    bf = lambda x: np.ascontiguousarray(np.asarray(x, dtype=ml_dtypes.bfloat16))
    sample_p = f32(inputs["sample_p"])
    eps = f32(inputs["eps"])
    p2 = sample_p.reshape(B, FDIM)
    pn = (p2.astype(np.float64) ** 2).sum(-1).astype(np.float32)

    common = {
        "inwT": f32(inputs["in_w"]).T.copy(),
        "inb": f32(inputs["in_b"]),
        "wqkvT": np.ascontiguousarray(f32(inputs["Wqkv"]).transpose(0, 2, 1)),
        "bqkv": f32(inputs["bqkv"]),
        "woT": np.ascontiguousarray(f32(inputs["Wo"]).transpose(0, 2, 1)),
        "bo": f32(inputs["bo"]),
        "ln1g": f32(inputs["ln1_g"]), "ln1b": f32(inputs["ln1_b"]),
        "w1T": np.ascontiguousarray(f32(inputs["W1"]).transpose(0, 2, 1)),
        "b1": f32(inputs["b1"]),
        "w2T": np.ascontiguousarray(f32(inputs["W2"]).transpose(0, 2, 1)),
        "b2": f32(inputs["b2"]),
        "ln2g": f32(inputs["ln2_g"]), "ln2b": f32(inputs["ln2_b"]),
        "outwT": f32(inputs["out_w"]).T.copy(),
        "outb": f32(inputs["out_b"]),
        "pT": bf(p2.T),
        "pnat": bf(p2),
        "pn_bc": np.broadcast_to(pn[None, :], (SC, B)).copy(),
        "attn_mask": np.kron(np.eye(4, dtype=np.float32), np.ones((32, 32), np.float32)),
    }
    in_maps = []
    for c in range(NC_):
        nd = np.zeros((SC, B), np.float32)
        nd[np.arange(SC), SC * c + np.arange(SC)] = 1e6
        m = dict(common)
        m["epsT"] = eps[c * SC:(c + 1) * SC].reshape(T, CH).T.copy()
        m["negdiag"] = nd
        in_maps.append(m)
    return in_maps


def kernel(**inputs) -> np.ndarray:
    nc = _get_nc()
    in_maps = _prep_inputs(inputs)
    res = run_bass_kernel_spmd(nc, in_maps, list(range(NC_)))
    total = sum(float(r["loss_part"][0, 0]) for r in res.results)
    return np.float32(total / (B * FDIM))



# revision 6
# speedup vs baseline: 1.4742x; 1.4742x over previous
"""Trainium2 Bass kernel for nn_DriftScene_88270167868070.

Contract: kernel(**inputs) takes FULL unsharded inputs (as produced by
setup_inputs()) and returns the FULL output (a scalar np.float32).

Strategy (8 NeuronCores, one SPMD launch):
  - Data-parallel transformer generator over the batch (64 scenes/core),
    fp32 matmuls (precision required: the final loss is dominated by fp32
    rounding of xf + V, so xf must be fp32-accurate; bf16/tf32 generators
    fail by 5-7e-2 relative).
  - Activations resident in transposed layout X_T [d_model on partitions,
    tokens on free]; weights pre-transposed on host.
  - Matching stage (cdist + double softmax + V) row-sharded, bf16 matmuls;
    one packed AllGather shares xf (both layouts) + ||xf||^2, one AllReduce
    shares column-softmax sums.
  - loss = mean((xf - fl32(xf + V))^2) with explicit fp32 rounding.
"""

import numpy as np
from contextlib import ExitStack

import concourse.bass as bass
import concourse.tile as tile
from concourse import bacc, mybir
from concourse.bass_utils import run_bass_kernel_spmd
from concourse.masks import make_identity
import ml_dtypes

F32 = mybir.dt.float32
BF16 = mybir.dt.bfloat16
AF = mybir.ActivationFunctionType
ALU = mybir.AluOpType
AX = mybir.AxisListType

# Problem dims (hardcoded per contract)
B, L, CH = 512, 32, 128
D, HEADS, DEPTH, FF = 512, 8, 4, 2048
DH = D // HEADS
LN_EPS = 1e-5
NC_ = 8                 # cores
SC = B // NC_           # 64 scenes per core
T = SC * L              # 2048 tokens per core
TB = 256                # tokens per t-block
NB = T // TB            # 8 t-blocks
NS = TB // 128          # 2 subtiles per block
KD = D // 128           # 4 d-tiles
KF = FF // 128          # 16 ff-tiles
FDIM = L * CH           # 4096 flattened feature dim
KFl = FDIM // 128       # 32 f-tiles
M_SHIFT = -20.0         # global shift for column softmax stabilization

# packed AllGather layout (bf16 element offsets)
AG_XFT = 0                      # xf_T   [4096, 64]
AG_XFN = FDIM * SC              # xf_nat [64, 4096]
AG_XN = 2 * FDIM * SC           # xn bits: f32 [64,1] viewed as bf16 [64,2]
AG_SZ = 2 * FDIM * SC + 2 * SC  # 524416


def _build_nc():
    nc = bacc.Bacc("TRN2", target_bir_lowering=False, debug=False, num_devices=NC_)

    # ---------------- I/O ----------------
    def inp(name, shape, dt=F32):
        return nc.dram_tensor(name, shape, dt, kind="ExternalInput").ap()

    epsT = inp("epsT", [128, T])              # eps shard, [ch, tok]
    inwT = inp("inwT", [128, D])              # in_w.T
    inb = inp("inb", [D])
    wqkvT = inp("wqkvT", [DEPTH, D, 3 * D])   # Wqkv[i].T
    bqkv = inp("bqkv", [DEPTH, 3 * D])
    woT = inp("woT", [DEPTH, D, D])
    bo = inp("bo", [DEPTH, D])
    ln1g = inp("ln1g", [DEPTH, D])
    ln1b = inp("ln1b", [DEPTH, D])
    w1T = inp("w1T", [DEPTH, D, FF])
    b1 = inp("b1", [DEPTH, FF])
    w2T = inp("w2T", [DEPTH, FF, D])
    b2 = inp("b2", [DEPTH, D])
    ln2g = inp("ln2g", [DEPTH, D])
    ln2b = inp("ln2b", [DEPTH, D])
    outwT = inp("outwT", [D, CH])             # out_w.T
    outb = inp("outb", [CH])
    pT = inp("pT", [FDIM, B], BF16)           # sample_p transposed [f, scene]
    pnat = inp("pnat", [B, FDIM], BF16)       # sample_p natural
    pn_bc = inp("pn_bc", [SC, B])             # ||p_j||^2 broadcast rows
    attn_mask = inp("attn_mask", [128, 128])  # 4-scene block-diag 0/1
    negdiag = inp("negdiag", [SC, B])         # 1e6 at (i, SC*core + i)

    loss_part = nc.dram_tensor("loss_part", [1, 1], F32, kind="ExternalOutput").ap()

    # ---------------- DRAM scratch ----------------
    ag_in = nc.dram_tensor("ag_in", [AG_SZ], BF16).ap()
    ag_out = nc.dram_tensor("ag_out", [NC_ * AG_SZ], BF16, addr_space="Shared").ap()
    xf32_d = nc.dram_tensor("xf32_d", [SC, FDIM], F32).ap()
    ar_in = nc.dram_tensor("ar_in", [1, 2 * B], F32).ap()
    ar_out = nc.dram_tensor("ar_out", [1, 2 * B], F32, addr_space="Shared").ap()

    with tile.TileContext(nc) as tc, ExitStack() as ctx:
        # ---------------- pools (bufs is PER TAG) ----------------
        const = ctx.enter_context(tc.tile_pool(name="const", bufs=1))
        xTp = ctx.enter_context(tc.tile_pool(name="xT", bufs=1))
        hp = ctx.enter_context(tc.tile_pool(name="h", bufs=3))
        sqp = ctx.enter_context(tc.tile_pool(name="sq", bufs=4))
        rowp = ctx.enter_context(tc.tile_pool(name="rows", bufs=3))
        mrow = ctx.enter_context(tc.tile_pool(name="mrow", bufs=1))
        bcp = ctx.enter_context(tc.tile_pool(name="bc", bufs=3))
        mbcp = ctx.enter_context(tc.tile_pool(name="mbc", bufs=1))
        bw_p = ctx.enter_context(tc.tile_pool(name="bigw", bufs=KD))
        wo_p = ctx.enter_context(tc.tile_pool(name="wo", bufs=KD))
        w2_p = ctx.enter_context(tc.tile_pool(name="w2", bufs=16))
        colp = ctx.enter_context(tc.tile_pool(name="colp", bufs=1))
        col2p = ctx.enter_context(tc.tile_pool(name="col2p", bufs=2))
        qk_p = ctx.enter_context(tc.tile_pool(name="qk", bufs=11))
        v65_p = ctx.enter_context(tc.tile_pool(name="v65", bufs=3))
        e_p = ctx.enter_context(tc.tile_pool(name="et", bufs=2))
        onat_p = ctx.enter_context(tc.tile_pool(name="onat", bufs=3))
        oT_p = ctx.enter_context(tc.tile_pool(name="oT", bufs=4))
        relu_p = ctx.enter_context(tc.tile_pool(name="relu", bufs=4))
        outp = ctx.enter_context(tc.tile_pool(name="outp", bufs=1))
        mtch = ctx.enter_context(tc.tile_pool(name="mtch", bufs=1))
        m2p = ctx.enter_context(tc.tile_pool(name="m2p", bufs=2))
        mov_p = ctx.enter_context(tc.tile_pool(name="mov", bufs=4))
        wT_p = ctx.enter_context(tc.tile_pool(name="wTp", bufs=8))

        ps_mm = ctx.enter_context(tc.tile_pool(name="ps_mm", bufs=2, space="PSUM"))
        ps_acc = ctx.enter_context(tc.tile_pool(name="ps_acc", bufs=4, space="PSUM"))
        ps_at = ctx.enter_context(tc.tile_pool(name="ps_at", bufs=2, space="PSUM"))

        # ---------------- constants ----------------
        ident = const.tile([128, 128], F32)
        make_identity(nc, ident[:])
        ones_col = const.tile([128, 1], F32)
        nc.vector.memset(ones_col[:], 1.0)
        mask_t = const.tile([128, 128], F32)
        nc.sync.dma_start(mask_t[:], attn_mask)
        pn_t = const.tile([SC, B], F32)
        nc.sync.dma_start(pn_t[:], pn_bc)
        nd_t = const.tile([SC, B], F32)
        nc.sync.dma_start(nd_t[:], negdiag)
        eps_col = const.tile([1, 1], F32)
        nc.vector.memset(eps_col[:], LN_EPS)
        m20_col = const.tile([SC, 1], F32)
        nc.vector.memset(m20_col[:], -M_SHIFT)

        # residual stream X_T: KD tiles [128, T] fp32, persistent
        xT = [xTp.tile([128, T], F32, tag=f"xT{k}", name=f"xT{k}") for k in range(KD)]

        def col(ap_1d, base, tag, pool=colp, n=128):
            t = pool.tile([n, 1], F32, tag=tag, name=tag)
            nc.sync.dma_start(t[:], ap_1d[base:base + n])
            return t

        # ========= input projection: X_T = (eps @ in_w.T).T =========
        inw_sb = bw_p.tile([128, FF], F32, tag="bigw", name="inw")
        nc.sync.dma_start(inw_sb[:, 0:D], inwT)
        ib_cols = [col(inb, k * 128, f"bo{k}") for k in range(KD)]
        for b_ in range(NB):
            bsl = slice(b_ * TB, (b_ + 1) * TB)
            eps_blk = sqp.tile([128, TB], F32, tag="sq", name="epsblk")
            nc.sync.dma_start(eps_blk[:], epsT[:, bsl])
            for dt_ in range(KD):
                ps = ps_mm.tile([128, TB], F32, tag="mm", name="ps")
                nc.tensor.matmul(ps[:], inw_sb[:, dt_ * 128:(dt_ + 1) * 128],
                                 eps_blk[:], start=True, stop=True)
                nc.scalar.activation(xT[dt_][:, bsl], ps[:],
                                     AF.Identity, bias=ib_cols[dt_][:], scale=1.0)

        # ========= per-block layernorm =========
        def ln_block(b_, g_cols, b_cols, htag):
            """LN over partition dim for tokens of block b_ -> h tiles."""
            bsl = slice(b_ * TB, (b_ + 1) * TB)
            s_row = rowp.tile([1, TB], F32, tag="srow", name="srow")
            ps_s = ps_mm.tile([1, TB], F32, tag="mm", name="ps")
            for k in range(KD):
                nc.tensor.matmul(ps_s[:], ones_col[:], xT[k][:, bsl],
                                 start=(k == 0), stop=(k == KD - 1))
            nc.vector.tensor_scalar_mul(s_row[:], ps_s[:], -1.0 / D)   # -mean
            q_row = rowp.tile([1, TB], F32, tag="qrow", name="qrow")
            ps_q = ps_mm.tile([1, TB], F32, tag="mm", name="ps")
            for k in range(KD):
                sq = sqp.tile([128, TB], F32, tag="sq", name="sq")
                nc.vector.tensor_mul(sq[:], xT[k][:, bsl], xT[k][:, bsl])
                nc.tensor.matmul(ps_q[:], ones_col[:], sq[:],
                                 start=(k == 0), stop=(k == KD - 1))
            msq = rowp.tile([1, TB], F32, tag="msq", name="msq")
            nc.vector.tensor_mul(msq[:], s_row[:], s_row[:])
            # var = q/D - m^2  (into q_row)
            nc.vector.scalar_tensor_tensor(q_row[:], ps_q[:], 1.0 / D, msq[:],
                                           op0=ALU.mult, op1=ALU.subtract)
            # rstd = 1/sqrt(var + eps): sqrt into msq, recip into q_row
            nc.scalar.activation(msq[:], q_row[:], AF.Sqrt, bias=eps_col[:], scale=1.0)
            nc.vector.reciprocal(q_row[:], msq[:])
            # shift = -m * rstd (into s_row)
            nc.vector.tensor_mul(s_row[:], s_row[:], q_row[:])
            rstd_bc = bcp.tile([128, TB], F32, tag="rstd_bc", name="rstdbc")
            nc.gpsimd.partition_broadcast(rstd_bc[:], q_row[:])
            shift_bc = bcp.tile([128, TB], F32, tag="shift_bc", name="shiftbc")
            nc.gpsimd.partition_broadcast(shift_bc[:], s_row[:])
            hs = []
            for k in range(KD):
                hh = hp.tile([128, TB], F32, tag=f"{htag}{k}", name=f"{htag}{k}")
                nc.vector.tensor_mul(hh[:], xT[k][:, bsl], rstd_bc[:])
                nc.vector.tensor_add(hh[:], hh[:], shift_bc[:])
                nc.scalar.activation(hh[:], hh[:], AF.Identity,
                                     bias=b_cols[k][:], scale=g_cols[k][:])
                hs.append(hh)
            return hs

        # ========= transformer layers =========
        for li in range(DEPTH):
            # ---- attention phase ----
            g1c = [col(ln1g[li], k * 128, f"lng{k}") for k in range(KD)]
            lb1c = [col(ln1b[li], k * 128, f"lnb{k}") for k in range(KD)]
            wq_sl = []
            for k in range(KD):
                w = bw_p.tile([128, FF], F32, tag="bigw", name="wqs")
                nc.sync.dma_start(w[:, 0:3 * D], wqkvT[li, k * 128:(k + 1) * 128, :])
                wq_sl.append(w)
            wo_sl = []
            for k in range(KD):
                w = wo_p.tile([128, D], F32, tag="wo", name="wos")
                nc.sync.dma_start(w[:], woT[li, k * 128:(k + 1) * 128, :])
                wo_sl.append(w)
            bq_cols = [col(bqkv[li], ot * 128, f"bq{ot}") for ot in range(8)]
            bv_bc = bcp.tile([128, D], F32, tag="bvbc", name="bvbc", bufs=1)
            nc.gpsimd.dma_start(bv_bc[:], bass.AP(
                tensor=bqkv.tensor, offset=bqkv.offset + li * 3 * D + 2 * D,
                ap=[[0, 128], [1, D]]))
            bo_cols = [col(bo[li], ot * 128, f"bo{ot}") for ot in range(KD)]

            for b_ in range(NB):
                tsl = slice(b_ * TB, (b_ + 1) * TB)
                h = ln_block(b_, g1c, lb1c, "h")
                # Q,K projections (transposed out)
                qk = []
                for ot in range(8):
                    ps = ps_mm.tile([128, TB], F32, tag="mm", name="ps")
                    for k in range(KD):
                        nc.tensor.matmul(ps[:], wq_sl[k][:, ot * 128:(ot + 1) * 128],
                                         h[k][:], start=(k == 0), stop=(k == KD - 1))
                    t = qk_p.tile([128, TB], F32, tag="qk", name="qk")
                    nc.scalar.activation(t[:], ps[:], AF.Identity,
                                         bias=bq_cols[ot][:], scale=1.0)
                    qk.append(t)
                onats = []
                for tt in range(NS):
                    ssl = slice(tt * 128, (tt + 1) * 128)
                    # V natural for this subtile, 65-strided with ones column
                    ps = ps_mm.tile([128, D], F32, tag="mm", name="ps")
                    for k in range(KD):
                        nc.tensor.matmul(ps[:], h[k][:, ssl], wq_sl[k][:, 2 * D:3 * D],
                                         start=(k == 0), stop=(k == KD - 1))
                    v = v65_p.tile([128, 8 * 65], F32, tag="v65", name="v65")
                    nc.vector.memset(
                        v[:].rearrange("p (hh c) -> p hh c", hh=8)[:, :, 64:65], 1.0)
                    for hh in range(8):
                        nc.vector.tensor_add(v[:, hh * 65:hh * 65 + 64],
                                             ps[:, hh * 64:(hh + 1) * 64],
                                             bv_bc[:, hh * 64:(hh + 1) * 64])
                    # attention
                    onat = onat_p.tile([128, D], F32, tag="onat", name="onat")
                    for hh in range(8):
                        bp = (hh % 2) * 64
                        kt = qk[4 + hh // 2]
                        qt = qk[hh // 2]
                        s_ps = ps_at.tile([128, 128], F32, tag="at", name="sps")
                        nc.tensor.matmul(s_ps[:], kt[bp:bp + 64, ssl], qt[bp:bp + 64, ssl],
                                         start=True, stop=True)
                        et = e_p.tile([128, 128], F32, tag="et", name="et")
                        nc.scalar.activation(et[:], s_ps[:], AF.Exp, bias=0.0, scale=0.125)
                        nc.vector.tensor_mul(et[:], et[:], mask_t[:])
                        o_ps = ps_at.tile([128, 65], F32, tag="at", name="ops")
                        nc.tensor.matmul(o_ps[:], et[:], v[:, hh * 65:(hh + 1) * 65],
                                         start=True, stop=True)
                        rcol = col2p.tile([128, 1], F32, tag="rcol", name="rcol")
                        nc.vector.reciprocal(rcol[:], o_ps[:, 64:65])
                        nc.vector.tensor_scalar_mul(onat[:, hh * 64:(hh + 1) * 64],
                                                    o_ps[:, 0:64], rcol[:])
                    onats.append(onat)
                # transpose O -> O_T
                oT = [oT_p.tile([128, TB], F32, tag="oT", name="oT") for _ in range(KD)]
                for tt in range(NS):
                    for k in range(KD):
                        tp = ps_at.tile([128, 128], F32, tag="at", name="tp")
                        nc.tensor.transpose(tp[:], onats[tt][:, k * 128:(k + 1) * 128],
                                            ident[:])
                        nc.vector.tensor_copy(oT[k][:, tt * 128:(tt + 1) * 128], tp[:])
                # Wo + residual
                for ot in range(KD):
                    ps = ps_mm.tile([128, TB], F32, tag="mm", name="ps")
                    for k in range(KD):
                        nc.tensor.matmul(ps[:], wo_sl[k][:, ot * 128:(ot + 1) * 128],
                                         oT[k][:], start=(k == 0), stop=(k == KD - 1))
                    nc.vector.scalar_tensor_tensor(xT[ot][:, tsl], ps[:], bo_cols[ot][:],
                                                   xT[ot][:, tsl], op0=ALU.add, op1=ALU.add)

            # ---- FF phase ----
            g2c = [col(ln2g[li], k * 128, f"lng{k}") for k in range(KD)]
            lb2c = [col(ln2b[li], k * 128, f"lnb{k}") for k in range(KD)]
            w1_sl = []
            for k in range(KD):
                w = bw_p.tile([128, FF], F32, tag="bigw", name="w1s")
                nc.sync.dma_start(w[:], w1T[li, k * 128:(k + 1) * 128, :])
                w1_sl.append(w)
            w2_sl = []
            for kf in range(KF):
                w = w2_p.tile([128, D], F32, tag="w2", name="w2s")
                nc.sync.dma_start(w[:], w2T[li, kf * 128:(kf + 1) * 128, :])
                w2_sl.append(w)
            bff_cols = [col(b2[li], ot * 128, f"bo{ot}") for ot in range(KD)]
            for b_ in range(NB):
                tsl = slice(b_ * TB, (b_ + 1) * TB)
                h2 = ln_block(b_, g2c, lb2c, "h")
                acc = [ps_acc.tile([128, TB], F32, tag="acc", name="facc")[:]
                       for _ in range(KD)]
                for kf in range(KF):
                    ps = ps_mm.tile([128, TB], F32, tag="mm", name="ps")
                    for k in range(KD):
                        nc.tensor.matmul(ps[:], w1_sl[k][:, kf * 128:(kf + 1) * 128],
                                         h2[k][:], start=(k == 0), stop=(k == KD - 1))
                    b1col = col(b1[li], kf * 128, "b1c", pool=col2p)
                    rl = relu_p.tile([128, TB], F32, tag="relu", name="rl")
                    nc.scalar.activation(rl[:], ps[:], AF.Relu, bias=b1col[:], scale=1.0)
                    for ot in range(KD):
                        nc.tensor.matmul(acc[ot], w2_sl[kf][:, ot * 128:(ot + 1) * 128],
                                         rl[:], start=(kf == 0), stop=(kf == KF - 1))
                for ot in range(KD):
                    nc.vector.scalar_tensor_tensor(xT[ot][:, tsl], acc[ot],
                                                   bff_cols[ot][:], xT[ot][:, tsl],
                                                   op0=ALU.add, op1=ALU.add)

        # ========= output projection =========
        outw_sb = bw_p.tile([128, FF], F32, tag="bigw", name="outw")
        for k in range(KD):
            nc.sync.dma_start(outw_sb[:, k * 128:k * 128 + CH],
                              outwT[k * 128:(k + 1) * 128, :])
        outb_col = col(outb, 0, "outbcol")
        outb_bc = mbcp.tile([128, CH], F32, tag="outbbc", name="outbbc")
        nc.gpsimd.dma_start(outb_bc[:], bass.AP(
            tensor=outb.tensor, offset=outb.offset, ap=[[0, 128], [1, CH]]))

        # y_T [ch, tok] in bf16 (stationary source for matching matmuls)
        yT_bf = outp.tile([128, T], BF16, tag="yTbf")
        for b_ in range(NB):
            ps = ps_mm.tile([128, TB], F32, tag="mm", name="ps")
            for k in range(KD):
                nc.tensor.matmul(ps[:], outw_sb[:, k * 128:k * 128 + CH],
                                 xT[k][:, b_ * TB:(b_ + 1) * TB],
                                 start=(k == 0), stop=(k == KD - 1))
            nc.scalar.activation(yT_bf[:, b_ * TB:(b_ + 1) * TB], ps[:], AF.Identity,
                                 bias=outb_col[:], scale=1.0)

        # y natural -> xf32_d DRAM fp32; bf16 shards straight into ag_in
        for tt in range(T // 128):
            ps = ps_at.tile([128, CH], F32, tag="at", name="yn_ps")
            for k in range(KD):
                nc.tensor.matmul(ps[:], xT[k][:, tt * 128:(tt + 1) * 128],
                                 outw_sb[:, k * 128:k * 128 + CH],
                                 start=(k == 0), stop=(k == KD - 1))
            yn = m2p.tile([128, CH], F32, tag="yn", name="yn")
            nc.vector.tensor_add(yn[:], ps[:], outb_bc[:])
            nc.sync.dma_start(
                xf32_d[tt * 4:(tt + 1) * 4, :].rearrange("p (l c) -> p l c", l=L),
                yn[:])
            ynbf = m2p.tile([128, CH], BF16, tag="ynbf", name="ynbf")
            nc.vector.tensor_copy(ynbf[:], yn[:])
            nc.sync.dma_start(
                ag_in[AG_XFN + tt * 4 * FDIM: AG_XFN + (tt + 1) * 4 * FDIM]
                .rearrange("(i l c) -> i l c", l=L, c=CH),
                ynbf[:])

        # xn = ||xf_i||^2 via gram diag (bf16 inputs, fp32 accum)
        xfT_st = yT_bf[:].rearrange("c (i l) -> c l i", l=L)   # [128, 32, 64]
        gram = ps_at.tile([SC, SC], F32, tag="at", name="gram")
        for l in range(KFl):
            nc.tensor.matmul(gram[:], xfT_st[:, l, :], xfT_st[:, l, :],
                             start=(l == 0), stop=(l == KFl - 1))
        gd = m2p.tile([SC, SC], F32, tag="gd", name="gd")
        nc.vector.tensor_mul(gd[:], gram[:], ident[0:SC, 0:SC])
        xn_col = colp.tile([SC, 1], F32, tag="xncol", name="xncol")
        nc.vector.reduce_sum(xn_col[:], gd[:], axis=AX.X)

        # write AG input: xf_T + xn bits (xf_nat already streamed above)
        # Compact [c, l, i] in SBUF first: the strided DMA straight from
        # xfT_st explodes into 2-byte descriptors (2.5ms stall).
        for half in range(2):
            xfT_cmp = outp.tile([128, T // 2], BF16, tag="xfTc", name="xfTc")
            for lh in range(L // 2):
                l = half * (L // 2) + lh
                nc.vector.tensor_copy(xfT_cmp[:, lh * SC:(lh + 1) * SC],
                                      xfT_st[:, l, :])
            nc.sync.dma_start(
                bass.AP(tensor=ag_in.tensor,
                        offset=ag_in.offset + AG_XFT + half * (T // 2),
                        ap=[[T, 128], [1, T // 2]]),
                xfT_cmp[:])
        nc.sync.dma_start(
            ag_in[AG_XN:AG_XN + 2 * SC].rearrange("(i bb) -> i bb", bb=2),
            xn_col[:].bitcast(BF16))
        nc.gpsimd.collective_compute(
            "AllGather", ALU.bypass, replica_groups=[list(range(NC_))],
            ins=[ag_in[:]], outs=[ag_out[:]])

        # xn_full row [1, 512] f32 + broadcast
        ago_f32 = ag_out.bitcast(F32)
        xn_row = mrow.tile([1, B], F32, tag="mr", name="xnrow")
        nc.sync.dma_start(
            xn_row[:],
            bass.AP(tensor=ago_f32.tensor, offset=ago_f32.offset + AG_XN // 2,
                    ap=[[1, 1], [AG_SZ // 2, NC_], [1, SC]]))
        xn_bc = mbcp.tile([SC, B], F32, tag="mbc", name="xnbc")
        nc.gpsimd.partition_broadcast(xn_bc[:], xn_row[:])

        # S_pos / S_neg (bf16 matmuls, fp32 accum)
        spos = ps_acc.tile([SC, B], F32, tag="acc", name="spos")
        for l in range(KFl):
            mv = mov_p.tile([128, B], BF16, tag="mv", name="mv")
            nc.sync.dma_start(mv[:], pT[l * 128:(l + 1) * 128, :])
            nc.tensor.matmul(spos[:], xfT_st[:, l, :], mv[:],
                             start=(l == 0), stop=(l == KFl - 1))
        sneg = ps_acc.tile([SC, B], F32, tag="acc", name="sneg")
        for l in range(KFl):
            mv = mov_p.tile([128, B], BF16, tag="mv", name="mv")
            nc.sync.dma_start(
                mv[:],
                bass.AP(tensor=ag_out.tensor, offset=ag_out.offset + AG_XFT + l * SC,
                        ap=[[T, 128], [AG_SZ, NC_], [1, SC]]))
            nc.tensor.matmul(sneg[:], xfT_st[:, l, :], mv[:],
                             start=(l == 0), stop=(l == KFl - 1))

        # distances -> logits -> E (in place)
        dist = mtch.tile([SC, 2 * B], F32, tag="dist")
        nc.vector.scalar_tensor_tensor(dist[:, 0:B], spos[:], -2.0, pn_t[:],
                                       op0=ALU.mult, op1=ALU.add)
        nc.vector.scalar_tensor_tensor(dist[:, B:2 * B], sneg[:], -2.0, xn_bc[:],
                                       op0=ALU.mult, op1=ALU.add)
        nc.vector.tensor_scalar_add(dist[:], dist[:], xn_col[:])
        nc.vector.tensor_scalar_max(dist[:], dist[:], 0.0)
        nc.scalar.activation(dist[:], dist[:], AF.Sqrt, bias=0.0, scale=1.0)
        nc.vector.tensor_add(dist[:, B:2 * B], dist[:, B:2 * B], nd_t[:])
        dmin = colp.tile([SC, 1], F32, tag="dmin", name="dmin")
        nc.vector.tensor_reduce(out=dmin[:], in_=dist[:], axis=AX.X, op=ALU.min)
        E = dist  # in place: E = exp(-d + dmin)
        nc.scalar.activation(E[:], dist[:], AF.Exp, bias=dmin[:], scale=-1.0)
        g_col = colp.tile([SC, 1], F32, tag="gcol", name="gcol")
        nc.scalar.activation(g_col[:], dmin[:], AF.Exp, bias=m20_col[:], scale=-1.0)
        sr_col = colp.tile([SC, 1], F32, tag="srcol", name="srcol")
        nc.vector.reduce_sum(sr_col[:], E[:], axis=AX.X)
        # partial colsums of G = E * g_i via g-weighted stationary
        cs_row = mrow.tile([1, 2 * B], F32, tag="mr", name="csrow")
        for b_ in range(2):
            ps = ps_mm.tile([1, B], F32, tag="mm", name="ps")
            nc.tensor.matmul(ps[:], g_col[:], E[:, b_ * B:(b_ + 1) * B],
                             start=True, stop=True)
            nc.vector.tensor_copy(cs_row[:, b_ * B:(b_ + 1) * B], ps[:])
        nc.sync.dma_start(ar_in, cs_row[:])
        nc.gpsimd.collective_compute(
            "AllReduce", ALU.add, replica_groups=[list(range(NC_))],
            ins=[ar_in[:]], outs=[ar_out[:]])
        cs_g = mrow.tile([1, 2 * B], F32, tag="mr", name="csg")
        nc.sync.dma_start(cs_g[:], ar_out)
        cs_bc = mbcp.tile([SC, 2 * B], F32, tag="csbc", name="csbc")
        nc.gpsimd.partition_broadcast(cs_bc[:], cs_g[:])
        nc.scalar.activation(cs_bc[:], cs_bc[:], AF.Sqrt, bias=0.0, scale=1.0)
        nc.vector.reciprocal(cs_bc[:], cs_bc[:])
        # E' = E * invsqrt(Sc); row scalars BEFORE overwriting E with W
        nc.vector.tensor_mul(E[:], E[:], cs_bc[:])
        snp = colp.tile([SC, 1], F32, tag="snp", name="snp")
        nc.vector.reduce_sum(snp[:], E[:, B:2 * B], axis=AX.X)
        spp = colp.tile([SC, 1], F32, tag="spp", name="spp")
        nc.vector.reduce_sum(spp[:], E[:, 0:B], axis=AX.X)
        tcol = colp.tile([SC, 1], F32, tag="tcol", name="tcol")
        nc.vector.reciprocal(tcol[:], sr_col[:])
        nc.vector.tensor_mul(tcol[:], tcol[:], g_col[:])
        ccol = colp.tile([SC, 1], F32, tag="ccol", name="ccol")
        nc.scalar.activation(ccol[:], tcol[:], AF.Sqrt, bias=0.0, scale=1.0)
        alpha = colp.tile([SC, 1], F32, tag="alpha", name="alpha")
        nc.vector.tensor_mul(alpha[:], tcol[:], snp[:])
        beta = colp.tile([SC, 1], F32, tag="beta", name="beta")
        nc.vector.tensor_mul(beta[:], alpha[:], spp[:])
        nc.vector.tensor_mul(beta[:], beta[:], ccol[:])
        nc.vector.tensor_scalar_mul(beta[:], beta[:], -1.0)
        # W = E' * alpha / -beta (in place), transpose, cast bf16
        nc.vector.tensor_scalar_mul(E[:, 0:B], E[:, 0:B], alpha[:])
        nc.vector.tensor_scalar_mul(E[:, B:2 * B], E[:, B:2 * B], beta[:])
        wT = []
        for half in range(2):
            for jt in range(4):
                tp = ps_at.tile([128, SC], F32, tag="at", name="wtp")
                nc.tensor.transpose(
                    tp[:], E[:, half * B + jt * 128: half * B + (jt + 1) * 128],
                    ident[0:SC, 0:SC])
                t = wT_p.tile([128, SC], BF16, tag="wT", name="wT")
                nc.vector.tensor_copy(t[:], tp[:])
                wT.append(t)
        # V and loss: V = Wpos @ p - Wneg @ xf_full, r = xf - fl(xf + V)
        lacc = m2p.tile([SC, 16], F32, tag="lacc", name="lacc", bufs=1)
        FBW = 256
        for fb in range(FDIM // FBW):
            vps = ps_acc.tile([SC, FBW], F32, tag="acc", name="vps")
            for jt in range(4):
                mv = mov_p.tile([128, FBW], BF16, tag="mv", name="mv")
                nc.sync.dma_start(mv[:], pnat[jt * 128:(jt + 1) * 128,
                                              fb * FBW:(fb + 1) * FBW])
                nc.tensor.matmul(vps[:], wT[jt][:], mv[:], start=(jt == 0), stop=False)
            for jt in range(4):
                mv = mov_p.tile([128, FBW], BF16, tag="mv", name="mv")
                nc.sync.dma_start(
                    mv[:],
                    bass.AP(tensor=ag_out.tensor,
                            offset=ag_out.offset + AG_XFN + 2 * jt * AG_SZ + fb * FBW,
                            ap=[[AG_SZ, 2], [FDIM, SC], [1, FBW]]))
                nc.tensor.matmul(vps[:], wT[4 + jt][:], mv[:], start=False, stop=(jt == 3))
            xfb = m2p.tile([SC, FBW], F32, tag="xfb", name="xfb")
            nc.sync.dma_start(xfb[:], xf32_d[:, fb * FBW:(fb + 1) * FBW])
            t1 = m2p.tile([SC, FBW], F32, tag="t1", name="t1")
            nc.vector.tensor_add(t1[:], xfb[:], vps[:])
            nc.vector.tensor_sub(t1[:], xfb[:], t1[:])
            nc.vector.tensor_mul(t1[:], t1[:], t1[:])
            nc.vector.reduce_sum(lacc[:, fb:fb + 1], t1[:], axis=AX.X)
        lsum = colp.tile([SC, 1], F32, tag="lsum", name="lsum")
        nc.vector.reduce_sum(lsum[:], lacc[:], axis=AX.X)
        tot = ps_mm.tile([1, 1], F32, tag="mm", name="tot")
        nc.tensor.matmul(tot[:], ones_col[0:SC, :], lsum[:], start=True, stop=True)
        tot_sb = colp.tile([1, 1], F32, tag="tot", name="totsb")
        nc.vector.tensor_copy(tot_sb[:], tot[:])
        nc.sync.dma_start(loss_part, tot_sb[:])

    nc.compile()
    return nc


_NC_CACHE = None


def _get_nc():
    global _NC_CACHE
    if _NC_CACHE is None:
        _NC_CACHE = _build_nc()
    return _NC_CACHE


def _prep_inputs(inputs):
    f32 = lambda x: np.ascontiguousarray(np.asarray(x), dtype=np.float32)
    bf = lambda x: np.ascontiguousarray(np.asarray(x, dtype=ml_dtypes.bfloat16))
    sample_p = f32(inputs["sample_p"])
    eps = f32(inputs["eps"])
    p2 = sample_p.reshape(B, FDIM)
    pn = (p2.astype(np.float64) ** 2).sum(-1).astype(np.float32)

    common = {
        "inwT": f32(inputs["in_w"]).T.copy(),
        "inb": f32(inputs["in_b"]),
        "wqkvT": np.ascontiguousarray(f32(inputs["Wqkv"]).transpose(0, 2, 1)),
        "bqkv": f32(inputs["bqkv"]),
        "woT": np.ascontiguousarray(f32(inputs["Wo"]).transpose(0, 2, 1)),
        "bo": f32(inputs["bo"]),
        "ln1g": f32(inputs["ln1_g"]), "ln1b": f32(inputs["ln1_b"]),
        "w1T": np.ascontiguousarray(f32(inputs["W1"]).transpose(0, 2, 1)),
        "b1": f32(inputs["b1"]),
        "w2T": np.ascontiguousarray(f32(inputs["W2"]).transpose(0, 2, 1)),
        "b2": f32(inputs["b2"]),
        "ln2g": f32(inputs["ln2_g"]), "ln2b": f32(inputs["ln2_b"]),
        "outwT": f32(inputs["out_w"]).T.copy(),
        "outb": f32(inputs["out_b"]),
        "pT": bf(p2.T),
        "pnat": bf(p2),
        "pn_bc": np.broadcast_to(pn[None, :], (SC, B)).copy(),
        "attn_mask": np.kron(np.eye(4, dtype=np.float32), np.ones((32, 32), np.float32)),
    }
    in_maps = []
    for c in range(NC_):
        nd = np.zeros((SC, B), np.float32)
        nd[np.arange(SC), SC * c + np.arange(SC)] = 1e6
        m = dict(common)
        m["epsT"] = eps[c * SC:(c + 1) * SC].reshape(T, CH).T.copy()
        m["negdiag"] = nd
        in_maps.append(m)
    return in_maps


def kernel(**inputs) -> np.ndarray:
    nc = _get_nc()
    in_maps = _prep_inputs(inputs)
    res = run_bass_kernel_spmd(nc, in_maps, list(range(NC_)))
    total = sum(float(r["loss_part"][0, 0]) for r in res.results)
    return np.float32(total / (B * FDIM))



# revision 11
# speedup vs baseline: 1.6972x; 1.1513x over previous
"""Trainium2 Bass kernel for nn_DriftScene_88270167868070.

Contract: kernel(**inputs) takes FULL unsharded inputs (as produced by
setup_inputs()) and returns the FULL output (a scalar np.float32).

Strategy (8 NeuronCores, one SPMD launch):
  - Data-parallel transformer generator over the batch (64 scenes/core).
  - Main matmuls use a 3-term fp32r/bf16 split: W ~ Wh(f32r,20bit) +
    Wl(bf16 of residual); activations h ~ hcat=[f32r(h)|f32r(h-hi)] plus a
    bf16 copy. y = Wh@hcat (one N=512 f32r matmul per k-step, hi and lo
    halves side by side in PSUM) + Wl@h_bf (bf16). Effective input
    precision ~2^-20..2^-24, ~2.6x faster than fp32 on the PE.
  - LN stats (mean/sumsq) via f32r ones-matmuls.
  - Attention kept fp32 (small share of PE time).
  - Matching stage row-sharded, bf16 matmuls; packed AllGather shares
    xf (both layouts) + ||xf||^2; AllReduce for column-softmax sums.
  - loss = mean((xf - fl32(xf + V))^2) with explicit fp32 rounding.
"""

import numpy as np
from contextlib import ExitStack

import concourse.bass as bass
import concourse.tile as tile
from concourse import bacc, mybir
from concourse.bass_utils import run_bass_kernel_spmd
from concourse.masks import make_identity
import ml_dtypes

F32 = mybir.dt.float32
F32R = mybir.dt.float32r
BF16 = mybir.dt.bfloat16
AF = mybir.ActivationFunctionType
ALU = mybir.AluOpType
AX = mybir.AxisListType

# Problem dims (hardcoded per contract)
B, L, CH = 512, 32, 128
D, HEADS, DEPTH, FF = 512, 8, 4, 2048
DH = D // HEADS
LN_EPS = 1e-5
NC_ = 8                 # cores
SC = B // NC_           # 64 scenes per core
T = SC * L              # 2048 tokens per core
TB = 256                # tokens per t-block
NB = T // TB            # 8 t-blocks
NS = TB // 128          # 2 subtiles per block
KD = D // 128           # 4 d-tiles
KF = FF // 128          # 16 ff-tiles
FDIM = L * CH           # 4096 flattened feature dim
KFl = FDIM // 128       # 32 f-tiles
M_SHIFT = -20.0         # global shift for column softmax stabilization

# packed AllGather layout (bf16 element offsets)
AG_XFT = 0                      # xf_T   [c(128), l(32), i(64)] c-major
AG_XFN = FDIM * SC              # xf_nat [64, 4096]
AG_XN = 2 * FDIM * SC           # xn bits: f32 [64,1] viewed as bf16 [64,2]
AG_SZ = 2 * FDIM * SC + 2 * SC  # 524416


def _build_nc():
    nc = bacc.Bacc("TRN2", target_bir_lowering=False, debug=False, num_devices=NC_)

    # ---------------- I/O ----------------
    def inp(name, shape, dt=F32):
        return nc.dram_tensor(name, shape, dt, kind="ExternalInput").ap()

    epsT = inp("epsT", [128, T])              # eps shard, [ch, tok]
    inwT = inp("inwT", [128, D])              # in_w.T (fp32)
    inb = inp("inb", [D])
    wqkvT_h = inp("wqkvT_h", [DEPTH, D, 3 * D], F32R)
    wqkvT_l = inp("wqkvT_l", [DEPTH, D, 3 * D], BF16)
    bqkv = inp("bqkv", [DEPTH, 3 * D])
    woT_h = inp("woT_h", [DEPTH, D, D], F32R)
    woT_l = inp("woT_l", [DEPTH, D, D], BF16)
    bo = inp("bo", [DEPTH, D])
    ln1g = inp("ln1g", [DEPTH, D])
    ln1b = inp("ln1b", [DEPTH, D])
    w1T_h = inp("w1T_h", [DEPTH, D, FF], F32R)
    w1T_l = inp("w1T_l", [DEPTH, D, FF], BF16)
    b1 = inp("b1", [DEPTH, FF])
    w2T_h = inp("w2T_h", [DEPTH, FF, D], F32R)
    w2T_l = inp("w2T_l", [DEPTH, FF, D], BF16)
    b2 = inp("b2", [DEPTH, D])
    ln2g = inp("ln2g", [DEPTH, D])
    ln2b = inp("ln2b", [DEPTH, D])
    outwT = inp("outwT", [D, CH])             # out_w.T (fp32)
    outb = inp("outb", [CH])
    pT = inp("pT", [FDIM, B], BF16)           # sample_p transposed [f, scene]
    pnat = inp("pnat", [B, FDIM], BF16)       # sample_p natural
    pn_bc = inp("pn_bc", [SC, B])             # ||p_j||^2 broadcast rows
    attn_mask = inp("attn_mask", [128, 128])  # 4-scene block-diag 0/1
    negdiag = inp("negdiag", [SC, B])         # 1e6 at (i, SC*core + i)

    loss_part = nc.dram_tensor("loss_part", [1, 1], F32, kind="ExternalOutput").ap()

    # ---------------- DRAM scratch ----------------
    ag_in = nc.dram_tensor("ag_in", [AG_SZ], BF16).ap()
    ag_out = nc.dram_tensor("ag_out", [NC_ * AG_SZ], BF16, addr_space="Shared").ap()
    xf32_d = nc.dram_tensor("xf32_d", [SC, FDIM], F32).ap()
    ar_in = nc.dram_tensor("ar_in", [1, 2 * B], F32).ap()
    ar_out = nc.dram_tensor("ar_out", [1, 2 * B], F32, addr_space="Shared").ap()

    with tile.TileContext(nc) as tc, ExitStack() as ctx:
        # ---------------- outer pools (whole kernel; bufs is PER TAG) -----
        const = ctx.enter_context(tc.tile_pool(name="const", bufs=1))
        xTp = ctx.enter_context(tc.tile_pool(name="xT", bufs=1))
        io_p = ctx.enter_context(tc.tile_pool(name="inout", bufs=2))   # inw/outw f32
        colp = ctx.enter_context(tc.tile_pool(name="colp", bufs=1))
        col2p = ctx.enter_context(tc.tile_pool(name="col2p", bufs=2))
        outp = ctx.enter_context(tc.tile_pool(name="outp", bufs=1))

        ps_mm = ctx.enter_context(tc.tile_pool(name="ps_mm", bufs=2, space="PSUM"))
        ps_acc = ctx.enter_context(tc.tile_pool(name="ps_acc", bufs=4, space="PSUM"))
        ps_at = ctx.enter_context(tc.tile_pool(name="ps_at", bufs=2, space="PSUM"))

        # ------------- generator-only pools (released before matching) ----
        gen_ctx = ExitStack()
        hcp = gen_ctx.enter_context(tc.tile_pool(name="hc", bufs=2))      # hcat f32r
        hbp = gen_ctx.enter_context(tc.tile_pool(name="hb", bufs=2))      # h bf16
        hsp = gen_ctx.enter_context(tc.tile_pool(name="hs", bufs=2))      # h f32 scratch
        sqp = gen_ctx.enter_context(tc.tile_pool(name="sq", bufs=3))
        xrp = gen_ctx.enter_context(tc.tile_pool(name="xr", bufs=1))      # x f32r stats
        rowp = gen_ctx.enter_context(tc.tile_pool(name="rows", bufs=1))
        bcp = gen_ctx.enter_context(tc.tile_pool(name="bc", bufs=2))
        bw_p = gen_ctx.enter_context(tc.tile_pool(name="bigw", bufs=KD))   # hi f32r
        bwl_p = gen_ctx.enter_context(tc.tile_pool(name="bigwlo", bufs=KD))  # lo bf16
        wo_p = gen_ctx.enter_context(tc.tile_pool(name="wo", bufs=KD))     # wo hi f32r
        wol_p = gen_ctx.enter_context(tc.tile_pool(name="wolo", bufs=KD))  # wo lo bf16
        w2_p = gen_ctx.enter_context(tc.tile_pool(name="w2", bufs=3))      # w2 hi strm
        w2l_p = gen_ctx.enter_context(tc.tile_pool(name="w2lo", bufs=3))   # w2 lo strm
        qk_p = gen_ctx.enter_context(tc.tile_pool(name="qk", bufs=8))
        v65_p = gen_ctx.enter_context(tc.tile_pool(name="v65", bufs=2))
        e_p = gen_ctx.enter_context(tc.tile_pool(name="et", bufs=2))
        onat_p = gen_ctx.enter_context(tc.tile_pool(name="onat", bufs=2))
        oT_p = gen_ctx.enter_context(tc.tile_pool(name="oT", bufs=4))      # oTcat f32r
        oTb_p = gen_ctx.enter_context(tc.tile_pool(name="oTb", bufs=4))    # oT bf16
        rc_p = gen_ctx.enter_context(tc.tile_pool(name="rlc", bufs=2))     # relu f32r
        rb_p = gen_ctx.enter_context(tc.tile_pool(name="rlb", bufs=2))     # relu bf16
        rs_p = gen_ctx.enter_context(tc.tile_pool(name="rls", bufs=2))     # relu f32

        # ---------------- constants ----------------
        ident = const.tile([128, 128], F32)
        make_identity(nc, ident[:])
        ones_col = const.tile([128, 1], F32)
        nc.vector.memset(ones_col[:], 1.0)
        ones_r = const.tile([128, 1], F32R)
        nc.vector.tensor_copy(ones_r[:], ones_col[:])
        mask_t = const.tile([128, 128], F32)
        nc.sync.dma_start(mask_t[:], attn_mask)
        pn_t = const.tile([SC, B], F32)
        nc.sync.dma_start(pn_t[:], pn_bc)
        nd_t = const.tile([SC, B], F32)
        nc.sync.dma_start(nd_t[:], negdiag)
        eps_col = const.tile([1, 1], F32)
        nc.vector.memset(eps_col[:], LN_EPS)
        m20_col = const.tile([SC, 1], F32)
        nc.vector.memset(m20_col[:], -M_SHIFT)

        # residual stream X_T: KD tiles [128, T] fp32, persistent
        xT = [xTp.tile([128, T], F32, tag=f"xT{k}", name=f"xT{k}") for k in range(KD)]

        def col(ap_1d, base, tag, pool=colp, n=128):
            t = pool.tile([n, 1], F32, tag=tag, name=tag)
            nc.sync.dma_start(t[:], ap_1d[base:base + n])
            return t

        # ========= input projection: X_T = (eps @ in_w.T).T (fp32) =========
        inw_sb = io_p.tile([128, D], F32, tag="inout", name="inw")
        nc.sync.dma_start(inw_sb[:], inwT)
        ib_cols = [col(inb, k * 128, f"bo{k}") for k in range(KD)]
        for b_ in range(NB):
            bsl = slice(b_ * TB, (b_ + 1) * TB)
            eps_blk = sqp.tile([128, TB], F32, tag="eps", name="epsblk", bufs=2)
            nc.sync.dma_start(eps_blk[:], epsT[:, bsl])
            for dt_ in range(KD):
                ps = ps_mm.tile([128, 2 * TB], F32, tag="mm", name="ps")
                nc.tensor.matmul(ps[:, 0:TB], inw_sb[:, dt_ * 128:(dt_ + 1) * 128],
                                 eps_blk[:], start=True, stop=True)
                nc.scalar.activation(xT[dt_][:, bsl], ps[:, 0:TB],
                                     AF.Identity, bias=ib_cols[dt_][:], scale=1.0)

        # ========= per-block layernorm -> hcat/hbf tiles =========
        def ln_block(b_, g_cols, b_cols):
            """LN over partition dim for block b_ -> (hcat f32r, hbf) tiles."""
            bsl = slice(b_ * TB, (b_ + 1) * TB)
            # f32r copies of x for stats matmuls
            xr = []
            for k in range(KD):
                xrt = xrp.tile([128, TB], F32R, tag=f"xr{k % 2}", name="xr")
                nc.vector.tensor_copy(xrt[:], xT[k][:, bsl])
                xr.append(xrt)
            s_row = rowp.tile([1, TB], F32, tag="srow", name="srow")
            ps_s = ps_at.tile([1, TB], F32, tag="at", name="pss")
            for k in range(KD):
                nc.tensor.matmul(ps_s[:], ones_r[:], xr[k][:],
                                 start=(k == 0), stop=(k == KD - 1))
            nc.vector.tensor_scalar_mul(s_row[:], ps_s[:], -1.0 / D)   # -mean
            q_row = rowp.tile([1, TB], F32, tag="qrow", name="qrow")
            ps_q = ps_at.tile([1, TB], F32, tag="at", name="psq")
            for k in range(KD):
                sq = sqp.tile([128, TB], F32R, tag="sq", name="sq")
                nc.vector.tensor_mul(sq[:], xT[k][:, bsl], xT[k][:, bsl])
                nc.tensor.matmul(ps_q[:], ones_r[:], sq[:],
                                 start=(k == 0), stop=(k == KD - 1))
            msq = rowp.tile([1, TB], F32, tag="msq", name="msq")
            nc.vector.tensor_mul(msq[:], s_row[:], s_row[:])
            # var = q/D - m^2  (into q_row)
            nc.vector.scalar_tensor_tensor(q_row[:], ps_q[:], 1.0 / D, msq[:],
                                           op0=ALU.mult, op1=ALU.subtract)
            # rstd = 1/sqrt(var + eps): sqrt into msq, recip into q_row
            nc.scalar.activation(msq[:], q_row[:], AF.Sqrt, bias=eps_col[:], scale=1.0)
            nc.vector.reciprocal(q_row[:], msq[:])
            # shift = -m * rstd (into s_row)
            nc.vector.tensor_mul(s_row[:], s_row[:], q_row[:])
            rstd_bc = bcp.tile([128, TB], F32, tag="rstd_bc", name="rstdbc")
            nc.gpsimd.partition_broadcast(rstd_bc[:], q_row[:])
            shift_bc = bcp.tile([128, TB], F32, tag="shift_bc", name="shiftbc")
            nc.gpsimd.partition_broadcast(shift_bc[:], s_row[:])
            hcs, hbs = [], []
            for k in range(KD):
                hs = hsp.tile([128, TB], F32, tag="hs", name="hs")
                nc.vector.tensor_mul(hs[:], xT[k][:, bsl], rstd_bc[:])
                nc.vector.tensor_add(hs[:], hs[:], shift_bc[:])
                hc = hcp.tile([128, 2 * TB], F32R, tag=f"hc{k}", name=f"hc{k}")
                hb = hbp.tile([128, TB], BF16, tag=f"hb{k}", name=f"hb{k}")
                hf = hsp.tile([128, TB], F32, tag="hf", name="hf")
                # h (f32) and hi (f32r) via two activations; lo = h - hi
                nc.scalar.activation(hf[:], hs[:], AF.Identity,
                                     bias=b_cols[k][:], scale=g_cols[k][:])
                nc.scalar.activation(hc[:, 0:TB], hs[:], AF.Identity,
                                     bias=b_cols[k][:], scale=g_cols[k][:])
                nc.vector.tensor_sub(hc[:, TB:2 * TB], hf[:],
                                     hc[:, 0:TB].bitcast(F32))
                nc.vector.tensor_copy(hb[:], hf[:])
                hcs.append(hc)
                hbs.append(hb)
            return hcs, hbs

        def split_matmul(ps, wh_slices, wl_slices, hcs, hbs, nk):
            """ps[:, 0:TB] + ps[:, TB:2TB] accumulates W^T@h via 3-term split.

            wh_slices[k]: f32r stationary [128,128]; wl_slices[k]: bf16.
            hcs[k]: [128, 2TB] f32r moving; hbs[k]: [128, TB] bf16 moving.
            """
            for k in range(nk):
                nc.tensor.matmul(ps[:, 0:2 * TB], wh_slices[k], hcs[k][:],
                                 start=(k == 0), stop=False)
            for k in range(nk):
                nc.tensor.matmul(ps[:, 0:TB], wl_slices[k], hbs[k][:],
                                 start=False, stop=(k == nk - 1))

        # ========= transformer layers =========
        for li in range(DEPTH):
            # ---- attention phase ----
            g1c = [col(ln1g[li], k * 128, f"lng{k}") for k in range(KD)]
            lb1c = [col(ln1b[li], k * 128, f"lnb{k}") for k in range(KD)]
            wq_h, wq_l = [], []
            for k in range(KD):
                wh = bw_p.tile([128, FF], F32R, tag="bigw", name="wqh")
                nc.sync.dma_start(wh[:, 0:3 * D], wqkvT_h[li, k * 128:(k + 1) * 128, :])
                wq_h.append(wh)
                wl = bwl_p.tile([128, FF], BF16, tag="bigwlo", name="wql")
                nc.sync.dma_start(wl[:, 0:3 * D], wqkvT_l[li, k * 128:(k + 1) * 128, :])
                wq_l.append(wl)
            wo_h, wo_l = [], []
            for k in range(KD):
                wh = wo_p.tile([128, D], F32R, tag="wo", name="woh")
                nc.sync.dma_start(wh[:], woT_h[li, k * 128:(k + 1) * 128, :])
                wo_h.append(wh)
                wl = wol_p.tile([128, D], BF16, tag="wolo", name="wol")
                nc.sync.dma_start(wl[:], woT_l[li, k * 128:(k + 1) * 128, :])
                wo_l.append(wl)
            bq_cols = [col(bqkv[li], ot * 128, f"bq{ot}") for ot in range(8)]
            bv_bc = bcp.tile([128, D], F32, tag="bvbc", name="bvbc", bufs=1)
            nc.gpsimd.dma_start(bv_bc[:], bass.AP(
                tensor=bqkv.tensor, offset=bqkv.offset + li * 3 * D + 2 * D,
                ap=[[0, 128], [1, D]]))
            bo_cols = [col(bo[li], ot * 128, f"bo{ot}") for ot in range(KD)]

            for b_ in range(NB):
                tsl = slice(b_ * TB, (b_ + 1) * TB)
                hcs, hbs = ln_block(b_, g1c, lb1c)
                # Q,K projections (transposed out)
                qk = []
                for ot in range(8):
                    ps = ps_mm.tile([128, 2 * TB], F32, tag="mm", name="ps")
                    split_matmul(ps,
                                 [wq_h[k][:, ot * 128:(ot + 1) * 128] for k in range(KD)],
                                 [wq_l[k][:, ot * 128:(ot + 1) * 128] for k in range(KD)],
                                 hcs, hbs, KD)
                    t = qk_p.tile([128, TB], F32, tag="qk", name="qk")
                    nc.scalar.activation(t[:], ps[:, 0:TB], AF.Identity,
                                         bias=bq_cols[ot][:], scale=1.0)
                    nc.vector.tensor_add(t[:], t[:], ps[:, TB:2 * TB])
                    qk.append(t)
                onats = []
                for tt in range(NS):
                    ssl = slice(tt * 128, (tt + 1) * 128)
                    # V natural for this subtile: 3-term with h stationary
                    ps = ps_mm.tile([128, 2 * TB], F32, tag="mm", name="ps")
                    for k in range(KD):
                        nc.tensor.matmul(ps[:, 0:D], hcs[k][:, tt * 128:(tt + 1) * 128],
                                         wq_h[k][:, 2 * D:3 * D],
                                         start=(k == 0), stop=False)
                        nc.tensor.matmul(ps[:, 0:D],
                                         hcs[k][:, TB + tt * 128:TB + (tt + 1) * 128],
                                         wq_h[k][:, 2 * D:3 * D],
                                         start=False, stop=False)
                        nc.tensor.matmul(ps[:, 0:D], hbs[k][:, ssl],
                                         wq_l[k][:, 2 * D:3 * D],
                                         start=False, stop=(k == KD - 1))
                    v = v65_p.tile([128, 8 * 65], F32, tag="v65", name="v65")
                    nc.vector.memset(
                        v[:].rearrange("p (hh c) -> p hh c", hh=8)[:, :, 64:65], 1.0)
                    for hh in range(8):
                        nc.vector.tensor_add(v[:, hh * 65:hh * 65 + 64],
                                             ps[:, hh * 64:(hh + 1) * 64],
                                             bv_bc[:, hh * 64:(hh + 1) * 64])
                    # attention
                    onat = onat_p.tile([128, D], F32, tag="onat", name="onat")
                    for hh in range(8):
                        bp = (hh % 2) * 64
                        kt = qk[4 + hh // 2]
                        qt = qk[hh // 2]
                        s_ps = ps_at.tile([128, 128], F32, tag="at", name="sps")
                        nc.tensor.matmul(s_ps[:], kt[bp:bp + 64, ssl], qt[bp:bp + 64, ssl],
                                         start=True, stop=True)
                        et = e_p.tile([128, 128], F32, tag="et", name="et")
                        nc.scalar.activation(et[:], s_ps[:], AF.Exp, bias=0.0, scale=0.125)
                        nc.vector.tensor_mul(et[:], et[:], mask_t[:])
                        o_ps = ps_at.tile([128, 65], F32, tag="at", name="ops")
                        nc.tensor.matmul(o_ps[:], et[:], v[:, hh * 65:(hh + 1) * 65],
                                         start=True, stop=True)
                        rcol = col2p.tile([128, 1], F32, tag="rcol", name="rcol")
                        nc.vector.reciprocal(rcol[:], o_ps[:, 64:65])
                        nc.vector.tensor_scalar_mul(onat[:, hh * 64:(hh + 1) * 64],
                                                    o_ps[:, 0:64], rcol[:])
                    onats.append(onat)
                # transpose O -> oTcat (f32r hi|lo) + oT bf16
                oTc = [oT_p.tile([128, 2 * TB], F32R, tag="oT", name="oT")
                       for _ in range(KD)]
                oTb = [oTb_p.tile([128, TB], BF16, tag="oTb", name="oTb")
                       for _ in range(KD)]
                for tt in range(NS):
                    csl = slice(tt * 128, (tt + 1) * 128)
                    for k in range(KD):
                        tp = ps_at.tile([128, 128], F32, tag="at", name="tp")
                        nc.tensor.transpose(tp[:], onats[tt][:, k * 128:(k + 1) * 128],
                                            ident[:])
                        nc.vector.tensor_copy(oTc[k][:, csl], tp[:])
                        nc.vector.tensor_sub(oTc[k][:, TB + tt * 128:TB + (tt + 1) * 128],
                                             tp[:], oTc[k][:, csl].bitcast(F32))
                        nc.vector.tensor_copy(oTb[k][:, csl], tp[:])
                # Wo + residual
                for ot in range(KD):
                    ps = ps_mm.tile([128, 2 * TB], F32, tag="mm", name="ps")
                    split_matmul(ps,
                                 [wo_h[k][:, ot * 128:(ot + 1) * 128] for k in range(KD)],
                                 [wo_l[k][:, ot * 128:(ot + 1) * 128] for k in range(KD)],
                                 oTc, oTb, KD)
                    t = hsp.tile([128, TB], F32, tag="res", name="res")
                    nc.scalar.activation(t[:], ps[:, 0:TB], AF.Identity,
                                         bias=bo_cols[ot][:], scale=1.0)
                    nc.vector.tensor_add(t[:], t[:], ps[:, TB:2 * TB])
                    nc.vector.tensor_add(xT[ot][:, tsl], xT[ot][:, tsl], t[:])

            # ---- FF phase ----
            g2c = [col(ln2g[li], k * 128, f"lng{k}") for k in range(KD)]
            lb2c = [col(ln2b[li], k * 128, f"lnb{k}") for k in range(KD)]
            w1_h, w1_l = [], []
            for k in range(KD):
                wh = bw_p.tile([128, FF], F32R, tag="bigw", name="w1h")
                nc.sync.dma_start(wh[:], w1T_h[li, k * 128:(k + 1) * 128, :])
                w1_h.append(wh)
                wl = bwl_p.tile([128, FF], BF16, tag="bigwlo", name="w1l")
                nc.sync.dma_start(wl[:], w1T_l[li, k * 128:(k + 1) * 128, :])
                w1_l.append(wl)
            bff_cols = [col(b2[li], ot * 128, f"bo{ot}") for ot in range(KD)]
            for b_ in range(NB):
                tsl = slice(b_ * TB, (b_ + 1) * TB)
                hcs, hbs = ln_block(b_, g2c, lb2c)
                acc = [ps_acc.tile([128, 2 * TB], F32, tag="acc", name="facc")[:]
                       for _ in range(KD)]
                for kf in range(KF):
                    # stream w2 tiles for this kf
                    w2h = w2_p.tile([128, D], F32R, tag="w2", name="w2h")
                    nc.sync.dma_start(w2h[:], w2T_h[li, kf * 128:(kf + 1) * 128, :])
                    w2l = w2l_p.tile([128, D], BF16, tag="w2lo", name="w2l")
                    nc.sync.dma_start(w2l[:], w2T_l[li, kf * 128:(kf + 1) * 128, :])
                    ps = ps_mm.tile([128, 2 * TB], F32, tag="mm", name="ps")
                    split_matmul(ps,
                                 [w1_h[k][:, kf * 128:(kf + 1) * 128] for k in range(KD)],
                                 [w1_l[k][:, kf * 128:(kf + 1) * 128] for k in range(KD)],
                                 hcs, hbs, KD)
                    b1col = col(b1[li], kf * 128, "b1c", pool=col2p)
                    rs = rs_p.tile([128, TB], F32, tag="rs", name="rs")
                    nc.scalar.activation(rs[:], ps[:, 0:TB], AF.Identity,
                                         bias=b1col[:], scale=1.0)
                    nc.vector.tensor_add(rs[:], rs[:], ps[:, TB:2 * TB])
                    rc = rc_p.tile([128, 2 * TB], F32R, tag="rc", name="rc")
                    rb = rb_p.tile([128, TB], BF16, tag="rb", name="rb")
                    rf = rs_p.tile([128, TB], F32, tag="rf", name="rf")
                    nc.scalar.activation(rf[:], rs[:], AF.Relu, bias=0.0, scale=1.0)
                    nc.scalar.activation(rc[:, 0:TB], rs[:], AF.Relu, bias=0.0, scale=1.0)
                    nc.vector.tensor_sub(rc[:, TB:2 * TB], rf[:],
                                         rc[:, 0:TB].bitcast(F32))
                    nc.vector.tensor_copy(rb[:], rf[:])
                    for ot in range(KD):
                        osl = slice(ot * 128, (ot + 1) * 128)
                        nc.tensor.matmul(acc[ot][:, 0:2 * TB], w2h[:, osl], rc[:],
                                         start=(kf == 0), stop=False)
                        nc.tensor.matmul(acc[ot][:, 0:TB], w2l[:, osl], rb[:],
                                         start=False, stop=(kf == KF - 1))
                for ot in range(KD):
                    t = hsp.tile([128, TB], F32, tag="res", name="res")
                    nc.scalar.activation(t[:], acc[ot][:, 0:TB], AF.Identity,
                                         bias=bff_cols[ot][:], scale=1.0)
                    nc.vector.tensor_add(t[:], t[:], acc[ot][:, TB:2 * TB])
                    nc.vector.tensor_add(xT[ot][:, tsl], xT[ot][:, tsl], t[:])

        # ---- release generator pools; open matching-stage pools ----
        gen_ctx.close()
        mtch = ctx.enter_context(tc.tile_pool(name="mtch", bufs=1))
        m2p = ctx.enter_context(tc.tile_pool(name="m2p", bufs=2))
        mrow = ctx.enter_context(tc.tile_pool(name="mrow", bufs=1))
        mbcp = ctx.enter_context(tc.tile_pool(name="mbc", bufs=1))
        mov_p = ctx.enter_context(tc.tile_pool(name="mov", bufs=3))
        wT_p = ctx.enter_context(tc.tile_pool(name="wTp", bufs=8))

        # ========= output projection (fp32) =========
        outw_sb = io_p.tile([128, D], F32, tag="inout", name="outw")
        for k in range(KD):
            nc.sync.dma_start(outw_sb[:, k * 128:k * 128 + CH],
                              outwT[k * 128:(k + 1) * 128, :])
        outb_col = col(outb, 0, "outbcol")
        outb_bc = mbcp.tile([128, CH], F32, tag="outbbc", name="outbbc")
        nc.gpsimd.dma_start(outb_bc[:], bass.AP(
            tensor=outb.tensor, offset=outb.offset, ap=[[0, 128], [1, CH]]))

        # y_T [ch, tok] in bf16 (stationary source for matching matmuls)
        yT_bf = outp.tile([128, T], BF16, tag="yTbf")
        for b_ in range(NB):
            ps = ps_mm.tile([128, 2 * TB], F32, tag="mm", name="ps")
            for k in range(KD):
                nc.tensor.matmul(ps[:, 0:TB], outw_sb[:, k * 128:k * 128 + CH],
                                 xT[k][:, b_ * TB:(b_ + 1) * TB],
                                 start=(k == 0), stop=(k == KD - 1))
            nc.scalar.activation(yT_bf[:, b_ * TB:(b_ + 1) * TB], ps[:, 0:TB],
                                 AF.Identity, bias=outb_col[:], scale=1.0)

        # y natural -> xf32_d DRAM fp32; bf16 shards straight into ag_in
        for tt in range(T // 128):
            ps = ps_at.tile([128, CH], F32, tag="at", name="yn_ps")
            for k in range(KD):
                nc.tensor.matmul(ps[:], xT[k][:, tt * 128:(tt + 1) * 128],
                                 outw_sb[:, k * 128:k * 128 + CH],
                                 start=(k == 0), stop=(k == KD - 1))
            yn = m2p.tile([128, CH], F32, tag="yn", name="yn")
            nc.vector.tensor_add(yn[:], ps[:], outb_bc[:])
            nc.sync.dma_start(
                xf32_d[tt * 4:(tt + 1) * 4, :].rearrange("p (l c) -> p l c", l=L),
                yn[:])
            ynbf = m2p.tile([128, CH], BF16, tag="ynbf", name="ynbf")
            nc.vector.tensor_copy(ynbf[:], yn[:])
            nc.sync.dma_start(
                ag_in[AG_XFN + tt * 4 * FDIM: AG_XFN + (tt + 1) * 4 * FDIM]
                .rearrange("(i l c) -> i l c", l=L, c=CH),
                ynbf[:])

        # xn = ||xf_i||^2 via gram diag (bf16 inputs, fp32 accum)
        xfT_st = yT_bf[:].rearrange("c (i l) -> c l i", l=L)   # [128, 32, 64]
        gram = ps_at.tile([SC, SC], F32, tag="at", name="gram")
        for l in range(KFl):
            nc.tensor.matmul(gram[:], xfT_st[:, l, :], xfT_st[:, l, :],
                             start=(l == 0), stop=(l == KFl - 1))
        gd = m2p.tile([SC, SC], F32, tag="gd", name="gd")
        nc.vector.tensor_mul(gd[:], gram[:], ident[0:SC, 0:SC])
        xn_col = colp.tile([SC, 1], F32, tag="xncol", name="xncol")
        nc.vector.reduce_sum(xn_col[:], gd[:], axis=AX.X)

        # write AG input: xf_T + xn bits (xf_nat already streamed above)
        # Compact [c, l, i] in SBUF first (strided DMA would explode into
        # 2-byte descriptors).
        for half in range(2):
            xfT_cmp = outp.tile([128, T // 2], BF16, tag="xfTc", name="xfTc")
            for lh in range(L // 2):
                l = half * (L // 2) + lh
                nc.vector.tensor_copy(xfT_cmp[:, lh * SC:(lh + 1) * SC],
                                      xfT_st[:, l, :])
            nc.sync.dma_start(
                bass.AP(tensor=ag_in.tensor,
                        offset=ag_in.offset + AG_XFT + half * (T // 2),
                        ap=[[T, 128], [1, T // 2]]),
                xfT_cmp[:])
        nc.sync.dma_start(
            ag_in[AG_XN:AG_XN + 2 * SC].rearrange("(i bb) -> i bb", bb=2),
            xn_col[:].bitcast(BF16))
        nc.gpsimd.collective_compute(
            "AllGather", ALU.bypass, replica_groups=[list(range(NC_))],
            ins=[ag_in[:]], outs=[ag_out[:]])

        # xn_full row [1, 512] f32 + broadcast
        ago_f32 = ag_out.bitcast(F32)
        xn_row = mrow.tile([1, B], F32, tag="mr", name="xnrow")
        nc.sync.dma_start(
            xn_row[:],
            bass.AP(tensor=ago_f32.tensor, offset=ago_f32.offset + AG_XN // 2,
                    ap=[[1, 1], [AG_SZ // 2, NC_], [1, SC]]))
        xn_bc = mbcp.tile([SC, B], F32, tag="mbc", name="xnbc")
        nc.gpsimd.partition_broadcast(xn_bc[:], xn_row[:])

        # S_pos / S_neg (bf16 matmuls, fp32 accum)
        spos = ps_acc.tile([SC, B], F32, tag="acc", name="spos")
        for l in range(KFl):
            mv = mov_p.tile([128, B], BF16, tag="mv", name="mv")
            nc.sync.dma_start(mv[:], pT[l * 128:(l + 1) * 128, :])
            nc.tensor.matmul(spos[:], xfT_st[:, l, :], mv[:],
                             start=(l == 0), stop=(l == KFl - 1))
        sneg = ps_acc.tile([SC, B], F32, tag="acc", name="sneg")
        for l in range(KFl):
            mv = mov_p.tile([128, B], BF16, tag="mv", name="mv")
            nc.sync.dma_start(
                mv[:],
                bass.AP(tensor=ag_out.tensor, offset=ag_out.offset + AG_XFT + l * SC,
                        ap=[[T, 128], [AG_SZ, NC_], [1, SC]]))
            nc.tensor.matmul(sneg[:], xfT_st[:, l, :], mv[:],
                             start=(l == 0), stop=(l == KFl - 1))

        # distances -> logits -> E (in place)
        dist = mtch.tile([SC, 2 * B], F32, tag="dist")
        nc.vector.scalar_tensor_tensor(dist[:, 0:B], spos[:], -2.0, pn_t[:],
                                       op0=ALU.mult, op1=ALU.add)
        nc.vector.scalar_tensor_tensor(dist[:, B:2 * B], sneg[:], -2.0, xn_bc[:],
                                       op0=ALU.mult, op1=ALU.add)
        nc.vector.tensor_scalar_add(dist[:], dist[:], xn_col[:])
        nc.vector.tensor_scalar_max(dist[:], dist[:], 0.0)
        nc.scalar.activation(dist[:], dist[:], AF.Sqrt, bias=0.0, scale=1.0)
        nc.vector.tensor_add(dist[:, B:2 * B], dist[:, B:2 * B], nd_t[:])
        dmin = colp.tile([SC, 1], F32, tag="dmin", name="dmin")
        nc.vector.tensor_reduce(out=dmin[:], in_=dist[:], axis=AX.X, op=ALU.min)
        E = dist  # in place: E = exp(-d + dmin)
        nc.scalar.activation(E[:], dist[:], AF.Exp, bias=dmin[:], scale=-1.0)
        g_col = colp.tile([SC, 1], F32, tag="gcol", name="gcol")
        nc.scalar.activation(g_col[:], dmin[:], AF.Exp, bias=m20_col[:], scale=-1.0)
        sr_col = colp.tile([SC, 1], F32, tag="srcol", name="srcol")
        nc.vector.reduce_sum(sr_col[:], E[:], axis=AX.X)
        # partial colsums of G = E * g_i via g-weighted stationary
        cs_row = mrow.tile([1, 2 * B], F32, tag="mr", name="csrow")
        for b_ in range(2):
            ps = ps_at.tile([1, B], F32, tag="at", name="pcs")
            nc.tensor.matmul(ps[:], g_col[:], E[:, b_ * B:(b_ + 1) * B],
                             start=True, stop=True)
            nc.vector.tensor_copy(cs_row[:, b_ * B:(b_ + 1) * B], ps[:])
        nc.sync.dma_start(ar_in, cs_row[:])
        nc.gpsimd.collective_compute(
            "AllReduce", ALU.add, replica_groups=[list(range(NC_))],
            ins=[ar_in[:]], outs=[ar_out[:]])
        cs_g = mrow.tile([1, 2 * B], F32, tag="mr", name="csg")
        nc.sync.dma_start(cs_g[:], ar_out)
        cs_bc = mbcp.tile([SC, 2 * B], F32, tag="csbc", name="csbc")
        nc.gpsimd.partition_broadcast(cs_bc[:], cs_g[:])
        nc.scalar.activation(cs_bc[:], cs_bc[:], AF.Sqrt, bias=0.0, scale=1.0)
        nc.vector.reciprocal(cs_bc[:], cs_bc[:])
        # E' = E * invsqrt(Sc); row scalars BEFORE overwriting E with W
        nc.vector.tensor_mul(E[:], E[:], cs_bc[:])
        snp = colp.tile([SC, 1], F32, tag="snp", name="snp")
        nc.vector.reduce_sum(snp[:], E[:, B:2 * B], axis=AX.X)
        spp = colp.tile([SC, 1], F32, tag="spp", name="spp")
        nc.vector.reduce_sum(spp[:], E[:, 0:B], axis=AX.X)
        tcol = colp.tile([SC, 1], F32, tag="tcol", name="tcol")
        nc.vector.reciprocal(tcol[:], sr_col[:])
        nc.vector.tensor_mul(tcol[:], tcol[:], g_col[:])
        ccol = colp.tile([SC, 1], F32, tag="ccol", name="ccol")
        nc.scalar.activation(ccol[:], tcol[:], AF.Sqrt, bias=0.0, scale=1.0)
        alpha = colp.tile([SC, 1], F32, tag="alpha", name="alpha")
        nc.vector.tensor_mul(alpha[:], tcol[:], snp[:])
        beta = colp.tile([SC, 1], F32, tag="beta", name="beta")
        nc.vector.tensor_mul(beta[:], alpha[:], spp[:])
        nc.vector.tensor_mul(beta[:], beta[:], ccol[:])
        nc.vector.tensor_scalar_mul(beta[:], beta[:], -1.0)
        # W = E' * alpha / -beta (in place), transpose, cast bf16
        nc.vector.tensor_scalar_mul(E[:, 0:B], E[:, 0:B], alpha[:])
        nc.vector.tensor_scalar_mul(E[:, B:2 * B], E[:, B:2 * B], beta[:])
        wT = []
        for half in range(2):
            for jt in range(4):
                tp = ps_at.tile([128, SC], F32, tag="at", name="wtp")
                nc.tensor.transpose(
                    tp[:], E[:, half * B + jt * 128: half * B + (jt + 1) * 128],
                    ident[0:SC, 0:SC])
                t = wT_p.tile([128, SC], BF16, tag="wT", name="wT")
                nc.vector.tensor_copy(t[:], tp[:])
                wT.append(t)
        # V and loss: V = Wpos @ p - Wneg @ xf_full, r = xf - fl(xf + V)
        lacc = m2p.tile([SC, 16], F32, tag="lacc", name="lacc", bufs=1)
        FBW = 256
        for fb in range(FDIM // FBW):
            vps = ps_acc.tile([SC, FBW], F32, tag="acc", name="vps")
            for jt in range(4):
                mv = mov_p.tile([128, FBW], BF16, tag="mv", name="mv")
                nc.sync.dma_start(mv[:], pnat[jt * 128:(jt + 1) * 128,
                                              fb * FBW:(fb + 1) * FBW])
                nc.tensor.matmul(vps[:], wT[jt][:], mv[:], start=(jt == 0), stop=False)
            for jt in range(4):
                mv = mov_p.tile([128, FBW], BF16, tag="mv", name="mv")
                nc.sync.dma_start(
                    mv[:],
                    bass.AP(tensor=ag_out.tensor,
                            offset=ag_out.offset + AG_XFN + 2 * jt * AG_SZ + fb * FBW,
                            ap=[[AG_SZ, 2], [FDIM, SC], [1, FBW]]))
                nc.tensor.matmul(vps[:], wT[4 + jt][:], mv[:], start=False, stop=(jt == 3))
            xfb = m2p.tile([SC, FBW], F32, tag="xfb", name="xfb")
            nc.sync.dma_start(xfb[:], xf32_d[:, fb * FBW:(fb + 1) * FBW])
            t1 = m2p.tile([SC, FBW], F32, tag="t1", name="t1")
            nc.vector.tensor_add(t1[:], xfb[:], vps[:])
            nc.vector.tensor_sub(t1[:], xfb[:], t1[:])
            nc.vector.tensor_mul(t1[:], t1[:], t1[:])
            nc.vector.reduce_sum(lacc[:, fb:fb + 1], t1[:], axis=AX.X)
        lsum = colp.tile([SC, 1], F32, tag="lsum", name="lsum")
        nc.vector.reduce_sum(lsum[:], lacc[:], axis=AX.X)
        tot = ps_at.tile([1, 1], F32, tag="at", name="tot")
        nc.tensor.matmul(tot[:], ones_col[0:SC, :], lsum[:], start=True, stop=True)
        tot_sb = colp.tile([1, 1], F32, tag="tot", name="totsb")
        nc.vector.tensor_copy(tot_sb[:], tot[:])
        nc.sync.dma_start(loss_part, tot_sb[:])

    nc.compile()
    return nc


_NC_CACHE = None


def _get_nc():
    global _NC_CACHE
    if _NC_CACHE is None:
        _NC_CACHE = _build_nc()
    return _NC_CACHE


def _round_f32r(a):
    """Round fp32 array to fp32r (keep top 12 mantissa bits, RNE)."""
    u = np.ascontiguousarray(a, dtype=np.float32).view(np.uint32)
    r = (u + 0x7FF + ((u >> 12) & 1)) & 0xFFFFF000
    return r.view(np.float32)


def _split_hi_lo(w):
    """w (fp32) -> (hi fp32r-rounded fp32, lo bf16 of residual)."""
    w = np.ascontiguousarray(w, dtype=np.float32)
    hi = _round_f32r(w)
    lo = (w - hi).astype(ml_dtypes.bfloat16)
    return hi, np.ascontiguousarray(lo)


def _prep_inputs(inputs):
    f32 = lambda x: np.ascontiguousarray(np.asarray(x), dtype=np.float32)
    bf = lambda x: np.ascontiguousarray(np.asarray(x, dtype=ml_dtypes.bfloat16))
    sample_p = f32(inputs["sample_p"])
    eps = f32(inputs["eps"])
    p2 = sample_p.reshape(B, FDIM)
    pn = (p2.astype(np.float64) ** 2).sum(-1).astype(np.float32)

    wqkvT = np.ascontiguousarray(f32(inputs["Wqkv"]).transpose(0, 2, 1))
    woT = np.ascontiguousarray(f32(inputs["Wo"]).transpose(0, 2, 1))
    w1T = np.ascontiguousarray(f32(inputs["W1"]).transpose(0, 2, 1))
    w2T = np.ascontiguousarray(f32(inputs["W2"]).transpose(0, 2, 1))
    wqkv_h, wqkv_l = _split_hi_lo(wqkvT)
    wo_h, wo_l = _split_hi_lo(woT)
    w1_h, w1_l = _split_hi_lo(w1T)
    w2_h, w2_l = _split_hi_lo(w2T)

    common = {
        "inwT": f32(inputs["in_w"]).T.copy(),
        "inb": f32(inputs["in_b"]),
        "wqkvT_h": wqkv_h, "wqkvT_l": wqkv_l,
        "bqkv": f32(inputs["bqkv"]),
        "woT_h": wo_h, "woT_l": wo_l,
        "bo": f32(inputs["bo"]),
        "ln1g": f32(inputs["ln1_g"]), "ln1b": f32(inputs["ln1_b"]),
        "w1T_h": w1_h, "w1T_l": w1_l,
        "b1": f32(inputs["b1"]),
        "w2T_h": w2_h, "w2T_l": w2_l,
        "b2": f32(inputs["b2"]),
        "ln2g": f32(inputs["ln2_g"]), "ln2b": f32(inputs["ln2_b"]),
        "outwT": f32(inputs["out_w"]).T.copy(),
        "outb": f32(inputs["out_b"]),
        "pT": bf(p2.T),
        "pnat": bf(p2),
        "pn_bc": np.broadcast_to(pn[None, :], (SC, B)).copy(),
        "attn_mask": np.kron(np.eye(4, dtype=np.float32), np.ones((32, 32), np.float32)),
    }
    in_maps = []
    for c in range(NC_):
        nd = np.zeros((SC, B), np.float32)
        nd[np.arange(SC), SC * c + np.arange(SC)] = 1e6
        m = dict(common)
        m["epsT"] = eps[c * SC:(c + 1) * SC].reshape(T, CH).T.copy()
        m["negdiag"] = nd
        in_maps.append(m)
    return in_maps


def kernel(**inputs) -> np.ndarray:
    nc = _get_nc()
    in_maps = _prep_inputs(inputs)
    res = run_bass_kernel_spmd(nc, in_maps, list(range(NC_)))
    total = sum(float(r["loss_part"][0, 0]) for r in res.results)
    return np.float32(total / (B * FDIM))


# revision 15
# speedup vs baseline: 1.7815x; 1.0497x over previous
"""Trainium2 Bass kernel for nn_DriftScene_88270167868070.

Contract: kernel(**inputs) takes FULL unsharded inputs (as produced by
setup_inputs()) and returns the FULL output (a scalar np.float32).

Strategy (8 NeuronCores, one SPMD launch):
  - Data-parallel transformer generator over the batch (64 scenes/core).
  - Main matmuls use a 3-term fp32r/bf16 split: W ~ Wh(f32r,20bit) +
    Wl(bf16 of residual); activations h ~ hcat=[f32r(h)|f32r(h-hi)] plus a
    bf16 copy. y = Wh@hcat (one N=512 f32r matmul per k-step, hi and lo
    halves side by side in PSUM) + Wl@h_bf (bf16). Effective input
    precision ~2^-20..2^-24, ~2.6x faster than fp32 on the PE.
  - LN stats (mean/sumsq) via f32r ones-matmuls.
  - Attention kept fp32 (small share of PE time).
  - Matching stage row-sharded, bf16 matmuls; packed AllGather shares
    xf (both layouts) + ||xf||^2; AllReduce for column-softmax sums.
  - loss = mean((xf - fl32(xf + V))^2) with explicit fp32 rounding.
"""

import numpy as np
from contextlib import ExitStack

import concourse.bass as bass
import concourse.tile as tile
from concourse import bacc, mybir
from concourse.bass_utils import run_bass_kernel_spmd
from concourse.masks import make_identity
import ml_dtypes

F32 = mybir.dt.float32
F32R = mybir.dt.float32r
BF16 = mybir.dt.bfloat16
AF = mybir.ActivationFunctionType
ALU = mybir.AluOpType
AX = mybir.AxisListType

# Problem dims (hardcoded per contract)
B, L, CH = 512, 32, 128
D, HEADS, DEPTH, FF = 512, 8, 4, 2048
DH = D // HEADS
LN_EPS = 1e-5
NC_ = 8                 # cores
SC = B // NC_           # 64 scenes per core
T = SC * L              # 2048 tokens per core
TB = 256                # tokens per t-block
NB = T // TB            # 8 t-blocks
NS = TB // 128          # 2 subtiles per block
KD = D // 128           # 4 d-tiles
KF = FF // 128          # 16 ff-tiles
FDIM = L * CH           # 4096 flattened feature dim
KFl = FDIM // 128       # 32 f-tiles
M_SHIFT = -20.0         # global shift for column softmax stabilization

# split AllGather layouts (bf16 element offsets)
AG_XFT = 0                      # ag1: xf_T [c(128), l(32), i(64)] c-major
AG_XN = FDIM * SC               # ag1: xn bits f32 [64,1] as bf16 [64,2]
AG1_SZ = FDIM * SC + 2 * SC     # 262272
AG2_SZ = FDIM * SC              # ag2: xf_nat [64, 4096]


def _build_nc():
    nc = bacc.Bacc("TRN2", target_bir_lowering=False, debug=False, num_devices=NC_)

    # ---------------- I/O ----------------
    def inp(name, shape, dt=F32):
        return nc.dram_tensor(name, shape, dt, kind="ExternalInput").ap()

    epsT = inp("epsT", [128, T])              # eps shard, [ch, tok]
    inwT = inp("inwT", [128, D])              # in_w.T (fp32)
    inb = inp("inb", [D])
    wqkvT_h = inp("wqkvT_h", [DEPTH, D, 3 * D], F32R)
    wqkvT_l = inp("wqkvT_l", [DEPTH, D, 3 * D], BF16)
    bqkv = inp("bqkv", [DEPTH, 3 * D])
    woT_h = inp("woT_h", [DEPTH, D, D], F32R)
    woT_l = inp("woT_l", [DEPTH, D, D], BF16)
    bo = inp("bo", [DEPTH, D])
    ln1g = inp("ln1g", [DEPTH, D])
    ln1b = inp("ln1b", [DEPTH, D])
    w1T_h = inp("w1T_h", [DEPTH, D, FF], F32R)
    w1T_l = inp("w1T_l", [DEPTH, D, FF], BF16)
    b1 = inp("b1", [DEPTH, FF])
    w2T_h = inp("w2T_h", [DEPTH, FF, D], F32R)
    w2T_l = inp("w2T_l", [DEPTH, FF, D], BF16)
    b2 = inp("b2", [DEPTH, D])
    ln2g = inp("ln2g", [DEPTH, D])
    ln2b = inp("ln2b", [DEPTH, D])
    outwT = inp("outwT", [D, CH])             # out_w.T (fp32)
    outb = inp("outb", [CH])
    pT = inp("pT", [FDIM, B], BF16)           # sample_p transposed [f, scene]
    pnat = inp("pnat", [B, FDIM], BF16)       # sample_p natural
    pn_bc = inp("pn_bc", [SC, B])             # ||p_j||^2 broadcast rows
    attn_mask = inp("attn_mask", [128, 128])  # 4-scene block-diag 0/1
    negdiag = inp("negdiag", [SC, B])         # 1e6 at (i, SC*core + i)

    loss_part = nc.dram_tensor("loss_part", [1, 1], F32, kind="ExternalOutput").ap()

    # ---------------- DRAM scratch ----------------
    ag1_in = nc.dram_tensor("ag1_in", [AG1_SZ], BF16).ap()
    ag1_out = nc.dram_tensor("ag1_out", [NC_ * AG1_SZ], BF16, addr_space="Shared").ap()
    ag2_in = nc.dram_tensor("ag2_in", [AG2_SZ], BF16).ap()
    ag2_out = nc.dram_tensor("ag2_out", [NC_ * AG2_SZ], BF16, addr_space="Shared").ap()
    xf32_d = nc.dram_tensor("xf32_d", [SC, FDIM], F32).ap()
    ar_in = nc.dram_tensor("ar_in", [1, 2 * B], F32).ap()
    ar_out = nc.dram_tensor("ar_out", [1, 2 * B], F32, addr_space="Shared").ap()

    with tile.TileContext(nc) as tc, ExitStack() as ctx:
        # ---------------- outer pools (whole kernel; bufs is PER TAG) -----
        const = ctx.enter_context(tc.tile_pool(name="const", bufs=1))
        xTp = ctx.enter_context(tc.tile_pool(name="xT", bufs=1))
        io_p = ctx.enter_context(tc.tile_pool(name="inout", bufs=2))   # inw/outw f32
        colp = ctx.enter_context(tc.tile_pool(name="colp", bufs=1))
        col2p = ctx.enter_context(tc.tile_pool(name="col2p", bufs=2))
        outp = ctx.enter_context(tc.tile_pool(name="outp", bufs=1))

        ps_mm = ctx.enter_context(tc.tile_pool(name="ps_mm", bufs=2, space="PSUM"))
        ps_acc = ctx.enter_context(tc.tile_pool(name="ps_acc", bufs=4, space="PSUM"))
        ps_at = ctx.enter_context(tc.tile_pool(name="ps_at", bufs=2, space="PSUM"))

        # ------------- generator-only pools (released before matching) ----
        gen_ctx = ExitStack()
        hcp = gen_ctx.enter_context(tc.tile_pool(name="hc", bufs=2))      # hcat f32r
        hbp = gen_ctx.enter_context(tc.tile_pool(name="hb", bufs=2))      # h bf16
        hsp = gen_ctx.enter_context(tc.tile_pool(name="hs", bufs=2))      # h f32 scratch
        sqp = gen_ctx.enter_context(tc.tile_pool(name="sq", bufs=3))
        xrp = gen_ctx.enter_context(tc.tile_pool(name="xr", bufs=2))      # x f32r stats
        rowp = gen_ctx.enter_context(tc.tile_pool(name="rows", bufs=2))
        bcp = gen_ctx.enter_context(tc.tile_pool(name="bc", bufs=2))
        bw_p = gen_ctx.enter_context(tc.tile_pool(name="bigw", bufs=KD))   # hi f32r
        bwl_p = gen_ctx.enter_context(tc.tile_pool(name="bigwlo", bufs=KD))  # lo bf16
        wo_p = gen_ctx.enter_context(tc.tile_pool(name="wo", bufs=KD))     # wo hi f32r
        wol_p = gen_ctx.enter_context(tc.tile_pool(name="wolo", bufs=KD))  # wo lo bf16
        w2_p = gen_ctx.enter_context(tc.tile_pool(name="w2", bufs=3))      # w2 hi strm
        w2l_p = gen_ctx.enter_context(tc.tile_pool(name="w2lo", bufs=3))   # w2 lo strm
        qk_p = gen_ctx.enter_context(tc.tile_pool(name="qk", bufs=8))
        v65_p = gen_ctx.enter_context(tc.tile_pool(name="v65", bufs=2))
        e_p = gen_ctx.enter_context(tc.tile_pool(name="et", bufs=2))
        onat_p = gen_ctx.enter_context(tc.tile_pool(name="onat", bufs=2))
        oT_p = gen_ctx.enter_context(tc.tile_pool(name="oT", bufs=4))      # oTcat f32r
        oTb_p = gen_ctx.enter_context(tc.tile_pool(name="oTb", bufs=4))    # oT bf16
        rc_p = gen_ctx.enter_context(tc.tile_pool(name="rlc", bufs=2))     # relu f32r
        rb_p = gen_ctx.enter_context(tc.tile_pool(name="rlb", bufs=2))     # relu bf16
        rs_p = gen_ctx.enter_context(tc.tile_pool(name="rls", bufs=2))     # relu f32

        # ---------------- constants ----------------
        ident = const.tile([128, 128], F32)
        make_identity(nc, ident[:])
        ones_col = const.tile([128, 1], F32)
        nc.vector.memset(ones_col[:], 1.0)
        ones_r = const.tile([128, 1], F32R)
        nc.vector.tensor_copy(ones_r[:], ones_col[:])
        mask_t = const.tile([128, 128], F32)
        nc.sync.dma_start(mask_t[:], attn_mask)
        pn_t = const.tile([SC, B], F32)
        nc.sync.dma_start(pn_t[:], pn_bc)
        nd_t = const.tile([SC, B], F32)
        nc.sync.dma_start(nd_t[:], negdiag)
        eps_col = const.tile([1, 1], F32)
        nc.vector.memset(eps_col[:], LN_EPS)
        m20_col = const.tile([SC, 1], F32)
        nc.vector.memset(m20_col[:], -M_SHIFT)

        # residual stream X_T: KD tiles [128, T] fp32, persistent
        xT = [xTp.tile([128, T], F32, tag=f"xT{k}", name=f"xT{k}") for k in range(KD)]

        def col(ap_1d, base, tag, pool=colp, n=128):
            t = pool.tile([n, 1], F32, tag=tag, name=tag)
            nc.sync.dma_start(t[:], ap_1d[base:base + n])
            return t

        # ========= input projection: X_T = (eps @ in_w.T).T (fp32) =========
        inw_sb = io_p.tile([128, D], F32, tag="inout", name="inw")
        nc.sync.dma_start(inw_sb[:], inwT)
        ib_cols = [col(inb, k * 128, f"bo{k}") for k in range(KD)]
        for b_ in range(NB):
            bsl = slice(b_ * TB, (b_ + 1) * TB)
            eps_blk = sqp.tile([128, TB], F32, tag="eps", name="epsblk", bufs=2)
            nc.sync.dma_start(eps_blk[:], epsT[:, bsl])
            for dt_ in range(KD):
                ps = ps_mm.tile([128, 2 * TB], F32, tag="mm", name="ps")
                nc.tensor.matmul(ps[:, 0:TB], inw_sb[:, dt_ * 128:(dt_ + 1) * 128],
                                 eps_blk[:], start=True, stop=True)
                nc.scalar.activation(xT[dt_][:, bsl], ps[:, 0:TB],
                                     AF.Identity, bias=ib_cols[dt_][:], scale=1.0)

        # ========= per-block layernorm -> hcat/hbf tiles =========
        def ln_block(b_, g_cols, b_cols):
            """LN over partition dim for block b_ -> (hcat f32r, hbf) tiles."""
            bsl = slice(b_ * TB, (b_ + 1) * TB)
            # f32r copies of x for stats matmuls
            xr = []
            for k in range(KD):
                xrt = xrp.tile([128, TB], F32R, tag=f"xr{k % 2}", name="xr")
                nc.vector.tensor_copy(xrt[:], xT[k][:, bsl])
                xr.append(xrt)
            s_row = rowp.tile([1, TB], F32, tag="srow", name="srow")
            ps_s = ps_at.tile([1, TB], F32, tag="at", name="pss")
            for k in range(KD):
                nc.tensor.matmul(ps_s[:], ones_r[:], xr[k][:],
                                 start=(k == 0), stop=(k == KD - 1))
            nc.vector.tensor_scalar_mul(s_row[:], ps_s[:], -1.0 / D)   # -mean
            q_row = rowp.tile([1, TB], F32, tag="qrow", name="qrow")
            ps_q = ps_at.tile([1, TB], F32, tag="at", name="psq")
            for k in range(KD):
                sq = sqp.tile([128, TB], F32R, tag="sq", name="sq")
                nc.vector.tensor_mul(sq[:], xT[k][:, bsl], xT[k][:, bsl])
                nc.tensor.matmul(ps_q[:], ones_r[:], sq[:],
                                 start=(k == 0), stop=(k == KD - 1))
            msq = rowp.tile([1, TB], F32, tag="msq", name="msq")
            nc.vector.tensor_mul(msq[:], s_row[:], s_row[:])
            # var = q/D - m^2  (into q_row)
            nc.vector.scalar_tensor_tensor(q_row[:], ps_q[:], 1.0 / D, msq[:],
                                           op0=ALU.mult, op1=ALU.subtract)
            # rstd = 1/sqrt(var + eps): sqrt into msq, recip into q_row
            nc.scalar.activation(msq[:], q_row[:], AF.Sqrt, bias=eps_col[:], scale=1.0)
            nc.vector.reciprocal(q_row[:], msq[:])
            # shift = -m * rstd (into s_row)
            nc.vector.tensor_mul(s_row[:], s_row[:], q_row[:])
            rstd_bc = bcp.tile([128, TB], F32, tag="rstd_bc", name="rstdbc")
            nc.gpsimd.partition_broadcast(rstd_bc[:], q_row[:])
            shift_bc = bcp.tile([128, TB], F32, tag="shift_bc", name="shiftbc")
            nc.gpsimd.partition_broadcast(shift_bc[:], s_row[:])
            hcs, hbs = [], []
            for k in range(KD):
                hs = hsp.tile([128, TB], F32, tag="hs", name="hs")
                nc.vector.tensor_mul(hs[:], xT[k][:, bsl], rstd_bc[:])
                nc.vector.tensor_add(hs[:], hs[:], shift_bc[:])
                hc = hcp.tile([128, 2 * TB], F32R, tag=f"hc{k}", name=f"hc{k}")
                hb = hbp.tile([128, TB], BF16, tag=f"hb{k}", name=f"hb{k}")
                hf = hsp.tile([128, TB], F32, tag="hf", name="hf")
                # h (f32) and hi (f32r) via two activations; lo = h - hi
                nc.scalar.activation(hf[:], hs[:], AF.Identity,
                                     bias=b_cols[k][:], scale=g_cols[k][:])
                nc.scalar.activation(hc[:, 0:TB], hs[:], AF.Identity,
                                     bias=b_cols[k][:], scale=g_cols[k][:])
                nc.vector.tensor_sub(hc[:, TB:2 * TB], hf[:],
                                     hc[:, 0:TB].bitcast(F32))
                nc.scalar.activation(hb[:], hf[:], AF.Identity, bias=0.0, scale=1.0)
                hcs.append(hc)
                hbs.append(hb)
            return hcs, hbs

        def split_matmul(ps, wh_slices, wl_slices, hcs, hbs, nk):
            """ps[:, 0:TB] + ps[:, TB:2TB] accumulates W^T@h via 3-term split.

            wh_slices[k]: f32r stationary [128,128]; wl_slices[k]: bf16.
            hcs[k]: [128, 2TB] f32r moving; hbs[k]: [128, TB] bf16 moving.
            """
            for k in range(nk):
                nc.tensor.matmul(ps[:, 0:2 * TB], wh_slices[k], hcs[k][:],
                                 start=(k == 0), stop=False)
            for k in range(nk):
                nc.tensor.matmul(ps[:, 0:TB], wl_slices[k], hbs[k][:],
                                 start=False, stop=(k == nk - 1))

        # ========= transformer layers =========
        for li in range(DEPTH):
            # ---- attention phase ----
            g1c = [col(ln1g[li], k * 128, f"lng{k}") for k in range(KD)]
            lb1c = [col(ln1b[li], k * 128, f"lnb{k}") for k in range(KD)]
            wq_h, wq_l = [], []
            for k in range(KD):
                wh = bw_p.tile([128, FF], F32R, tag="bigw", name="wqh")
                nc.sync.dma_start(wh[:, 0:3 * D], wqkvT_h[li, k * 128:(k + 1) * 128, :])
                wq_h.append(wh)
                wl = bwl_p.tile([128, FF], BF16, tag="bigwlo", name="wql")
                nc.sync.dma_start(wl[:, 0:3 * D], wqkvT_l[li, k * 128:(k + 1) * 128, :])
                wq_l.append(wl)
            wo_h, wo_l = [], []
            for k in range(KD):
                wh = wo_p.tile([128, D], F32R, tag="wo", name="woh")
                nc.sync.dma_start(wh[:], woT_h[li, k * 128:(k + 1) * 128, :])
                wo_h.append(wh)
                wl = wol_p.tile([128, D], BF16, tag="wolo", name="wol")
                nc.sync.dma_start(wl[:], woT_l[li, k * 128:(k + 1) * 128, :])
                wo_l.append(wl)
            bq_cols = [col(bqkv[li], ot * 128, f"bq{ot}") for ot in range(8)]
            bv_bc = bcp.tile([128, D], F32, tag="bvbc", name="bvbc", bufs=1)
            nc.gpsimd.dma_start(bv_bc[:], bass.AP(
                tensor=bqkv.tensor, offset=bqkv.offset + li * 3 * D + 2 * D,
                ap=[[0, 128], [1, D]]))
            bo_cols = [col(bo[li], ot * 128, f"bo{ot}") for ot in range(KD)]

            ln_next = ln_block(0, g1c, lb1c)
            for b_ in range(NB):
                tsl = slice(b_ * TB, (b_ + 1) * TB)
                hcs, hbs = ln_next
                if b_ + 1 < NB:
                    ln_next = ln_block(b_ + 1, g1c, lb1c)
                # Q,K projections (transposed out)
                qk = []
                for ot in range(8):
                    ps = ps_mm.tile([128, 2 * TB], F32, tag="mm", name="ps")
                    split_matmul(ps,
                                 [wq_h[k][:, ot * 128:(ot + 1) * 128] for k in range(KD)],
                                 [wq_l[k][:, ot * 128:(ot + 1) * 128] for k in range(KD)],
                                 hcs, hbs, KD)
                    t = qk_p.tile([128, TB], F32, tag="qk", name="qk")
                    nc.scalar.activation(t[:], ps[:, 0:TB], AF.Identity,
                                         bias=bq_cols[ot][:], scale=1.0)
                    nc.vector.tensor_add(t[:], t[:], ps[:, TB:2 * TB])
                    qk.append(t)
                onats = []
                for tt in range(NS):
                    ssl = slice(tt * 128, (tt + 1) * 128)
                    # V natural for this subtile: 3-term with h stationary
                    ps = ps_mm.tile([128, 2 * TB], F32, tag="mm", name="ps")
                    for k in range(KD):
                        nc.tensor.matmul(ps[:, 0:D], hcs[k][:, tt * 128:(tt + 1) * 128],
                                         wq_h[k][:, 2 * D:3 * D],
                                         start=(k == 0), stop=False)
                        nc.tensor.matmul(ps[:, 0:D],
                                         hcs[k][:, TB + tt * 128:TB + (tt + 1) * 128],
                                         wq_h[k][:, 2 * D:3 * D],
                                         start=False, stop=False)
                        nc.tensor.matmul(ps[:, 0:D], hbs[k][:, ssl],
                                         wq_l[k][:, 2 * D:3 * D],
                                         start=False, stop=(k == KD - 1))
                    v = v65_p.tile([128, 8 * 65], F32, tag="v65", name="v65")
                    nc.vector.memset(
                        v[:].rearrange("p (hh c) -> p hh c", hh=8)[:, :, 64:65], 1.0)
                    for hh in range(8):
                        nc.vector.tensor_add(v[:, hh * 65:hh * 65 + 64],
                                             ps[:, hh * 64:(hh + 1) * 64],
                                             bv_bc[:, hh * 64:(hh + 1) * 64])
                    # attention
                    onat = onat_p.tile([128, D], F32, tag="onat", name="onat")
                    for hh in range(8):
                        bp = (hh % 2) * 64
                        kt = qk[4 + hh // 2]
                        qt = qk[hh // 2]
                        s_ps = ps_at.tile([128, 128], F32, tag="at", name="sps")
                        nc.tensor.matmul(s_ps[:], kt[bp:bp + 64, ssl], qt[bp:bp + 64, ssl],
                                         start=True, stop=True)
                        et = e_p.tile([128, 128], F32, tag="et", name="et")
                        nc.scalar.activation(et[:], s_ps[:], AF.Exp, bias=0.0, scale=0.125)
                        nc.vector.tensor_mul(et[:], et[:], mask_t[:])
                        o_ps = ps_at.tile([128, 65], F32, tag="at", name="ops")
                        nc.tensor.matmul(o_ps[:], et[:], v[:, hh * 65:(hh + 1) * 65],
                                         start=True, stop=True)
                        rcol = col2p.tile([128, 1], F32, tag="rcol", name="rcol")
                        nc.vector.reciprocal(rcol[:], o_ps[:, 64:65])
                        nc.vector.tensor_scalar_mul(onat[:, hh * 64:(hh + 1) * 64],
                                                    o_ps[:, 0:64], rcol[:])
                    onats.append(onat)
                # transpose O -> oTcat (f32r hi|lo) + oT bf16
                oTc = [oT_p.tile([128, 2 * TB], F32R, tag="oT", name="oT")
                       for _ in range(KD)]
                oTb = [oTb_p.tile([128, TB], BF16, tag="oTb", name="oTb")
                       for _ in range(KD)]
                for tt in range(NS):
                    csl = slice(tt * 128, (tt + 1) * 128)
                    for k in range(KD):
                        tp = ps_at.tile([128, 128], F32, tag="at", name="tp")
                        nc.tensor.transpose(tp[:], onats[tt][:, k * 128:(k + 1) * 128],
                                            ident[:])
                        nc.vector.tensor_copy(oTc[k][:, csl], tp[:])
                        nc.vector.tensor_sub(oTc[k][:, TB + tt * 128:TB + (tt + 1) * 128],
                                             tp[:], oTc[k][:, csl].bitcast(F32))
                        nc.scalar.activation(oTb[k][:, csl], tp[:], AF.Identity,
                                             bias=0.0, scale=1.0)
                # Wo + residual
                for ot in range(KD):
                    ps = ps_mm.tile([128, 2 * TB], F32, tag="mm", name="ps")
                    split_matmul(ps,
                                 [wo_h[k][:, ot * 128:(ot + 1) * 128] for k in range(KD)],
                                 [wo_l[k][:, ot * 128:(ot + 1) * 128] for k in range(KD)],
                                 oTc, oTb, KD)
                    t = hsp.tile([128, TB], F32, tag="res", name="res")
                    nc.scalar.activation(t[:], ps[:, 0:TB], AF.Identity,
                                         bias=bo_cols[ot][:], scale=1.0)
                    nc.vector.tensor_add(t[:], t[:], ps[:, TB:2 * TB])
                    nc.vector.tensor_add(xT[ot][:, tsl], xT[ot][:, tsl], t[:])

            # ---- FF phase ----
            g2c = [col(ln2g[li], k * 128, f"lng{k}") for k in range(KD)]
            lb2c = [col(ln2b[li], k * 128, f"lnb{k}") for k in range(KD)]
            w1_h, w1_l = [], []
            for k in range(KD):
                wh = bw_p.tile([128, FF], F32R, tag="bigw", name="w1h")
                nc.sync.dma_start(wh[:], w1T_h[li, k * 128:(k + 1) * 128, :])
                w1_h.append(wh)
                wl = bwl_p.tile([128, FF], BF16, tag="bigwlo", name="w1l")
                nc.sync.dma_start(wl[:], w1T_l[li, k * 128:(k + 1) * 128, :])
                w1_l.append(wl)
            bff_cols = [col(b2[li], ot * 128, f"bo{ot}") for ot in range(KD)]
            ln_next = ln_block(0, g2c, lb2c)
            for b_ in range(NB):
                tsl = slice(b_ * TB, (b_ + 1) * TB)
                hcs, hbs = ln_next
                if b_ + 1 < NB:
                    ln_next = ln_block(b_ + 1, g2c, lb2c)
                acc = [ps_acc.tile([128, 2 * TB], F32, tag="acc", name="facc")[:]
                       for _ in range(KD)]
                for kf in range(KF):
                    # stream w2 tiles for this kf
                    w2h = w2_p.tile([128, D], F32R, tag="w2", name="w2h")
                    nc.sync.dma_start(w2h[:], w2T_h[li, kf * 128:(kf + 1) * 128, :])
                    w2l = w2l_p.tile([128, D], BF16, tag="w2lo", name="w2l")
                    nc.sync.dma_start(w2l[:], w2T_l[li, kf * 128:(kf + 1) * 128, :])
                    ps = ps_mm.tile([128, 2 * TB], F32, tag="mm", name="ps")
                    split_matmul(ps,
                                 [w1_h[k][:, kf * 128:(kf + 1) * 128] for k in range(KD)],
                                 [w1_l[k][:, kf * 128:(kf + 1) * 128] for k in range(KD)],
                                 hcs, hbs, KD)
                    b1col = col(b1[li], kf * 128, "b1c", pool=col2p)
                    rs = rs_p.tile([128, TB], F32, tag="rs", name="rs")
                    nc.scalar.activation(rs[:], ps[:, 0:TB], AF.Identity,
                                         bias=b1col[:], scale=1.0)
                    nc.vector.tensor_add(rs[:], rs[:], ps[:, TB:2 * TB])
                    rc = rc_p.tile([128, 2 * TB], F32R, tag="rc", name="rc")
                    rb = rb_p.tile([128, TB], BF16, tag="rb", name="rb")
                    rf = rs_p.tile([128, TB], F32, tag="rf", name="rf")
                    nc.scalar.activation(rf[:], rs[:], AF.Relu, bias=0.0, scale=1.0)
                    nc.scalar.activation(rc[:, 0:TB], rs[:], AF.Relu, bias=0.0, scale=1.0)
                    nc.vector.tensor_sub(rc[:, TB:2 * TB], rf[:],
                                         rc[:, 0:TB].bitcast(F32))
                    nc.scalar.activation(rb[:], rf[:], AF.Identity, bias=0.0, scale=1.0)
                    for ot in range(KD):
                        osl = slice(ot * 128, (ot + 1) * 128)
                        nc.tensor.matmul(acc[ot][:, 0:2 * TB], w2h[:, osl], rc[:],
                                         start=(kf == 0), stop=False)
                        nc.tensor.matmul(acc[ot][:, 0:TB], w2l[:, osl], rb[:],
                                         start=False, stop=(kf == KF - 1))
                for ot in range(KD):
                    t = hsp.tile([128, TB], F32, tag="res", name="res")
                    nc.scalar.activation(t[:], acc[ot][:, 0:TB], AF.Identity,
                                         bias=bff_cols[ot][:], scale=1.0)
                    nc.vector.tensor_add(t[:], t[:], acc[ot][:, TB:2 * TB])
                    nc.vector.tensor_add(xT[ot][:, tsl], xT[ot][:, tsl], t[:])

        # ---- release generator pools; open matching-stage pools ----
        gen_ctx.close()
        mtch = ctx.enter_context(tc.tile_pool(name="mtch", bufs=1))
        m2p = ctx.enter_context(tc.tile_pool(name="m2p", bufs=2))
        mrow = ctx.enter_context(tc.tile_pool(name="mrow", bufs=1))
        mbcp = ctx.enter_context(tc.tile_pool(name="mbc", bufs=1))
        mov_p = ctx.enter_context(tc.tile_pool(name="mov", bufs=16))
        wT_p = ctx.enter_context(tc.tile_pool(name="wTp", bufs=8))

        # ========= output projection (fp32) =========
        outw_sb = io_p.tile([128, D], F32, tag="inout", name="outw")
        for k in range(KD):
            nc.sync.dma_start(outw_sb[:, k * 128:k * 128 + CH],
                              outwT[k * 128:(k + 1) * 128, :])
        outb_col = col(outb, 0, "outbcol")
        outb_bc = mbcp.tile([128, CH], F32, tag="outbbc", name="outbbc")
        nc.gpsimd.dma_start(outb_bc[:], bass.AP(
            tensor=outb.tensor, offset=outb.offset, ap=[[0, 128], [1, CH]]))

        # y_T [ch, tok] in bf16 (stationary source for matching matmuls)
        yT_bf = outp.tile([128, T], BF16, tag="yTbf")
        for b_ in range(NB):
            ps = ps_mm.tile([128, 2 * TB], F32, tag="mm", name="ps")
            for k in range(KD):
                nc.tensor.matmul(ps[:, 0:TB], outw_sb[:, k * 128:k * 128 + CH],
                                 xT[k][:, b_ * TB:(b_ + 1) * TB],
                                 start=(k == 0), stop=(k == KD - 1))
            nc.scalar.activation(yT_bf[:, b_ * TB:(b_ + 1) * TB], ps[:, 0:TB],
                                 AF.Identity, bias=outb_col[:], scale=1.0)

        # xn = ||xf_i||^2 via gram diag (bf16 inputs, fp32 accum)
        xfT_st = yT_bf[:].rearrange("c (i l) -> c l i", l=L)   # [128, 32, 64]
        gram = ps_at.tile([SC, SC], F32, tag="at", name="gram")
        for l in range(KFl):
            nc.tensor.matmul(gram[:], xfT_st[:, l, :], xfT_st[:, l, :],
                             start=(l == 0), stop=(l == KFl - 1))
        gd = m2p.tile([SC, SC], F32, tag="gd", name="gd")
        nc.vector.tensor_mul(gd[:], gram[:], ident[0:SC, 0:SC])
        xn_col = colp.tile([SC, 1], F32, tag="xncol", name="xncol")
        nc.vector.reduce_sum(xn_col[:], gd[:], axis=AX.X)

        # write AG input: xf_T + xn bits (xf_nat already streamed above)
        # Compact [c, l, i] in SBUF first (strided DMA would explode into
        # 2-byte descriptors).
        for half in range(2):
            xfT_cmp = outp.tile([128, T // 2], BF16, tag="xfTc", name="xfTc")
            for lh in range(L // 2):
                l = half * (L // 2) + lh
                nc.vector.tensor_copy(xfT_cmp[:, lh * SC:(lh + 1) * SC],
                                      xfT_st[:, l, :])
            nc.sync.dma_start(
                bass.AP(tensor=ag1_in.tensor,
                        offset=ag1_in.offset + AG_XFT + half * (T // 2),
                        ap=[[T, 128], [1, T // 2]]),
                xfT_cmp[:])
        nc.sync.dma_start(
            ag1_in[AG_XN:AG_XN + 2 * SC].rearrange("(i bb) -> i bb", bb=2),
            xn_col[:].bitcast(BF16))
        nc.gpsimd.collective_compute(
            "AllGather", ALU.bypass, replica_groups=[list(range(NC_))],
            ins=[ag1_in[:]], outs=[ag1_out[:]])
        # S_pos / S_neg (bf16 matmuls, fp32 accum)
        spos = ps_acc.tile([SC, B], F32, tag="acc", name="spos")
        for l in range(KFl):
            mv = mov_p.tile([128, B], BF16, tag="mv", name="mv")
            nc.sync.dma_start(mv[:], pT[l * 128:(l + 1) * 128, :])
            nc.tensor.matmul(spos[:], xfT_st[:, l, :], mv[:],
                             start=(l == 0), stop=(l == KFl - 1))
        # y natural -> xf32_d DRAM fp32; bf16 shards straight into ag_in
        for tt in range(T // 128):
            ps = ps_at.tile([128, CH], F32, tag="at", name="yn_ps")
            for k in range(KD):
                nc.tensor.matmul(ps[:], xT[k][:, tt * 128:(tt + 1) * 128],
                                 outw_sb[:, k * 128:k * 128 + CH],
                                 start=(k == 0), stop=(k == KD - 1))
            yn = m2p.tile([128, CH], F32, tag="yn", name="yn")
            nc.vector.tensor_add(yn[:], ps[:], outb_bc[:])
            nc.sync.dma_start(
                xf32_d[tt * 4:(tt + 1) * 4, :].rearrange("p (l c) -> p l c", l=L),
                yn[:])
            ynbf = m2p.tile([128, CH], BF16, tag="ynbf", name="ynbf")
            nc.vector.tensor_copy(ynbf[:], yn[:])
            nc.sync.dma_start(
                ag2_in[tt * 4 * FDIM:(tt + 1) * 4 * FDIM]
                .rearrange("(i l c) -> i l c", l=L, c=CH),
                ynbf[:])
        nc.gpsimd.collective_compute(
            "AllGather", ALU.bypass, replica_groups=[list(range(NC_))],
            ins=[ag2_in[:]], outs=[ag2_out[:]])
        # xn_full row [1, 512] f32 + broadcast
        ago_f32 = ag1_out.bitcast(F32)
        xn_row = mrow.tile([1, B], F32, tag="mr", name="xnrow")
        nc.sync.dma_start(
            xn_row[:],
            bass.AP(tensor=ago_f32.tensor, offset=ago_f32.offset + AG_XN // 2,
                    ap=[[1, 1], [AG1_SZ // 2, NC_], [1, SC]]))
        xn_bc = mbcp.tile([SC, B], F32, tag="mbc", name="xnbc")
        nc.gpsimd.partition_broadcast(xn_bc[:], xn_row[:])

        sneg = ps_acc.tile([SC, B], F32, tag="acc", name="sneg")
        for l in range(KFl):
            mv = mov_p.tile([128, B], BF16, tag="mv", name="mv")
            nc.sync.dma_start(
                mv[:],
                bass.AP(tensor=ag1_out.tensor, offset=ag1_out.offset + AG_XFT + l * SC,
                        ap=[[T, 128], [AG1_SZ, NC_], [1, SC]]))
            nc.tensor.matmul(sneg[:], xfT_st[:, l, :], mv[:],
                             start=(l == 0), stop=(l == KFl - 1))

        # distances -> logits -> E (in place)
        dist = mtch.tile([SC, 2 * B], F32, tag="dist")
        nc.vector.scalar_tensor_tensor(dist[:, 0:B], spos[:], -2.0, pn_t[:],
                                       op0=ALU.mult, op1=ALU.add)
        nc.vector.scalar_tensor_tensor(dist[:, B:2 * B], sneg[:], -2.0, xn_bc[:],
                                       op0=ALU.mult, op1=ALU.add)
        nc.vector.tensor_scalar_add(dist[:], dist[:], xn_col[:])
        nc.vector.tensor_scalar_max(dist[:], dist[:], 0.0)
        nc.scalar.activation(dist[:], dist[:], AF.Sqrt, bias=0.0, scale=1.0)
        nc.vector.tensor_add(dist[:, B:2 * B], dist[:, B:2 * B], nd_t[:])
        dmin = colp.tile([SC, 1], F32, tag="dmin", name="dmin")
        nc.vector.tensor_reduce(out=dmin[:], in_=dist[:], axis=AX.X, op=ALU.min)
        E = dist  # in place: E = exp(-d + dmin)
        nc.scalar.activation(E[:], dist[:], AF.Exp, bias=dmin[:], scale=-1.0)
        g_col = colp.tile([SC, 1], F32, tag="gcol", name="gcol")
        nc.scalar.activation(g_col[:], dmin[:], AF.Exp, bias=m20_col[:], scale=-1.0)
        sr_col = colp.tile([SC, 1], F32, tag="srcol", name="srcol")
        nc.vector.reduce_sum(sr_col[:], E[:], axis=AX.X)
        # partial colsums of G = E * g_i via g-weighted stationary
        cs_row = mrow.tile([1, 2 * B], F32, tag="mr", name="csrow")
        for b_ in range(2):
            ps = ps_at.tile([1, B], F32, tag="at", name="pcs")
            nc.tensor.matmul(ps[:], g_col[:], E[:, b_ * B:(b_ + 1) * B],
                             start=True, stop=True)
            nc.vector.tensor_copy(cs_row[:, b_ * B:(b_ + 1) * B], ps[:])
        nc.sync.dma_start(ar_in, cs_row[:])
        nc.gpsimd.collective_compute(
            "AllReduce", ALU.add, replica_groups=[list(range(NC_))],
            ins=[ar_in[:]], outs=[ar_out[:]])
        cs_g = mrow.tile([1, 2 * B], F32, tag="mr", name="csg")
        nc.sync.dma_start(cs_g[:], ar_out)
        cs_bc = mbcp.tile([SC, 2 * B], F32, tag="csbc", name="csbc")
        nc.gpsimd.partition_broadcast(cs_bc[:], cs_g[:])
        nc.scalar.activation(cs_bc[:], cs_bc[:], AF.Sqrt, bias=0.0, scale=1.0)
        nc.vector.reciprocal(cs_bc[:], cs_bc[:])
        # E' = E * invsqrt(Sc); row scalars BEFORE overwriting E with W
        nc.vector.tensor_mul(E[:], E[:], cs_bc[:])
        snp = colp.tile([SC, 1], F32, tag="snp", name="snp")
        nc.vector.reduce_sum(snp[:], E[:, B:2 * B], axis=AX.X)
        spp = colp.tile([SC, 1], F32, tag="spp", name="spp")
        nc.vector.reduce_sum(spp[:], E[:, 0:B], axis=AX.X)
        tcol = colp.tile([SC, 1], F32, tag="tcol", name="tcol")
        nc.vector.reciprocal(tcol[:], sr_col[:])
        nc.vector.tensor_mul(tcol[:], tcol[:], g_col[:])
        ccol = colp.tile([SC, 1], F32, tag="ccol", name="ccol")
        nc.scalar.activation(ccol[:], tcol[:], AF.Sqrt, bias=0.0, scale=1.0)
        alpha = colp.tile([SC, 1], F32, tag="alpha", name="alpha")
        nc.vector.tensor_mul(alpha[:], tcol[:], snp[:])
        beta = colp.tile([SC, 1], F32, tag="beta", name="beta")
        nc.vector.tensor_mul(beta[:], alpha[:], spp[:])
        nc.vector.tensor_mul(beta[:], beta[:], ccol[:])
        nc.vector.tensor_scalar_mul(beta[:], beta[:], -1.0)
        # W = E' * alpha / -beta (in place), transpose, cast bf16
        nc.vector.tensor_scalar_mul(E[:, 0:B], E[:, 0:B], alpha[:])
        nc.vector.tensor_scalar_mul(E[:, B:2 * B], E[:, B:2 * B], beta[:])
        wT = []
        for half in range(2):
            for jt in range(4):
                tp = ps_at.tile([128, SC], F32, tag="at", name="wtp")
                nc.tensor.transpose(
                    tp[:], E[:, half * B + jt * 128: half * B + (jt + 1) * 128],
                    ident[0:SC, 0:SC])
                t = wT_p.tile([128, SC], BF16, tag="wT", name="wT")
                nc.vector.tensor_copy(t[:], tp[:])
                wT.append(t)
        # V and loss: V = Wpos @ p - Wneg @ xf_full, r = xf - fl(xf + V)
        lacc = m2p.tile([SC, 16], F32, tag="lacc", name="lacc", bufs=1)
        FBW = 256
        for fb in range(FDIM // FBW):
            vps = ps_acc.tile([SC, FBW], F32, tag="acc", name="vps")
            for jt in range(4):
                mv = mov_p.tile([128, FBW], BF16, tag="mv", name="mv")
                nc.sync.dma_start(mv[:], pnat[jt * 128:(jt + 1) * 128,
                                              fb * FBW:(fb + 1) * FBW])
                nc.tensor.matmul(vps[:], wT[jt][:], mv[:], start=(jt == 0), stop=False)
            for jt in range(4):
                mv = mov_p.tile([128, FBW], BF16, tag="mv", name="mv")
                nc.sync.dma_start(
                    mv[:],
                    bass.AP(tensor=ag2_out.tensor,
                            offset=ag2_out.offset + 2 * jt * AG2_SZ + fb * FBW,
                            ap=[[AG2_SZ, 2], [FDIM, SC], [1, FBW]]))
                nc.tensor.matmul(vps[:], wT[4 + jt][:], mv[:], start=False, stop=(jt == 3))
            xfb = m2p.tile([SC, FBW], F32, tag="xfb", name="xfb")
            nc.sync.dma_start(xfb[:], xf32_d[:, fb * FBW:(fb + 1) * FBW])
            t1 = m2p.tile([SC, FBW], F32, tag="t1", name="t1")
            nc.vector.tensor_add(t1[:], xfb[:], vps[:])
            nc.vector.tensor_sub(t1[:], xfb[:], t1[:])
            nc.vector.tensor_mul(t1[:], t1[:], t1[:])
            nc.vector.reduce_sum(lacc[:, fb:fb + 1], t1[:], axis=AX.X)
        lsum = colp.tile([SC, 1], F32, tag="lsum", name="lsum")
        nc.vector.reduce_sum(lsum[:], lacc[:], axis=AX.X)
        tot = ps_at.tile([1, 1], F32, tag="at", name="tot")
        nc.tensor.matmul(tot[:], ones_col[0:SC, :], lsum[:], start=True, stop=True)
        tot_sb = colp.tile([1, 1], F32, tag="tot", name="totsb")
        nc.vector.tensor_copy(tot_sb[:], tot[:])
        nc.sync.dma_start(loss_part, tot_sb[:])

    nc.compile()
    return nc


_NC_CACHE = None


def _get_nc():
    global _NC_CACHE
    if _NC_CACHE is None:
        _NC_CACHE = _build_nc()
    return _NC_CACHE


def _round_f32r(a):
    """Round fp32 array to fp32r (keep top 12 mantissa bits, RNE)."""
    u = np.ascontiguousarray(a, dtype=np.float32).view(np.uint32)
    r = (u + 0x7FF + ((u >> 12) & 1)) & 0xFFFFF000
    return r.view(np.float32)


def _split_hi_lo(w):
    """w (fp32) -> (hi fp32r-rounded fp32, lo bf16 of residual)."""
    w = np.ascontiguousarray(w, dtype=np.float32)
    hi = _round_f32r(w)
    lo = (w - hi).astype(ml_dtypes.bfloat16)
    return hi, np.ascontiguousarray(lo)


def _prep_inputs(inputs):
    f32 = lambda x: np.ascontiguousarray(np.asarray(x), dtype=np.float32)
    bf = lambda x: np.ascontiguousarray(np.asarray(x, dtype=ml_dtypes.bfloat16))
    sample_p = f32(inputs["sample_p"])
    eps = f32(inputs["eps"])
    p2 = sample_p.reshape(B, FDIM)
    pn = (p2.astype(np.float64) ** 2).sum(-1).astype(np.float32)

    wqkvT = np.ascontiguousarray(f32(inputs["Wqkv"]).transpose(0, 2, 1))
    woT = np.ascontiguousarray(f32(inputs["Wo"]).transpose(0, 2, 1))
    w1T = np.ascontiguousarray(f32(inputs["W1"]).transpose(0, 2, 1))
    w2T = np.ascontiguousarray(f32(inputs["W2"]).transpose(0, 2, 1))
    wqkv_h, wqkv_l = _split_hi_lo(wqkvT)
    wo_h, wo_l = _split_hi_lo(woT)
    w1_h, w1_l = _split_hi_lo(w1T)
    w2_h, w2_l = _split_hi_lo(w2T)

    common = {
        "inwT": f32(inputs["in_w"]).T.copy(),
        "inb": f32(inputs["in_b"]),
        "wqkvT_h": wqkv_h, "wqkvT_l": wqkv_l,
        "bqkv": f32(inputs["bqkv"]),
        "woT_h": wo_h, "woT_l": wo_l,
        "bo": f32(inputs["bo"]),
        "ln1g": f32(inputs["ln1_g"]), "ln1b": f32(inputs["ln1_b"]),
        "w1T_h": w1_h, "w1T_l": w1_l,
        "b1": f32(inputs["b1"]),
        "w2T_h": w2_h, "w2T_l": w2_l,
        "b2": f32(inputs["b2"]),
        "ln2g": f32(inputs["ln2_g"]), "ln2b": f32(inputs["ln2_b"]),
        "outwT": f32(inputs["out_w"]).T.copy(),
        "outb": f32(inputs["out_b"]),
        "pT": bf(p2.T),
        "pnat": bf(p2),
        "pn_bc": np.broadcast_to(pn[None, :], (SC, B)).copy(),
        "attn_mask": np.kron(np.eye(4, dtype=np.float32), np.ones((32, 32), np.float32)),
    }
    in_maps = []
    for c in range(NC_):
        nd = np.zeros((SC, B), np.float32)
        nd[np.arange(SC), SC * c + np.arange(SC)] = 1e6
        m = dict(common)
        m["epsT"] = eps[c * SC:(c + 1) * SC].reshape(T, CH).T.copy()
        m["negdiag"] = nd
        in_maps.append(m)
    return in_maps


def kernel(**inputs) -> np.ndarray:
    nc = _get_nc()
    in_maps = _prep_inputs(inputs)
    res = run_bass_kernel_spmd(nc, in_maps, list(range(NC_)))
    total = sum(float(r["loss_part"][0, 0]) for r in res.results)
    return np.float32(total / (B * FDIM))


# revision 17
# speedup vs baseline: 1.9847x; 1.1141x over previous
"""Trainium2 Bass kernel for nn_DriftScene_88270167868070.

Contract: kernel(**inputs) takes FULL unsharded inputs (as produced by
setup_inputs()) and returns the FULL output (a scalar np.float32).

Strategy (8 NeuronCores, one SPMD launch):
  - Data-parallel transformer generator over the batch (64 scenes/core).
  - Main matmuls use a 3-term fp32r/bf16 split: W ~ Wh(f32r,20bit) +
    Wl(bf16 of residual); activations h ~ hcat=[f32r(h)|f32r(h-hi)] plus a
    bf16 copy. y = Wh@hcat (one N=512 f32r matmul per k-step, hi and lo
    halves side by side in PSUM) + Wl@h_bf (bf16). Effective input
    precision ~2^-20..2^-24, ~2.6x faster than fp32 on the PE.
  - LN stats (mean/sumsq) via f32r ones-matmuls.
  - Attention kept fp32 (small share of PE time).
  - Matching stage row-sharded, bf16 matmuls; packed AllGather shares
    xf (both layouts) + ||xf||^2; AllReduce for column-softmax sums.
  - loss = mean((xf - fl32(xf + V))^2) with explicit fp32 rounding.
"""

import numpy as np
from contextlib import ExitStack

import concourse.bass as bass
import concourse.tile as tile
from concourse import bacc, mybir
from concourse.bass_utils import run_bass_kernel_spmd
from concourse.masks import make_identity
import ml_dtypes

F32 = mybir.dt.float32
F32R = mybir.dt.float32r
BF16 = mybir.dt.bfloat16
AF = mybir.ActivationFunctionType
ALU = mybir.AluOpType
AX = mybir.AxisListType

# Problem dims (hardcoded per contract)
B, L, CH = 512, 32, 128
D, HEADS, DEPTH, FF = 512, 8, 4, 2048
DH = D // HEADS
LN_EPS = 1e-5
NC_ = 8                 # cores
SC = B // NC_           # 64 scenes per core
T = SC * L              # 2048 tokens per core
TB = 256                # tokens per t-block
NB = T // TB            # 8 t-blocks
NS = TB // 128          # 2 subtiles per block
KD = D // 128           # 4 d-tiles
KF = FF // 128          # 16 ff-tiles
FDIM = L * CH           # 4096 flattened feature dim
KFl = FDIM // 128       # 32 f-tiles
M_SHIFT = -20.0         # global shift for column softmax stabilization

# split AllGather layouts (bf16 element offsets)
AG_XFT = 0                      # ag1: xf_T [c(128), l(32), i(64)] c-major
AG_XN = FDIM * SC               # ag1: xn bits f32 [64,1] as bf16 [64,2]
AG1_SZ = FDIM * SC + 2 * SC     # 262272
AG2_SZ = FDIM * SC              # ag2: xf_nat [64, 4096]


def _build_nc():
    nc = bacc.Bacc("TRN2", target_bir_lowering=False, debug=False, num_devices=NC_)

    # ---------------- I/O ----------------
    def inp(name, shape, dt=F32):
        return nc.dram_tensor(name, shape, dt, kind="ExternalInput").ap()

    epsT = inp("epsT", [128, T])              # eps shard, [ch, tok]
    inwT = inp("inwT", [128, D])              # in_w.T (fp32)
    inb = inp("inb", [D])
    wqkvT_h = inp("wqkvT_h", [DEPTH, D, 3 * D], F32R)
    wqkvT_l = inp("wqkvT_l", [DEPTH, D, 3 * D], BF16)
    bqkv = inp("bqkv", [DEPTH, 3 * D])
    woT_h = inp("woT_h", [DEPTH, D, D], F32R)
    woT_l = inp("woT_l", [DEPTH, D, D], BF16)
    bo = inp("bo", [DEPTH, D])
    ln1g = inp("ln1g", [DEPTH, D])
    ln1b = inp("ln1b", [DEPTH, D])
    w1T_h = inp("w1T_h", [DEPTH, D, FF], F32R)
    w1T_l = inp("w1T_l", [DEPTH, D, FF], BF16)
    b1 = inp("b1", [DEPTH, FF])
    w2T_h = inp("w2T_h", [DEPTH, FF, D], F32R)
    w2T_l = inp("w2T_l", [DEPTH, FF, D], BF16)
    b2 = inp("b2", [DEPTH, D])
    ln2g = inp("ln2g", [DEPTH, D])
    ln2b = inp("ln2b", [DEPTH, D])
    outwT = inp("outwT", [D, CH])             # out_w.T (fp32)
    outb = inp("outb", [CH])
    pT = inp("pT", [FDIM, B], BF16)           # sample_p transposed [f, scene]
    pnat = inp("pnat", [B, FDIM], BF16)       # sample_p natural
    pn_bc = inp("pn_bc", [SC, B])             # ||p_j||^2 broadcast rows
    attn_mask = inp("attn_mask", [128, 128])  # 4-scene block-diag 0/1
    negdiag = inp("negdiag", [SC, B])         # 1e6 at (i, SC*core + i)

    loss_part = nc.dram_tensor("loss_part", [1, 1], F32, kind="ExternalOutput").ap()

    # ---------------- DRAM scratch ----------------
    ag1_in = nc.dram_tensor("ag1_in", [AG1_SZ], BF16).ap()
    ag1_out = nc.dram_tensor("ag1_out", [NC_ * AG1_SZ], BF16, addr_space="Shared").ap()
    ag2_in = nc.dram_tensor("ag2_in", [AG2_SZ], BF16).ap()
    ag2_out = nc.dram_tensor("ag2_out", [NC_ * AG2_SZ], BF16, addr_space="Shared").ap()
    xf32_d = nc.dram_tensor("xf32_d", [SC, FDIM], F32).ap()
    ar_in = nc.dram_tensor("ar_in", [1, 2 * B], F32).ap()
    ar_out = nc.dram_tensor("ar_out", [1, 2 * B], F32, addr_space="Shared").ap()

    with tile.TileContext(nc) as tc, ExitStack() as ctx:
        # ---------------- outer pools (whole kernel; bufs is PER TAG) -----
        const = ctx.enter_context(tc.tile_pool(name="const", bufs=1))
        xTp = ctx.enter_context(tc.tile_pool(name="xT", bufs=1))
        io_p = ctx.enter_context(tc.tile_pool(name="inout", bufs=2))   # inw/outw f32
        colp = ctx.enter_context(tc.tile_pool(name="colp", bufs=1))
        col2p = ctx.enter_context(tc.tile_pool(name="col2p", bufs=2))
        outp = ctx.enter_context(tc.tile_pool(name="outp", bufs=1))

        ps_mm = ctx.enter_context(tc.tile_pool(name="ps_mm", bufs=2, space="PSUM"))
        ps_acc = ctx.enter_context(tc.tile_pool(name="ps_acc", bufs=4, space="PSUM"))
        ps_at = ctx.enter_context(tc.tile_pool(name="ps_at", bufs=2, space="PSUM"))

        # ------------- generator-only pools (released before matching) ----
        gen_ctx = ExitStack()
        hcp = gen_ctx.enter_context(tc.tile_pool(name="hc", bufs=2))      # hcat f32r
        hbp = gen_ctx.enter_context(tc.tile_pool(name="hb", bufs=2))      # h bf16
        hsp = gen_ctx.enter_context(tc.tile_pool(name="hs", bufs=2))      # h f32 scratch
        sqp = gen_ctx.enter_context(tc.tile_pool(name="sq", bufs=3))
        xrp = gen_ctx.enter_context(tc.tile_pool(name="xr", bufs=2))      # x f32r stats
        rowp = gen_ctx.enter_context(tc.tile_pool(name="rows", bufs=2))
        bcp = gen_ctx.enter_context(tc.tile_pool(name="bc", bufs=2))
        bw_p = gen_ctx.enter_context(tc.tile_pool(name="bigw", bufs=KD))   # hi f32r
        bwl_p = gen_ctx.enter_context(tc.tile_pool(name="bigwlo", bufs=KD))  # lo bf16
        wo_p = gen_ctx.enter_context(tc.tile_pool(name="wo", bufs=KD))     # wo hi f32r
        wol_p = gen_ctx.enter_context(tc.tile_pool(name="wolo", bufs=KD))  # wo lo bf16
        w2_p = gen_ctx.enter_context(tc.tile_pool(name="w2", bufs=3))      # w2 hi strm
        w2l_p = gen_ctx.enter_context(tc.tile_pool(name="w2lo", bufs=3))   # w2 lo strm
        qk_p = gen_ctx.enter_context(tc.tile_pool(name="qk", bufs=8))
        v65_p = gen_ctx.enter_context(tc.tile_pool(name="v65", bufs=2))
        e_p = gen_ctx.enter_context(tc.tile_pool(name="et", bufs=2))
        onat_p = gen_ctx.enter_context(tc.tile_pool(name="onat", bufs=2))
        oT_p = gen_ctx.enter_context(tc.tile_pool(name="oT", bufs=4))      # oTcat f32r
        oTb_p = gen_ctx.enter_context(tc.tile_pool(name="oTb", bufs=4))    # oT bf16
        rc_p = gen_ctx.enter_context(tc.tile_pool(name="rlc", bufs=2))     # relu f32r
        rb_p = gen_ctx.enter_context(tc.tile_pool(name="rlb", bufs=2))     # relu bf16
        rs_p = gen_ctx.enter_context(tc.tile_pool(name="rls", bufs=2))     # relu f32

        # ---------------- constants ----------------
        ident = const.tile([128, 128], F32)
        make_identity(nc, ident[:])
        ones_col = const.tile([128, 1], F32)
        nc.vector.memset(ones_col[:], 1.0)
        ones_r = const.tile([128, 1], F32R)
        nc.vector.tensor_copy(ones_r[:], ones_col[:])
        mask_t = const.tile([128, 128], F32)
        nc.sync.dma_start(mask_t[:], attn_mask)
        pn_t = const.tile([SC, B], F32)
        nc.sync.dma_start(pn_t[:], pn_bc)
        nd_t = const.tile([SC, B], F32)
        nc.sync.dma_start(nd_t[:], negdiag)
        eps_col = const.tile([1, 1], F32)
        nc.vector.memset(eps_col[:], LN_EPS)
        m20_col = const.tile([SC, 1], F32)
        nc.vector.memset(m20_col[:], -M_SHIFT)

        # residual stream X_T: KD tiles [128, T] fp32, persistent
        xT = [xTp.tile([128, T], F32, tag=f"xT{k}", name=f"xT{k}") for k in range(KD)]

        def col(ap_1d, base, tag, pool=colp, n=128):
            t = pool.tile([n, 1], F32, tag=tag, name=tag)
            nc.sync.dma_start(t[:], ap_1d[base:base + n])
            return t

        # ========= input projection: X_T = (eps @ in_w.T).T (fp32) =========
        inw_sb = io_p.tile([128, D], F32, tag="inout", name="inw")
        nc.sync.dma_start(inw_sb[:], inwT)
        ib_cols = [col(inb, k * 128, f"bo{k}") for k in range(KD)]
        for b_ in range(NB):
            bsl = slice(b_ * TB, (b_ + 1) * TB)
            eps_blk = sqp.tile([128, TB], F32, tag="eps", name="epsblk", bufs=2)
            nc.sync.dma_start(eps_blk[:], epsT[:, bsl])
            for dt_ in range(KD):
                ps = ps_mm.tile([128, 2 * TB], F32, tag="mm", name="ps")
                nc.tensor.matmul(ps[:, 0:TB], inw_sb[:, dt_ * 128:(dt_ + 1) * 128],
                                 eps_blk[:], start=True, stop=True)
                nc.scalar.activation(xT[dt_][:, bsl], ps[:, 0:TB],
                                     AF.Identity, bias=ib_cols[dt_][:], scale=1.0)

        # ========= per-block layernorm -> hcat/hbf tiles =========
        def ln_block(b_, g_cols, b_cols):
            """LN over partition dim for block b_ -> (hcat f32r, hbf) tiles."""
            bsl = slice(b_ * TB, (b_ + 1) * TB)
            # f32r copies of x for stats matmuls
            xr = []
            for k in range(KD):
                xrt = xrp.tile([128, TB], F32R, tag=f"xr{k % 2}", name="xr")
                nc.vector.tensor_copy(xrt[:], xT[k][:, bsl])
                xr.append(xrt)
            s_row = rowp.tile([1, TB], F32, tag="srow", name="srow")
            ps_s = ps_at.tile([1, TB], F32, tag="at", name="pss")
            for k in range(KD):
                nc.tensor.matmul(ps_s[:], ones_r[:], xr[k][:],
                                 start=(k == 0), stop=(k == KD - 1))
            nc.vector.tensor_scalar_mul(s_row[:], ps_s[:], -1.0 / D)   # -mean
            q_row = rowp.tile([1, TB], F32, tag="qrow", name="qrow")
            ps_q = ps_at.tile([1, TB], F32, tag="at", name="psq")
            for k in range(KD):
                sq = sqp.tile([128, TB], F32R, tag="sq", name="sq")
                nc.vector.tensor_mul(sq[:], xT[k][:, bsl], xT[k][:, bsl])
                nc.tensor.matmul(ps_q[:], ones_r[:], sq[:],
                                 start=(k == 0), stop=(k == KD - 1))
            msq = rowp.tile([1, TB], F32, tag="msq", name="msq")
            nc.vector.tensor_mul(msq[:], s_row[:], s_row[:])
            # var = q/D - m^2  (into q_row)
            nc.vector.scalar_tensor_tensor(q_row[:], ps_q[:], 1.0 / D, msq[:],
                                           op0=ALU.mult, op1=ALU.subtract)
            # rstd = 1/sqrt(var + eps): sqrt into msq, recip into q_row
            nc.scalar.activation(msq[:], q_row[:], AF.Sqrt, bias=eps_col[:], scale=1.0)
            nc.vector.reciprocal(q_row[:], msq[:])
            # shift = -m * rstd (into s_row)
            nc.vector.tensor_mul(s_row[:], s_row[:], q_row[:])
            rstd_bc = bcp.tile([128, TB], F32, tag="rstd_bc", name="rstdbc")
            nc.gpsimd.partition_broadcast(rstd_bc[:], q_row[:])
            shift_bc = bcp.tile([128, TB], F32, tag="shift_bc", name="shiftbc")
            nc.gpsimd.partition_broadcast(shift_bc[:], s_row[:])
            hcs, hbs = [], []
            for k in range(KD):
                hs = hsp.tile([128, TB], F32, tag="hs", name="hs")
                nc.vector.tensor_mul(hs[:], xT[k][:, bsl], rstd_bc[:])
                nc.vector.tensor_add(hs[:], hs[:], shift_bc[:])
                hc = hcp.tile([128, 2 * TB], F32R, tag=f"hc{k}", name=f"hc{k}")
                hb = hbp.tile([128, TB], BF16, tag=f"hb{k}", name=f"hb{k}")
                hf = hsp.tile([128, TB], F32, tag="hf", name="hf")
                # h (f32) and hi (f32r) via two activations; lo = h - hi
                nc.scalar.activation(hf[:], hs[:], AF.Identity,
                                     bias=b_cols[k][:], scale=g_cols[k][:])
                nc.scalar.activation(hc[:, 0:TB], hs[:], AF.Identity,
                                     bias=b_cols[k][:], scale=g_cols[k][:])
                nc.vector.tensor_sub(hc[:, TB:2 * TB], hf[:],
                                     hc[:, 0:TB].bitcast(F32))
                nc.scalar.activation(hb[:], hf[:], AF.Identity, bias=0.0, scale=1.0)
                hcs.append(hc)
                hbs.append(hb)
            return hcs, hbs

        def split_matmul(ps, wh_slices, wl_slices, hcs, hbs, nk):
            """ps[:, 0:TB] + ps[:, TB:2TB] accumulates W^T@h via 3-term split.

            wh_slices[k]: f32r stationary [128,128]; wl_slices[k]: bf16.
            hcs[k]: [128, 2TB] f32r moving; hbs[k]: [128, TB] bf16 moving.
            """
            for k in range(nk):
                nc.tensor.matmul(ps[:, 0:2 * TB], wh_slices[k], hcs[k][:],
                                 start=(k == 0), stop=False)
            for k in range(nk):
                nc.tensor.matmul(ps[:, 0:TB], wl_slices[k], hbs[k][:],
                                 start=False, stop=(k == nk - 1))

        # ========= transformer layers =========
        for li in range(DEPTH):
            # ---- attention phase ----
            g1c = [col(ln1g[li], k * 128, f"lng{k}") for k in range(KD)]
            lb1c = [col(ln1b[li], k * 128, f"lnb{k}") for k in range(KD)]
            wq_h, wq_l = [], []
            for k in range(KD):
                wh = bw_p.tile([128, FF], F32R, tag="bigw", name="wqh")
                nc.sync.dma_start(wh[:, 0:3 * D], wqkvT_h[li, k * 128:(k + 1) * 128, :])
                wq_h.append(wh)
                wl = bwl_p.tile([128, FF], BF16, tag="bigwlo", name="wql")
                nc.sync.dma_start(wl[:, 0:3 * D], wqkvT_l[li, k * 128:(k + 1) * 128, :])
                wq_l.append(wl)
            wo_h, wo_l = [], []
            for k in range(KD):
                wh = wo_p.tile([128, D], F32R, tag="wo", name="woh")
                nc.sync.dma_start(wh[:], woT_h[li, k * 128:(k + 1) * 128, :])
                wo_h.append(wh)
                wl = wol_p.tile([128, D], BF16, tag="wolo", name="wol")
                nc.sync.dma_start(wl[:], woT_l[li, k * 128:(k + 1) * 128, :])
                wo_l.append(wl)
            bq_cols = [col(bqkv[li], ot * 128, f"bq{ot}") for ot in range(8)]
            bv_bc = bcp.tile([128, D], F32, tag="bvbc", name="bvbc", bufs=1)
            nc.gpsimd.dma_start(bv_bc[:], bass.AP(
                tensor=bqkv.tensor, offset=bqkv.offset + li * 3 * D + 2 * D,
                ap=[[0, 128], [1, D]]))
            bo_cols = [col(bo[li], ot * 128, f"bo{ot}") for ot in range(KD)]

            ln_next = ln_block(0, g1c, lb1c)
            for b_ in range(NB):
                tsl = slice(b_ * TB, (b_ + 1) * TB)
                hcs, hbs = ln_next
                if b_ + 1 < NB:
                    ln_next = ln_block(b_ + 1, g1c, lb1c)
                # Q,K projections (transposed out)
                qk = []
                for ot in range(8):
                    ps = ps_mm.tile([128, 2 * TB], F32, tag="mm", name="ps")
                    split_matmul(ps,
                                 [wq_h[k][:, ot * 128:(ot + 1) * 128] for k in range(KD)],
                                 [wq_l[k][:, ot * 128:(ot + 1) * 128] for k in range(KD)],
                                 hcs, hbs, KD)
                    t = qk_p.tile([128, TB], F32, tag="qk", name="qk")
                    nc.scalar.activation(t[:], ps[:, 0:TB], AF.Identity,
                                         bias=bq_cols[ot][:], scale=1.0)
                    nc.vector.tensor_add(t[:], t[:], ps[:, TB:2 * TB])
                    qk.append(t)
                onats = []
                for tt in range(NS):
                    ssl = slice(tt * 128, (tt + 1) * 128)
                    # V natural for this subtile: 3-term with h stationary
                    ps = ps_mm.tile([128, 2 * TB], F32, tag="mm", name="ps")
                    for k in range(KD):
                        nc.tensor.matmul(ps[:, 0:D], hcs[k][:, tt * 128:(tt + 1) * 128],
                                         wq_h[k][:, 2 * D:3 * D],
                                         start=(k == 0), stop=False)
                        nc.tensor.matmul(ps[:, 0:D],
                                         hcs[k][:, TB + tt * 128:TB + (tt + 1) * 128],
                                         wq_h[k][:, 2 * D:3 * D],
                                         start=False, stop=False)
                        nc.tensor.matmul(ps[:, 0:D], hbs[k][:, ssl],
                                         wq_l[k][:, 2 * D:3 * D],
                                         start=False, stop=(k == KD - 1))
                    v = v65_p.tile([128, 8 * 65], F32, tag="v65", name="v65")
                    nc.vector.memset(
                        v[:].rearrange("p (hh c) -> p hh c", hh=8)[:, :, 64:65], 1.0)
                    for hh in range(8):
                        nc.vector.tensor_add(v[:, hh * 65:hh * 65 + 64],
                                             ps[:, hh * 64:(hh + 1) * 64],
                                             bv_bc[:, hh * 64:(hh + 1) * 64])
                    # attention
                    onat = onat_p.tile([128, D], F32, tag="onat", name="onat")
                    for hh in range(8):
                        bp = (hh % 2) * 64
                        kt = qk[4 + hh // 2]
                        qt = qk[hh // 2]
                        s_ps = ps_acc.tile([128, 128], F32, tag="acc", name="sps")
                        nc.tensor.matmul(s_ps[:], kt[bp:bp + 64, ssl], qt[bp:bp + 64, ssl],
                                         start=True, stop=True)
                        et = e_p.tile([128, 128], F32, tag="et", name="et")
                        nc.scalar.activation(et[:], s_ps[:], AF.Exp, bias=0.0, scale=0.125)
                        nc.vector.tensor_mul(et[:], et[:], mask_t[:])
                        o_ps = ps_acc.tile([128, 65], F32, tag="acc", name="ops")
                        nc.tensor.matmul(o_ps[:], et[:], v[:, hh * 65:(hh + 1) * 65],
                                         start=True, stop=True)
                        rcol = col2p.tile([128, 1], F32, tag="rcol", name="rcol")
                        nc.vector.reciprocal(rcol[:], o_ps[:, 64:65])
                        nc.vector.tensor_scalar_mul(onat[:, hh * 64:(hh + 1) * 64],
                                                    o_ps[:, 0:64], rcol[:])
                    onats.append(onat)
                # transpose O -> oTcat (f32r hi|lo) + oT bf16
                oTc = [oT_p.tile([128, 2 * TB], F32R, tag="oT", name="oT")
                       for _ in range(KD)]
                oTb = [oTb_p.tile([128, TB], BF16, tag="oTb", name="oTb")
                       for _ in range(KD)]
                for tt in range(NS):
                    csl = slice(tt * 128, (tt + 1) * 128)
                    for k in range(KD):
                        tp = ps_acc.tile([128, 128], F32, tag="acc", name="tp")
                        nc.tensor.transpose(tp[:], onats[tt][:, k * 128:(k + 1) * 128],
                                            ident[:])
                        nc.vector.tensor_copy(oTc[k][:, csl], tp[:])
                        nc.vector.tensor_sub(oTc[k][:, TB + tt * 128:TB + (tt + 1) * 128],
                                             tp[:], oTc[k][:, csl].bitcast(F32))
                        nc.scalar.activation(oTb[k][:, csl], tp[:], AF.Identity,
                                             bias=0.0, scale=1.0)
                # Wo + residual
                for ot in range(KD):
                    ps = ps_mm.tile([128, 2 * TB], F32, tag="mm", name="ps")
                    split_matmul(ps,
                                 [wo_h[k][:, ot * 128:(ot + 1) * 128] for k in range(KD)],
                                 [wo_l[k][:, ot * 128:(ot + 1) * 128] for k in range(KD)],
                                 oTc, oTb, KD)
                    t = hsp.tile([128, TB], F32, tag="res", name="res")
                    nc.scalar.activation(t[:], ps[:, 0:TB], AF.Identity,
                                         bias=bo_cols[ot][:], scale=1.0)
                    nc.vector.tensor_add(t[:], t[:], ps[:, TB:2 * TB])
                    nc.vector.tensor_add(xT[ot][:, tsl], xT[ot][:, tsl], t[:])

            # ---- FF phase ----
            g2c = [col(ln2g[li], k * 128, f"lng{k}") for k in range(KD)]
            lb2c = [col(ln2b[li], k * 128, f"lnb{k}") for k in range(KD)]
            w1_h, w1_l = [], []
            for k in range(KD):
                wh = bw_p.tile([128, FF], F32R, tag="bigw", name="w1h")
                nc.sync.dma_start(wh[:], w1T_h[li, k * 128:(k + 1) * 128, :])
                w1_h.append(wh)
                wl = bwl_p.tile([128, FF], BF16, tag="bigwlo", name="w1l")
                nc.sync.dma_start(wl[:], w1T_l[li, k * 128:(k + 1) * 128, :])
                w1_l.append(wl)
            bff_cols = [col(b2[li], ot * 128, f"bo{ot}") for ot in range(KD)]
            ln_next = ln_block(0, g2c, lb2c)
            for b_ in range(NB):
                tsl = slice(b_ * TB, (b_ + 1) * TB)
                hcs, hbs = ln_next
                if b_ + 1 < NB:
                    ln_next = ln_block(b_ + 1, g2c, lb2c)
                acc = [ps_acc.tile([128, 2 * TB], F32, tag="acc", name="facc")[:]
                       for _ in range(KD)]
                for kf in range(KF):
                    # stream w2 tiles for this kf
                    w2h = w2_p.tile([128, D], F32R, tag="w2", name="w2h")
                    nc.sync.dma_start(w2h[:], w2T_h[li, kf * 128:(kf + 1) * 128, :])
                    w2l = w2l_p.tile([128, D], BF16, tag="w2lo", name="w2l")
                    nc.sync.dma_start(w2l[:], w2T_l[li, kf * 128:(kf + 1) * 128, :])
                    ps = ps_mm.tile([128, 2 * TB], F32, tag="mm", name="ps")
                    split_matmul(ps,
                                 [w1_h[k][:, kf * 128:(kf + 1) * 128] for k in range(KD)],
                                 [w1_l[k][:, kf * 128:(kf + 1) * 128] for k in range(KD)],
                                 hcs, hbs, KD)
                    b1col = col(b1[li], kf * 128, "b1c", pool=col2p)
                    rs = rs_p.tile([128, TB], F32, tag="rs", name="rs")
                    nc.scalar.activation(rs[:], ps[:, 0:TB], AF.Identity,
                                         bias=b1col[:], scale=1.0)
                    nc.vector.tensor_add(rs[:], rs[:], ps[:, TB:2 * TB])
                    rc = rc_p.tile([128, 2 * TB], F32R, tag="rc", name="rc")
                    rb = rb_p.tile([128, TB], BF16, tag="rb", name="rb")
                    rf = rs_p.tile([128, TB], F32, tag="rf", name="rf")
                    nc.scalar.activation(rf[:], rs[:], AF.Relu, bias=0.0, scale=1.0)
                    nc.scalar.activation(rc[:, 0:TB], rs[:], AF.Relu, bias=0.0, scale=1.0)
                    nc.vector.tensor_sub(rc[:, TB:2 * TB], rf[:],
                                         rc[:, 0:TB].bitcast(F32))
                    nc.scalar.activation(rb[:], rf[:], AF.Identity, bias=0.0, scale=1.0)
                    for ot in range(KD):
                        osl = slice(ot * 128, (ot + 1) * 128)
                        nc.tensor.matmul(acc[ot][:, 0:2 * TB], w2h[:, osl], rc[:],
                                         start=(kf == 0), stop=False)
                        nc.tensor.matmul(acc[ot][:, 0:TB], w2l[:, osl], rb[:],
                                         start=False, stop=(kf == KF - 1))
                for ot in range(KD):
                    t = hsp.tile([128, TB], F32, tag="res", name="res")
                    nc.scalar.activation(t[:], acc[ot][:, 0:TB], AF.Identity,
                                         bias=bff_cols[ot][:], scale=1.0)
                    nc.vector.tensor_add(t[:], t[:], acc[ot][:, TB:2 * TB])
                    nc.vector.tensor_add(xT[ot][:, tsl], xT[ot][:, tsl], t[:])

        # ---- release generator pools; open matching-stage pools ----
        gen_ctx.close()
        mtch = ctx.enter_context(tc.tile_pool(name="mtch", bufs=1))
        m2p = ctx.enter_context(tc.tile_pool(name="m2p", bufs=2))
        mrow = ctx.enter_context(tc.tile_pool(name="mrow", bufs=1))
        mbcp = ctx.enter_context(tc.tile_pool(name="mbc", bufs=1))
        mov_p = ctx.enter_context(tc.tile_pool(name="mov", bufs=16))
        wT_p = ctx.enter_context(tc.tile_pool(name="wTp", bufs=8))

        # ========= output projection (fp32) =========
        outw_sb = io_p.tile([128, D], F32, tag="inout", name="outw")
        for k in range(KD):
            nc.sync.dma_start(outw_sb[:, k * 128:k * 128 + CH],
                              outwT[k * 128:(k + 1) * 128, :])
        outb_col = col(outb, 0, "outbcol")
        outb_bc = mbcp.tile([128, CH], F32, tag="outbbc", name="outbbc")
        nc.gpsimd.dma_start(outb_bc[:], bass.AP(
            tensor=outb.tensor, offset=outb.offset, ap=[[0, 128], [1, CH]]))

        # y_T [ch, tok] in bf16 (stationary source for matching matmuls)
        yT_bf = outp.tile([128, T], BF16, tag="yTbf")
        for b_ in range(NB):
            ps = ps_mm.tile([128, 2 * TB], F32, tag="mm", name="ps")
            for k in range(KD):
                nc.tensor.matmul(ps[:, 0:TB], outw_sb[:, k * 128:k * 128 + CH],
                                 xT[k][:, b_ * TB:(b_ + 1) * TB],
                                 start=(k == 0), stop=(k == KD - 1))
            nc.scalar.activation(yT_bf[:, b_ * TB:(b_ + 1) * TB], ps[:, 0:TB],
                                 AF.Identity, bias=outb_col[:], scale=1.0)

        # xn = ||xf_i||^2 via gram diag (bf16 inputs, fp32 accum)
        xfT_st = yT_bf[:].rearrange("c (i l) -> c l i", l=L)   # [128, 32, 64]
        gram = ps_at.tile([SC, SC], F32, tag="at", name="gram")
        for l in range(KFl):
            nc.tensor.matmul(gram[:], xfT_st[:, l, :], xfT_st[:, l, :],
                             start=(l == 0), stop=(l == KFl - 1))
        gd = m2p.tile([SC, SC], F32, tag="gd", name="gd")
        nc.vector.tensor_mul(gd[:], gram[:], ident[0:SC, 0:SC])
        xn_col = colp.tile([SC, 1], F32, tag="xncol", name="xncol")
        nc.vector.reduce_sum(xn_col[:], gd[:], axis=AX.X)

        # write AG input: xf_T + xn bits (xf_nat already streamed above)
        # Compact [c, l, i] in SBUF first (strided DMA would explode into
        # 2-byte descriptors).
        for half in range(2):
            xfT_cmp = outp.tile([128, T // 2], BF16, tag="xfTc", name="xfTc")
            for lh in range(L // 2):
                l = half * (L // 2) + lh
                nc.vector.tensor_copy(xfT_cmp[:, lh * SC:(lh + 1) * SC],
                                      xfT_st[:, l, :])
            nc.sync.dma_start(
                bass.AP(tensor=ag1_in.tensor,
                        offset=ag1_in.offset + AG_XFT + half * (T // 2),
                        ap=[[T, 128], [1, T // 2]]),
                xfT_cmp[:])
        nc.sync.dma_start(
            ag1_in[AG_XN:AG_XN + 2 * SC].rearrange("(i bb) -> i bb", bb=2),
            xn_col[:].bitcast(BF16))
        nc.gpsimd.collective_compute(
            "AllGather", ALU.bypass, replica_groups=[list(range(NC_))],
            ins=[ag1_in[:]], outs=[ag1_out[:]])
        # S_pos / S_neg (bf16 matmuls, fp32 accum)
        spos = ps_acc.tile([SC, B], F32, tag="acc", name="spos")
        for l in range(KFl):
            mv = mov_p.tile([128, B], BF16, tag="mv", name="mv")
            nc.sync.dma_start(mv[:], pT[l * 128:(l + 1) * 128, :])
            nc.tensor.matmul(spos[:], xfT_st[:, l, :], mv[:],
                             start=(l == 0), stop=(l == KFl - 1))
        # y natural -> xf32_d DRAM fp32; bf16 shards straight into ag_in
        for tt in range(T // 128):
            ps = ps_at.tile([128, CH], F32, tag="at", name="yn_ps")
            for k in range(KD):
                nc.tensor.matmul(ps[:], xT[k][:, tt * 128:(tt + 1) * 128],
                                 outw_sb[:, k * 128:k * 128 + CH],
                                 start=(k == 0), stop=(k == KD - 1))
            yn = m2p.tile([128, CH], F32, tag="yn", name="yn")
            nc.vector.tensor_add(yn[:], ps[:], outb_bc[:])
            nc.sync.dma_start(
                xf32_d[tt * 4:(tt + 1) * 4, :].rearrange("p (l c) -> p l c", l=L),
                yn[:])
            ynbf = m2p.tile([128, CH], BF16, tag="ynbf", name="ynbf")
            nc.vector.tensor_copy(ynbf[:], yn[:])
            nc.sync.dma_start(
                ag2_in[tt * 4 * FDIM:(tt + 1) * 4 * FDIM]
                .rearrange("(i l c) -> i l c", l=L, c=CH),
                ynbf[:])
        nc.gpsimd.collective_compute(
            "AllGather", ALU.bypass, replica_groups=[list(range(NC_))],
            ins=[ag2_in[:]], outs=[ag2_out[:]])
        # xn_full row [1, 512] f32 + broadcast
        ago_f32 = ag1_out.bitcast(F32)
        xn_row = mrow.tile([1, B], F32, tag="mr", name="xnrow")
        nc.sync.dma_start(
            xn_row[:],
            bass.AP(tensor=ago_f32.tensor, offset=ago_f32.offset + AG_XN // 2,
                    ap=[[1, 1], [AG1_SZ // 2, NC_], [1, SC]]))
        xn_bc = mbcp.tile([SC, B], F32, tag="mbc", name="xnbc")
        nc.gpsimd.partition_broadcast(xn_bc[:], xn_row[:])

        sneg = ps_acc.tile([SC, B], F32, tag="acc", name="sneg")
        for l in range(KFl):
            mv = mov_p.tile([128, B], BF16, tag="mv", name="mv")
            nc.sync.dma_start(
                mv[:],
                bass.AP(tensor=ag1_out.tensor, offset=ag1_out.offset + AG_XFT + l * SC,
                        ap=[[T, 128], [AG1_SZ, NC_], [1, SC]]))
            nc.tensor.matmul(sneg[:], xfT_st[:, l, :], mv[:],
                             start=(l == 0), stop=(l == KFl - 1))

        # distances -> logits -> E (in place)
        dist = mtch.tile([SC, 2 * B], F32, tag="dist")
        nc.vector.scalar_tensor_tensor(dist[:, 0:B], spos[:], -2.0, pn_t[:],
                                       op0=ALU.mult, op1=ALU.add)
        nc.vector.scalar_tensor_tensor(dist[:, B:2 * B], sneg[:], -2.0, xn_bc[:],
                                       op0=ALU.mult, op1=ALU.add)
        nc.vector.tensor_scalar_add(dist[:], dist[:], xn_col[:])
        nc.vector.tensor_scalar_max(dist[:], dist[:], 0.0)
        nc.scalar.activation(dist[:], dist[:], AF.Sqrt, bias=0.0, scale=1.0)
        nc.vector.tensor_add(dist[:, B:2 * B], dist[:, B:2 * B], nd_t[:])
        dmin = colp.tile([SC, 1], F32, tag="dmin", name="dmin")
        nc.vector.tensor_reduce(out=dmin[:], in_=dist[:], axis=AX.X, op=ALU.min)
        E = dist  # in place: E = exp(-d + dmin)
        nc.scalar.activation(E[:], dist[:], AF.Exp, bias=dmin[:], scale=-1.0)
        g_col = colp.tile([SC, 1], F32, tag="gcol", name="gcol")
        nc.scalar.activation(g_col[:], dmin[:], AF.Exp, bias=m20_col[:], scale=-1.0)
        sr_col = colp.tile([SC, 1], F32, tag="srcol", name="srcol")
        nc.vector.reduce_sum(sr_col[:], E[:], axis=AX.X)
        # partial colsums of G = E * g_i via g-weighted stationary
        cs_row = mrow.tile([1, 2 * B], F32, tag="mr", name="csrow")
        for b_ in range(2):
            ps = ps_at.tile([1, B], F32, tag="at", name="pcs")
            nc.tensor.matmul(ps[:], g_col[:], E[:, b_ * B:(b_ + 1) * B],
                             start=True, stop=True)
            nc.vector.tensor_copy(cs_row[:, b_ * B:(b_ + 1) * B], ps[:])
        nc.sync.dma_start(ar_in, cs_row[:])
        nc.gpsimd.collective_compute(
            "AllReduce", ALU.add, replica_groups=[list(range(NC_))],
            ins=[ar_in[:]], outs=[ar_out[:]])
        # raw-E transposes run BEFORE the AllReduce completes (fills the wait)
        wTr = []
        for half in range(2):
            for jt in range(4):
                tp = ps_at.tile([128, SC], F32, tag="at", name="wtp")
                nc.tensor.transpose(
                    tp[:], E[:, half * B + jt * 128: half * B + (jt + 1) * 128],
                    ident[0:SC, 0:SC])
                t = wT_p.tile([128, SC], F32, tag="wTr", name="wTr")
                nc.vector.tensor_copy(t[:], tp[:])
                wTr.append(t)
        cs_g = mrow.tile([1, 2 * B], F32, tag="mr", name="csg")
        nc.sync.dma_start(cs_g[:], ar_out)
        cs_bc = mbcp.tile([SC, 2 * B], F32, tag="csbc", name="csbc")
        nc.gpsimd.partition_broadcast(cs_bc[:], cs_g[:])
        nc.scalar.activation(cs_bc[:], cs_bc[:], AF.Sqrt, bias=0.0, scale=1.0)
        nc.vector.reciprocal(cs_bc[:], cs_bc[:])
        # u as columns [128, 8] for wT scaling: load colsum transposed from DRAM
        ucolT = mrow.tile([128, 2 * B // 128], F32, tag="ucolT", name="ucolT")
        nc.sync.dma_start(
            ucolT[:],
            bass.AP(tensor=ar_out.tensor, offset=ar_out.offset,
                    ap=[[1, 128], [128, 2 * B // 128]]))
        nc.scalar.activation(ucolT[:], ucolT[:], AF.Sqrt, bias=0.0, scale=1.0)
        nc.vector.reciprocal(ucolT[:], ucolT[:])
        # E' = E * u (temp; only for the row sums)
        Etmp = mbcp.tile([SC, 2 * B], F32, tag="etmp", name="etmp")
        nc.vector.tensor_mul(Etmp[:], E[:], cs_bc[:])
        snp = colp.tile([SC, 1], F32, tag="snp", name="snp")
        nc.vector.reduce_sum(snp[:], Etmp[:, B:2 * B], axis=AX.X)
        spp = colp.tile([SC, 1], F32, tag="spp", name="spp")
        nc.vector.reduce_sum(spp[:], Etmp[:, 0:B], axis=AX.X)
        tcol = colp.tile([SC, 1], F32, tag="tcol", name="tcol")
        nc.vector.reciprocal(tcol[:], sr_col[:])
        nc.vector.tensor_mul(tcol[:], tcol[:], g_col[:])
        ccol = colp.tile([SC, 1], F32, tag="ccol", name="ccol")
        nc.scalar.activation(ccol[:], tcol[:], AF.Sqrt, bias=0.0, scale=1.0)
        alpha = colp.tile([SC, 1], F32, tag="alpha", name="alpha")
        nc.vector.tensor_mul(alpha[:], tcol[:], snp[:])
        beta = colp.tile([SC, 1], F32, tag="beta", name="beta")
        nc.vector.tensor_mul(beta[:], alpha[:], spp[:])
        nc.vector.tensor_mul(beta[:], beta[:], ccol[:])
        nc.vector.tensor_scalar_mul(beta[:], beta[:], -1.0)
        # wT = raw-E^T scaled by u_j (partition dim), cast bf16
        wT = []
        for idx in range(8):
            t = wT_p.tile([128, SC], BF16, tag="wT", name="wT")
            nc.vector.tensor_scalar_mul(t[:], wTr[idx][:], ucolT[:, idx:idx + 1])
            wT.append(t)
        # V and loss: V = alpha*(WposRaw @ p) + beta*(WnegRaw @ xf_full)
        # (beta carries the minus sign); r = xf - fl(xf + V)
        FBW = 512
        lacc = m2p.tile([SC, FDIM // FBW], F32, tag="lacc", name="lacc", bufs=1)
        for fb in range(FDIM // FBW):
            vpos = ps_acc.tile([SC, FBW], F32, tag="acc", name="vpos")
            for jt in range(4):
                mv = mov_p.tile([128, FBW], BF16, tag="mv", name="mv")
                nc.sync.dma_start(mv[:], pnat[jt * 128:(jt + 1) * 128,
                                              fb * FBW:(fb + 1) * FBW])
                nc.tensor.matmul(vpos[:], wT[jt][:], mv[:],
                                 start=(jt == 0), stop=(jt == 3))
            vneg = ps_acc.tile([SC, FBW], F32, tag="acc", name="vneg")
            for jt in range(4):
                mv = mov_p.tile([128, FBW], BF16, tag="mv", name="mv")
                nc.sync.dma_start(
                    mv[:],
                    bass.AP(tensor=ag2_out.tensor,
                            offset=ag2_out.offset + 2 * jt * AG2_SZ + fb * FBW,
                            ap=[[AG2_SZ, 2], [FDIM, SC], [1, FBW]]))
                nc.tensor.matmul(vneg[:], wT[4 + jt][:], mv[:],
                                 start=(jt == 0), stop=(jt == 3))
            xfb = m2p.tile([SC, FBW], F32, tag="xfb", name="xfb")
            nc.sync.dma_start(xfb[:], xf32_d[:, fb * FBW:(fb + 1) * FBW])
            s1 = m2p.tile([SC, FBW], F32, tag="s1", name="s1")
            nc.vector.tensor_scalar_mul(s1[:], vpos[:], alpha[:])
            nc.vector.scalar_tensor_tensor(s1[:], vneg[:], beta[:], s1[:],
                                           op0=ALU.mult, op1=ALU.add)
            t1 = m2p.tile([SC, FBW], F32, tag="t1", name="t1")
            nc.vector.tensor_add(t1[:], xfb[:], s1[:])
            nc.vector.tensor_sub(t1[:], xfb[:], t1[:])
            nc.vector.tensor_mul(t1[:], t1[:], t1[:])
            nc.vector.reduce_sum(lacc[:, fb:fb + 1], t1[:], axis=AX.X)
        lsum = colp.tile([SC, 1], F32, tag="lsum", name="lsum")
        nc.vector.reduce_sum(lsum[:], lacc[:], axis=AX.X)
        tot = ps_at.tile([1, 1], F32, tag="at", name="tot")
        nc.tensor.matmul(tot[:], ones_col[0:SC, :], lsum[:], start=True, stop=True)
        tot_sb = colp.tile([1, 1], F32, tag="tot", name="totsb")
        nc.vector.tensor_copy(tot_sb[:], tot[:])
        nc.sync.dma_start(loss_part, tot_sb[:])

    nc.compile()
    return nc


_NC_CACHE = None


def _get_nc():
    global _NC_CACHE
    if _NC_CACHE is None:
        _NC_CACHE = _build_nc()
    return _NC_CACHE


def _round_f32r(a):
    """Round fp32 array to fp32r (keep top 12 mantissa bits, RNE)."""
    u = np.ascontiguousarray(a, dtype=np.float32).view(np.uint32)
    r = (u + 0x7FF + ((u >> 12) & 1)) & 0xFFFFF000
    return r.view(np.float32)


def _split_hi_lo(w):
    """w (fp32) -> (hi fp32r-rounded fp32, lo bf16 of residual)."""
    w = np.ascontiguousarray(w, dtype=np.float32)
    hi = _round_f32r(w)
    lo = (w - hi).astype(ml_dtypes.bfloat16)
    return hi, np.ascontiguousarray(lo)


def _prep_inputs(inputs):
    f32 = lambda x: np.ascontiguousarray(np.asarray(x), dtype=np.float32)
    bf = lambda x: np.ascontiguousarray(np.asarray(x, dtype=ml_dtypes.bfloat16))
    sample_p = f32(inputs["sample_p"])
    eps = f32(inputs["eps"])
    p2 = sample_p.reshape(B, FDIM)
    pn = (p2.astype(np.float64) ** 2).sum(-1).astype(np.float32)

    wqkvT = np.ascontiguousarray(f32(inputs["Wqkv"]).transpose(0, 2, 1))
    woT = np.ascontiguousarray(f32(inputs["Wo"]).transpose(0, 2, 1))
    w1T = np.ascontiguousarray(f32(inputs["W1"]).transpose(0, 2, 1))
    w2T = np.ascontiguousarray(f32(inputs["W2"]).transpose(0, 2, 1))
    wqkv_h, wqkv_l = _split_hi_lo(wqkvT)
    wo_h, wo_l = _split_hi_lo(woT)
    w1_h, w1_l = _split_hi_lo(w1T)
    w2_h, w2_l = _split_hi_lo(w2T)

    common = {
        "inwT": f32(inputs["in_w"]).T.copy(),
        "inb": f32(inputs["in_b"]),
        "wqkvT_h": wqkv_h, "wqkvT_l": wqkv_l,
        "bqkv": f32(inputs["bqkv"]),
        "woT_h": wo_h, "woT_l": wo_l,
        "bo": f32(inputs["bo"]),
        "ln1g": f32(inputs["ln1_g"]), "ln1b": f32(inputs["ln1_b"]),
        "w1T_h": w1_h, "w1T_l": w1_l,
        "b1": f32(inputs["b1"]),
        "w2T_h": w2_h, "w2T_l": w2_l,
        "b2": f32(inputs["b2"]),
        "ln2g": f32(inputs["ln2_g"]), "ln2b": f32(inputs["ln2_b"]),
        "outwT": f32(inputs["out_w"]).T.copy(),
        "outb": f32(inputs["out_b"]),
        "pT": bf(p2.T),
        "pnat": bf(p2),
        "pn_bc": np.broadcast_to(pn[None, :], (SC, B)).copy(),
        "attn_mask": np.kron(np.eye(4, dtype=np.float32), np.ones((32, 32), np.float32)),
    }
    in_maps = []
    for c in range(NC_):
        nd = np.zeros((SC, B), np.float32)
        nd[np.arange(SC), SC * c + np.arange(SC)] = 1e6
        m = dict(common)
        m["epsT"] = eps[c * SC:(c + 1) * SC].reshape(T, CH).T.copy()
        m["negdiag"] = nd
        in_maps.append(m)
    return in_maps


def kernel(**inputs) -> np.ndarray:
    nc = _get_nc()
    in_maps = _prep_inputs(inputs)
    res = run_bass_kernel_spmd(nc, in_maps, list(range(NC_)))
    total = sum(float(r["loss_part"][0, 0]) for r in res.results)
    return np.float32(total / (B * FDIM))
